# revision 2
# baseline (speedup 1.0000x reference)
"""Soft-DTW ranking loss kernel for Trainium2 (8 NeuronCores, SPMD data parallel).

Math: loss = mean((diff - labels)^2) where
  diff_b = sdtw(OTH_b,X_b) - sdtw(TGT_b,X_b) - 0.5*sdtw(OTH_b,OTH_b) + 0.5*sdtw(TGT_b,TGT_b)
(the sdtw(X,X) terms of the normalized soft-DTW cancel exactly).

Soft-DTW (gamma=1) is computed in the probability domain:
  E[i,j] = K[i,j] * (E[i-1,j] + E[i-1,j-1] + E[i,j-1]),  K = exp(<xn_i,yn_j> - 1)
which maps one DP row onto a single DVE tensor_tensor_scan:
  state = (s[t] + state) * K[t],   s = E_prev + shift1(E_prev)
with periodic per-instance rescaling (log-scale accumulated in C) to stay in
fp32 range. Each core handles 8 batch items x 4 DTW instances = 32 independent
DPs vectorized across SBUF partitions.
"""

import os
import sys

import numpy as np

for _p in ("/root/.axon_site", "/root/.axon_site/_ro/trn_rl_repo",
           "/root/.axon_site/_ro/pypackages", "/opt/trn_rl_repo", "/opt/pypackages"):
    if os.path.isdir(_p) and _p not in sys.path:
        sys.path.append(_p)

import concourse.bass as bass
import concourse.tile as tile
from concourse.tile import add_dep_helper
from concourse import bacc, mybir
from concourse.bass_utils import run_bass_kernel_spmd
from concourse.masks import make_identity

F32 = mybir.dt.float32
F32R = mybir.dt.float32r
AX = mybir.AxisListType
OP = mybir.AluOpType
AF = mybir.ActivationFunctionType

B, T, D = 64, 512, 64
NCORES = 8
BPC = B // NCORES          # batch items per core
NTYPE = 4                  # (OTH,X), (TGT,X), (OTH,OTH), (TGT,TGT)
RESC = 32                  # rescale cadence (rows)


def _emit(tc: tile.TileContext, ins: dict, outs: dict, kbuf: bass.AP,
          t_len: int, bpc: int, resc: int):
    nc = tc.nc
    ni = NTYPE * bpc
    nrowt = t_len // 128

    with (
        tc.tile_pool(name="const", bufs=1) as p_const,
        tc.tile_pool(name="ain", bufs=2) as p_in,
        tc.tile_pool(name="astat", bufs=2) as p_astat,
        tc.tile_pool(name="asn", bufs=2) as p_asn,
        tc.tile_pool(name="apsT", bufs=2, space="PSUM") as p_psT,
        tc.tile_pool(name="ant", bufs=2) as p_nt,
        tc.tile_pool(name="aG", bufs=2, space="PSUM") as p_G,
        tc.tile_pool(name="aK", bufs=3) as p_K,
        tc.tile_pool(name="bE", bufs=1) as p_E,
        tc.tile_pool(name="bS", bufs=2) as p_s,
        tc.tile_pool(name="bK", bufs=4) as p_k,
        tc.tile_pool(name="bstat", bufs=2) as p_stat,
        tc.tile_pool(name="bacc", bufs=1) as p_acc,
    ):
        ident = p_const.tile([128, 128], F32, tag="ident")
        make_identity(nc, ident[:])
        bias_m1 = p_const.tile([128, 1], F32, tag="biasm1")
        nc.gpsimd.memset(bias_m1[:], -1.0)

        # ---------------- Phase A: K = exp(<xn,yn> - 1) for all pairs -------
        for b in range(bpc):
            nT = {}
            for sname in ("OTH", "TGT", "X"):
                src = ins[sname]
                xin = p_in.tile([128, nrowt * D], F32, tag=f"in_{sname}")
                nc.sync.dma_start(
                    xin[:].rearrange("p (t d) -> p t d", d=D),
                    src[b].rearrange("(t p) d -> p t d", p=128),
                )
                sq = p_astat.tile([128, nrowt * D], F32, tag=f"sq_{sname}")
                ss = p_astat.tile([128, nrowt], F32, tag=f"ss_{sname}")
                for t in range(nrowt):
                    nc.scalar.activation(
                        sq[:, t * D:(t + 1) * D], xin[:, t * D:(t + 1) * D],
                        AF.Square, accum_out=ss[:, t:t + 1],
                    )
                nrm = p_astat.tile([128, nrowt], F32, tag=f"nrm_{sname}")
                nc.scalar.activation(nrm[:], ss[:], AF.Sqrt)
                rnm = p_astat.tile([128, nrowt], F32, tag=f"rnm_{sname}")
                nc.vector.reciprocal(rnm[:], nrm[:])
                sn = p_asn.tile([128, nrowt * D], F32, tag=f"sn_{sname}")
                for t in range(nrowt):
                    nc.vector.tensor_scalar_mul(
                        sn[:, t * D:(t + 1) * D], xin[:, t * D:(t + 1) * D],
                        rnm[:, t:t + 1],
                    )
                snT = p_nt.tile([D, t_len], F32R, tag=f"nt_{sname}")
                for t in range(nrowt):
                    tp = p_psT.tile([D, 128], F32, tag="psT")
                    nc.tensor.transpose(tp[:], sn[:, t * D:(t + 1) * D], ident[:])
                    nc.scalar.copy(snT[:, t * 128:(t + 1) * 128], tp[:])
                nT[sname] = snT

            pairs = [("OTH", "X"), ("TGT", "X"), ("OTH", "OTH"), ("TGT", "TGT")]
            for ptype, (an, cn) in enumerate(pairs):
                inst = ptype * bpc + b
                aT, cT = nT[an], nT[cn]
                for rt in range(nrowt):
                    g = p_G.tile([128, t_len], F32, tag="G")
                    nc.tensor.matmul(
                        g[:],
                        aT[:, rt * 128:(rt + 1) * 128],
                        cT[:],
                        start=True, stop=True,
                    )
                    kt = p_K.tile([128, t_len], F32, tag="K")
                    nc.scalar.activation(kt[:], g[:], AF.Exp, bias=bias_m1[:])
                    nc.sync.dma_start(kbuf[inst, rt * 128:(rt + 1) * 128, :], kt[:])

        # ---------------- Phase B: row-scan DP over all instances -----------
        Ea = p_E.tile([ni, t_len + 1], F32, tag="Ea")
        Eb = p_E.tile([ni, t_len + 1], F32, tag="Eb")
        cacc = p_acc.tile([ni, 1], F32, tag="C")
        nc.gpsimd.memset(Ea[:], 0.0)
        nc.gpsimd.memset(Eb[:], 0.0)
        nc.gpsimd.memset(cacc[:], 0.0)
        nc.gpsimd.memset(Ea[:, 0:1], 1.0)  # E[-1][-1] = exp(-0)

        cur, nxt = Ea, Eb
        for r in range(t_len):
            kt = p_k.tile([ni, t_len], F32, tag="krow")
            nc.sync.dma_start(kt[:], kbuf[:, r, :])
            s = p_s.tile([ni, t_len], F32, tag="s")
            nc.vector.tensor_add(s[:], cur[:, 1:t_len + 1], cur[:, 0:t_len])
            nc.vector.tensor_tensor_scan(
                nxt[:, 1:t_len + 1], s[:], kt[:], 0.0, OP.add, OP.mult,
            )
            if r == 0:
                # E[0][-1] = 0: clear the one-time E[-1][-1] = 1 boundary
                nc.vector.memset(Ea[:, 0:1], 0.0)
            if (r + 1) % resc == 0 and r != t_len - 1:
                mx = p_stat.tile([ni, 1], F32, tag="mx")
                nc.vector.tensor_reduce(mx[:], nxt[:, 1:t_len + 1], AX.X, OP.max)
                rec = p_stat.tile([ni, 1], F32, tag="rec")
                nc.vector.reciprocal(rec[:], mx[:])
                nc.vector.tensor_scalar_mul(nxt[:, 1:t_len + 1],
                                            nxt[:, 1:t_len + 1], rec[:])
                lg = p_stat.tile([ni, 1], F32, tag="lg")
                nc.scalar.activation(lg[:], mx[:], AF.Ln)
                nc.vector.tensor_add(cacc[:], cacc[:], lg[:])
            cur, nxt = nxt, cur

        nc.sync.dma_start(outs["EOUT"].rearrange("(a b) -> a b", b=1),
                          cur[:, t_len:t_len + 1])
        nc.sync.dma_start(outs["COUT"].rearrange("(a b) -> a b", b=1), cacc[:])


def _emit_wave(tc: tile.TileContext, ins: dict, outs: dict, kbuf: bass.AP,
               t_len: int, bpc: int, resc: int):
    """Wavefront DP: CH=t_len/128 column chunks on partition groups.

    Partition p = g*ni + inst handles column chunk g of instance inst.
    Wavefront step w: group g processes row r = w - g (K rows padded with 3
    zero rows on each side so inactive groups compute zeros). Cross-chunk
    carries (scan initial / shifted-row boundary) move between partition
    groups via a constant shift matmul on the (otherwise idle) PE.
    """
    nc = tc.nc
    ni = NTYPE * bpc
    ch = t_len // 128
    npart = ch * ni
    nrowt = ch
    nsteps = t_len + ch - 1

    with (
        tc.tile_pool(name="const", bufs=1) as p_const,
        tc.tile_pool(name="ain", bufs=2) as p_in,
        tc.tile_pool(name="astat", bufs=2) as p_astat,
        tc.tile_pool(name="asn", bufs=2) as p_asn,
        tc.tile_pool(name="apsT", bufs=2, space="PSUM") as p_psT,
        tc.tile_pool(name="ant", bufs=2) as p_nt,
        tc.tile_pool(name="aG", bufs=2, space="PSUM") as p_G,
        tc.tile_pool(name="aK", bufs=3) as p_K,
        tc.tile_pool(name="bE", bufs=1) as p_E,
        tc.tile_pool(name="bS", bufs=2) as p_s,
        tc.tile_pool(name="bK", bufs=8) as p_k,
        tc.tile_pool(name="bC", bufs=3, space="PSUM") as p_carry,
        tc.tile_pool(name="bB", bufs=1, space="PSUM") as p_bc,
        tc.tile_pool(name="bstat", bufs=2) as p_stat,
        tc.tile_pool(name="bacc", bufs=1) as p_acc,
    ):
        ident = p_const.tile([128, 128], F32, tag="ident")
        make_identity(nc, ident[:])
        bias_m1 = p_const.tile([128, 1], F32, tag="biasm1")
        nc.gpsimd.memset(bias_m1[:], -1.0)
        # shiftM[k, p] = 1 iff k == p - ni  (moves group g-1 -> g)
        shiftM = p_const.tile([npart, npart], F32, tag="shiftM")
        nc.gpsimd.memset(shiftM[:], 0.0)
        nc.gpsimd.affine_select(
            out=shiftM[:], in_=shiftM[:], compare_op=OP.not_equal, fill=1.0,
            base=ni, pattern=[[-1, npart]], channel_multiplier=1,
        )
        # bcastM[k, (g, j)] = 1 iff k == j  (broadcast group-0 col to all groups)
        bcastM = p_const.tile([ni, npart], F32, tag="bcastM")
        nc.gpsimd.memset(bcastM[:], 0.0)
        nc.gpsimd.affine_select(
            out=bcastM[:].rearrange("k (g j) -> k g j", j=ni),
            in_=bcastM[:].rearrange("k (g j) -> k g j", j=ni),
            compare_op=OP.not_equal, fill=1.0,
            base=0, pattern=[[0, ch], [-1, ni]], channel_multiplier=1,
        )

        # zero the 3+3 pad rows of kbuf (layout [ni, t_len+6, t_len])
        zpad = p_const.tile([ni, 3 * t_len], F32, tag="zpad")
        nc.gpsimd.memset(zpad[:], 0.0)
        nc.sync.dma_start(
            kbuf[:, 0:3, :].rearrange("i r c -> i (r c)"), zpad[:])
        nc.sync.dma_start(
            kbuf[:, t_len + 3:t_len + 6, :].rearrange("i r c -> i (r c)"), zpad[:])

        # ---------------- Phase A (same as v1, +3 row offset into kbuf) -----
        for b in range(bpc):
            nT = {}
            for sname in ("OTH", "TGT", "X"):
                src = ins[sname]
                xin = p_in.tile([128, nrowt * D], F32, tag=f"in_{sname}")
                nc.sync.dma_start(
                    xin[:].rearrange("p (t d) -> p t d", d=D),
                    src[b].rearrange("(t p) d -> p t d", p=128),
                )
                sq = p_astat.tile([128, nrowt * D], F32, tag=f"sq_{sname}")
                ss = p_astat.tile([128, nrowt], F32, tag=f"ss_{sname}")
                for t in range(nrowt):
                    nc.scalar.activation(
                        sq[:, t * D:(t + 1) * D], xin[:, t * D:(t + 1) * D],
                        AF.Square, accum_out=ss[:, t:t + 1],
                    )
                nrm = p_astat.tile([128, nrowt], F32, tag=f"nrm_{sname}")
                nc.scalar.activation(nrm[:], ss[:], AF.Sqrt)
                rnm = p_astat.tile([128, nrowt], F32, tag=f"rnm_{sname}")
                nc.vector.reciprocal(rnm[:], nrm[:])
                sn = p_asn.tile([128, nrowt * D], F32, tag=f"sn_{sname}")
                for t in range(nrowt):
                    nc.vector.tensor_scalar_mul(
                        sn[:, t * D:(t + 1) * D], xin[:, t * D:(t + 1) * D],
                        rnm[:, t:t + 1],
                    )
                snT = p_nt.tile([D, t_len], F32R, tag=f"nt_{sname}")
                for t in range(nrowt):
                    tp = p_psT.tile([D, 128], F32, tag="psT")
                    nc.tensor.transpose(tp[:], sn[:, t * D:(t + 1) * D], ident[:])
                    nc.scalar.copy(snT[:, t * 128:(t + 1) * 128], tp[:])
                nT[sname] = snT

            pairs = [("OTH", "X"), ("TGT", "X"), ("OTH", "OTH"), ("TGT", "TGT")]
            for ptype, (an, cn) in enumerate(pairs):
                inst = ptype * bpc + b
                aT, cT = nT[an], nT[cn]
                for rt in range(nrowt):
                    g = p_G.tile([128, t_len], F32, tag="G")
                    nc.tensor.matmul(
                        g[:], aT[:, rt * 128:(rt + 1) * 128], cT[:],
                        start=True, stop=True,
                    )
                    kt = p_K.tile([128, t_len], F32, tag="K")
                    nc.scalar.activation(kt[:], g[:], AF.Exp, bias=bias_m1[:])
                    nc.sync.dma_start(
                        kbuf[inst, 3 + rt * 128:3 + (rt + 1) * 128, :], kt[:])

        # ---------------- Phase B: wavefront row-scan -----------------------
        Ea = p_E.tile([npart, 129], F32, tag="Ea")
        Eb = p_E.tile([npart, 129], F32, tag="Eb")
        Etiles = [Ea, Eb]
        cacc = p_acc.tile([npart, 1], F32, tag="C")
        nc.gpsimd.memset(Ea[:], 0.0)
        nc.gpsimd.memset(Eb[:], 0.0)
        nc.gpsimd.memset(cacc[:], 0.0)
        nc.gpsimd.memset(Ea[0:ni, 0:1], 1.0)  # E[-1][-1] = 1 for group 0
        car_prev = p_carry.tile([npart, 1], F32, tag="car")
        car_prev_mm = nc.vector.memset(car_prev[:], 0.0)

        for w in range(nsteps):
            prev = Etiles[w % 2]
            newt = Etiles[(w + 1) % 2]
            kt = p_k.tile([npart, 128], F32, tag="krow")
            for g in range(ch):
                nc.sync.dma_start(
                    kt[g * ni:(g + 1) * ni, :],
                    kbuf[:, w - g + 3, g * 128:(g + 1) * 128],
                )
            s = p_s.tile([npart, 128], F32, tag="s")
            nc.vector.tensor_add(s[:], prev[:, 1:129], prev[:, 0:128])
            scan_i = nc.vector.tensor_tensor_scan(
                newt[:, 1:129], s[:], kt[:], car_prev[:, 0:1],
                OP.add, OP.mult,
            )
            add_dep_helper(scan_i.ins, car_prev_mm.ins,
                           reason="scan initial after PE carry shift")
            if (w + 1) % resc == 0 and w + 1 < t_len:
                # per-partition chunk max -> per-instance max across groups
                pmax = p_stat.tile([npart, 1], F32, tag="pmax")
                nc.vector.tensor_reduce(pmax[:], newt[:, 1:129], AX.X, OP.max)
                pmT = p_bc.tile([1, npart], F32, tag="bc")
                t1 = nc.tensor.transpose(pmT[:], pmax[:],
                                         ident[0:npart, 0:npart])
                mxrow = p_stat.tile([1, ni], F32, tag="mxrow")
                rd2 = nc.vector.tensor_reduce(
                    mxrow[:], pmT[:].rearrange("a (g i) -> a i g", i=ni),
                    AX.X, OP.max)
                add_dep_helper(rd2.ins, t1.ins, reason="reduce after PE T1")
                mxps = p_bc.tile([ni, 1], F32, tag="bc")
                t2 = nc.tensor.transpose(mxps[:], mxrow[:], ident[0:1, 0:1])
                mxcol = p_stat.tile([ni, 1], F32, tag="mxcol")
                cpm = nc.scalar.copy(mxcol[:], mxps[:])
                add_dep_helper(cpm.ins, t2.ins, reason="copy after PE T2")
                bc = p_bc.tile([npart, 1], F32, tag="bc")
                bc_mm = nc.tensor.matmul(bc[:], bcastM[:], mxcol[:],
                                         start=True, stop=True)
                rec = p_stat.tile([npart, 1], F32, tag="rec")
                rcp = nc.vector.reciprocal(rec[:], bc[:])
                add_dep_helper(rcp.ins, bc_mm.ins,
                               reason="recip after PE broadcast")
                nc.vector.tensor_scalar_mul(newt[:, 0:129], newt[:, 0:129], rec[:])
                lgr = p_stat.tile([npart, 1], F32, tag="lgr")
                nc.scalar.activation(lgr[:], rec[:], AF.Ln)
                nc.vector.tensor_sub(cacc[:], cacc[:], lgr[:])
            car = p_carry.tile([npart, 1], F32, tag="car")
            car_mm = nc.tensor.matmul(car[:], shiftM[:], newt[:, 128:129],
                                      start=True, stop=True)
            cp = nc.scalar.copy(prev[:, 0:1], car[:])
            add_dep_helper(cp.ins, car_mm.ins,
                           reason="carry copy after PE shift")
            car_prev = car
            car_prev_mm = car_mm

        last = Etiles[nsteps % 2]
        nc.sync.dma_start(outs["EOUT"].rearrange("(a b) -> a b", b=1),
                          last[(ch - 1) * ni:ch * ni, 128:129])
        nc.sync.dma_start(outs["COUT"].rearrange("(a b) -> a b", b=1),
                          cacc[(ch - 1) * ni:ch * ni, 0:1])


def _build(t_len=T, bpc=BPC, resc=RESC, num_devices=NCORES, wave=False):
    ni = NTYPE * bpc
    nc = bacc.Bacc(
        "TRN2", target_bir_lowering=False, debug=False, num_devices=num_devices,
    )
    ins = {
        name: nc.dram_tensor(name, [bpc, t_len, D], F32, kind="ExternalInput").ap()
        for name in ("TGT", "OTH", "X")
    }
    outs = {
        "EOUT": nc.dram_tensor("EOUT", [ni], F32, kind="ExternalOutput").ap(),
        "COUT": nc.dram_tensor("COUT", [ni], F32, kind="ExternalOutput").ap(),
    }
    if wave:
        kbuf = nc.dram_tensor("KBUF", [ni, t_len + 6, t_len], F32).ap()
        with tile.TileContext(nc) as tc:
            _emit_wave(tc, ins, outs, kbuf, t_len, bpc, resc)
    else:
        kbuf = nc.dram_tensor("KBUF", [ni, t_len, t_len], F32).ap()
        with tile.TileContext(nc) as tc:
            _emit(tc, ins, outs, kbuf, t_len, bpc, resc)
    nc.compile()
    return nc


_NC = None


def _get_nc():
    global _NC
    if _NC is None:
        _NC = _build(wave=os.environ.get("KWAVE", "0") == "1")
    return _NC


def _postprocess(results, labels):
    E = np.stack([r["EOUT"] for r in results])  # [8, 32]
    C = np.stack([r["COUT"] for r in results])  # [8, 32]
    R = -(np.log(E) + C)                        # [core, type*8+b]
    R = R.reshape(NCORES, NTYPE, BPC).transpose(1, 0, 2).reshape(NTYPE, B)
    diff = (R[0] - R[1] - 0.5 * R[2] + 0.5 * R[3]).astype(np.float32)
    lab = np.asarray(labels, dtype=np.float32)
    return np.float32(np.mean((diff - lab) ** 2, dtype=np.float32))


def kernel(TGT, OTH, X, labels):
    nc = _get_nc()
    TGT = np.ascontiguousarray(np.asarray(TGT, dtype=np.float32))
    OTH = np.ascontiguousarray(np.asarray(OTH, dtype=np.float32))
    X = np.ascontiguousarray(np.asarray(X, dtype=np.float32))
    in_maps = [
        {
            "TGT": TGT[c * BPC:(c + 1) * BPC],
            "OTH": OTH[c * BPC:(c + 1) * BPC],
            "X": X[c * BPC:(c + 1) * BPC],
        }
        for c in range(NCORES)
    ]
    res = run_bass_kernel_spmd(nc, in_maps, core_ids=list(range(NCORES)))
    return _postprocess(res.results, labels)



# revision 14
# speedup vs baseline: 1.4405x; 1.4405x over previous
"""Soft-DTW ranking loss kernel for Trainium2 (8 NeuronCores, SPMD data parallel).

Math: loss = mean((diff - labels)^2) where
  diff_b = sdtw(OTH_b,X_b) - sdtw(TGT_b,X_b) - 0.5*sdtw(OTH_b,OTH_b) + 0.5*sdtw(TGT_b,TGT_b)
(the sdtw(X,X) terms of the normalized soft-DTW cancel exactly).

Soft-DTW (gamma=1) is computed in the probability domain:
  E[i,j] = K[i,j] * (E[i-1,j] + E[i-1,j-1] + E[i,j-1]),  K = exp(<xn_i,yn_j> - 1)
which maps one DP row onto a single DVE tensor_tensor_scan:
  state = (s[t] + state) * K[t],   s = E_prev + shift1(E_prev)
with periodic per-instance rescaling (log-scale accumulated in C) to stay in
fp32 range. Each core handles 8 batch items x 4 DTW instances = 32 independent
DPs vectorized across SBUF partitions.
"""

import os
import sys

import numpy as np

for _p in ("/root/.axon_site", "/root/.axon_site/_ro/trn_rl_repo",
           "/root/.axon_site/_ro/pypackages", "/opt/trn_rl_repo", "/opt/pypackages"):
    if os.path.isdir(_p) and _p not in sys.path:
        sys.path.append(_p)

import concourse.bass as bass
import concourse.tile as tile
from concourse.tile import add_dep_helper
from concourse import bacc, mybir
from concourse.bass_utils import run_bass_kernel_spmd
from concourse.masks import make_identity

F32 = mybir.dt.float32
F32R = mybir.dt.float32r
AX = mybir.AxisListType
OP = mybir.AluOpType
AF = mybir.ActivationFunctionType

B, T, D = 64, 512, 64
NCORES = 8
BPC = B // NCORES          # batch items per core
NTYPE = 4                  # (OTH,X), (TGT,X), (OTH,OTH), (TGT,TGT)
RESC = 32                  # rescale cadence (rows)


def _emit(tc: tile.TileContext, ins: dict, outs: dict, kbuf: bass.AP,
          t_len: int, bpc: int, resc: int):
    nc = tc.nc
    ni = NTYPE * bpc
    nrowt = t_len // 128

    with (
        tc.tile_pool(name="const", bufs=1) as p_const,
        tc.tile_pool(name="ain", bufs=2) as p_in,
        tc.tile_pool(name="astat", bufs=2) as p_astat,
        tc.tile_pool(name="asn", bufs=2) as p_asn,
        tc.tile_pool(name="apsT", bufs=2, space="PSUM") as p_psT,
        tc.tile_pool(name="ant", bufs=2) as p_nt,
        tc.tile_pool(name="aG", bufs=2, space="PSUM") as p_G,
        tc.tile_pool(name="aK", bufs=3) as p_K,
        tc.tile_pool(name="bE", bufs=1) as p_E,
        tc.tile_pool(name="bS", bufs=2) as p_s,
        tc.tile_pool(name="bK", bufs=4) as p_k,
        tc.tile_pool(name="bstat", bufs=2) as p_stat,
        tc.tile_pool(name="bacc", bufs=1) as p_acc,
    ):
        ident = p_const.tile([128, 128], F32, tag="ident")
        make_identity(nc, ident[:])
        bias_m1 = p_const.tile([128, 1], F32, tag="biasm1")
        nc.gpsimd.memset(bias_m1[:], -1.0)

        # ---------------- Phase A: K = exp(<xn,yn> - 1) for all pairs -------
        for b in range(bpc):
            nT = {}
            for sname in ("OTH", "TGT", "X"):
                src = ins[sname]
                xin = p_in.tile([128, nrowt * D], F32, tag=f"in_{sname}")
                nc.sync.dma_start(
                    xin[:].rearrange("p (t d) -> p t d", d=D),
                    src[b].rearrange("(t p) d -> p t d", p=128),
                )
                sq = p_astat.tile([128, nrowt * D], F32, tag=f"sq_{sname}")
                ss = p_astat.tile([128, nrowt], F32, tag=f"ss_{sname}")
                for t in range(nrowt):
                    nc.scalar.activation(
                        sq[:, t * D:(t + 1) * D], xin[:, t * D:(t + 1) * D],
                        AF.Square, accum_out=ss[:, t:t + 1],
                    )
                nrm = p_astat.tile([128, nrowt], F32, tag=f"nrm_{sname}")
                nc.scalar.activation(nrm[:], ss[:], AF.Sqrt)
                rnm = p_astat.tile([128, nrowt], F32, tag=f"rnm_{sname}")
                nc.vector.reciprocal(rnm[:], nrm[:])
                sn = p_asn.tile([128, nrowt * D], F32, tag=f"sn_{sname}")
                for t in range(nrowt):
                    nc.vector.tensor_scalar_mul(
                        sn[:, t * D:(t + 1) * D], xin[:, t * D:(t + 1) * D],
                        rnm[:, t:t + 1],
                    )
                snT = p_nt.tile([D, t_len], F32R, tag=f"nt_{sname}")
                for t in range(nrowt):
                    tp = p_psT.tile([D, 128], F32, tag="psT")
                    nc.tensor.transpose(tp[:], sn[:, t * D:(t + 1) * D], ident[:])
                    nc.scalar.copy(snT[:, t * 128:(t + 1) * 128], tp[:])
                nT[sname] = snT

            pairs = [("OTH", "X"), ("TGT", "X"), ("OTH", "OTH"), ("TGT", "TGT")]
            for ptype, (an, cn) in enumerate(pairs):
                inst = ptype * bpc + b
                aT, cT = nT[an], nT[cn]
                for rt in range(nrowt):
                    g = p_G.tile([128, t_len], F32, tag="G")
                    nc.tensor.matmul(
                        g[:],
                        aT[:, rt * 128:(rt + 1) * 128],
                        cT[:],
                        start=True, stop=True,
                    )
                    kt = p_K.tile([128, t_len], F32, tag="K")
                    nc.scalar.activation(kt[:], g[:], AF.Exp, bias=bias_m1[:])
                    nc.sync.dma_start(kbuf[inst, rt * 128:(rt + 1) * 128, :], kt[:])

        # ---------------- Phase B: row-scan DP over all instances -----------
        Ea = p_E.tile([ni, t_len + 1], F32, tag="Ea")
        Eb = p_E.tile([ni, t_len + 1], F32, tag="Eb")
        cacc = p_acc.tile([ni, 1], F32, tag="C")
        nc.gpsimd.memset(Ea[:], 0.0)
        nc.gpsimd.memset(Eb[:], 0.0)
        nc.gpsimd.memset(cacc[:], 0.0)
        nc.gpsimd.memset(Ea[:, 0:1], 1.0)  # E[-1][-1] = exp(-0)

        cur, nxt = Ea, Eb
        for r in range(t_len):
            kt = p_k.tile([ni, t_len], F32, tag="krow")
            nc.sync.dma_start(kt[:], kbuf[:, r, :])
            s = p_s.tile([ni, t_len], F32, tag="s")
            nc.vector.tensor_add(s[:], cur[:, 1:t_len + 1], cur[:, 0:t_len])
            nc.vector.tensor_tensor_scan(
                nxt[:, 1:t_len + 1], s[:], kt[:], 0.0, OP.add, OP.mult,
            )
            if r == 0:
                # E[0][-1] = 0: clear the one-time E[-1][-1] = 1 boundary
                nc.vector.memset(Ea[:, 0:1], 0.0)
            if (r + 1) % resc == 0 and r != t_len - 1:
                mx = p_stat.tile([ni, 1], F32, tag="mx")
                nc.vector.tensor_reduce(mx[:], nxt[:, 1:t_len + 1], AX.X, OP.max)
                rec = p_stat.tile([ni, 1], F32, tag="rec")
                nc.vector.reciprocal(rec[:], mx[:])
                nc.vector.tensor_scalar_mul(nxt[:, 1:t_len + 1],
                                            nxt[:, 1:t_len + 1], rec[:])
                lg = p_stat.tile([ni, 1], F32, tag="lg")
                nc.scalar.activation(lg[:], mx[:], AF.Ln)
                nc.vector.tensor_add(cacc[:], cacc[:], lg[:])
            cur, nxt = nxt, cur

        nc.sync.dma_start(outs["EOUT"].rearrange("(a b) -> a b", b=1),
                          cur[:, t_len:t_len + 1])
        nc.sync.dma_start(outs["COUT"].rearrange("(a b) -> a b", b=1), cacc[:])


def _emit_wave(tc: tile.TileContext, ins: dict, outs: dict, kbuf: bass.AP,
               t_len: int, bpc: int, resc: int):
    """Wavefront DP: CH=t_len/128 column chunks on partition groups.

    Partition p = g*ni + inst handles column chunk g of instance inst.
    Wavefront step w: group g processes row r = w - g (K rows padded with 3
    zero rows on each side so inactive groups compute zeros). Cross-chunk
    carries (scan initial / shifted-row boundary) move between partition
    groups via a constant shift matmul on the (otherwise idle) PE.
    """
    nc = tc.nc
    ni = NTYPE * bpc
    ch = t_len // 128
    npart = ch * ni
    nrowt = ch
    nsteps = t_len + ch - 1

    with (
        tc.tile_pool(name="const", bufs=1) as p_const,
        tc.tile_pool(name="ain", bufs=2) as p_in,
        tc.tile_pool(name="astat", bufs=2) as p_astat,
        tc.tile_pool(name="asn", bufs=2) as p_asn,
        tc.tile_pool(name="apsT", bufs=2, space="PSUM") as p_psT,
        tc.tile_pool(name="ant", bufs=2) as p_nt,
        tc.tile_pool(name="aG", bufs=2, space="PSUM") as p_G,
        tc.tile_pool(name="aK", bufs=3) as p_K,
        tc.tile_pool(name="bE", bufs=1) as p_E,
        tc.tile_pool(name="bS", bufs=2) as p_s,
        tc.tile_pool(name="bK", bufs=8) as p_k,
        tc.tile_pool(name="bC", bufs=3, space="PSUM") as p_carry,
        tc.tile_pool(name="bB", bufs=1, space="PSUM") as p_bc,
        tc.tile_pool(name="bstat", bufs=2) as p_stat,
        tc.tile_pool(name="bacc", bufs=1) as p_acc,
    ):
        ident = p_const.tile([128, 128], F32, tag="ident")
        make_identity(nc, ident[:])
        bias_m1 = p_const.tile([128, 1], F32, tag="biasm1")
        nc.gpsimd.memset(bias_m1[:], -1.0)
        # shiftM[k, p] = 1 iff k == p - ni  (moves group g-1 -> g)
        shiftM = p_const.tile([npart, npart], F32, tag="shiftM")
        nc.gpsimd.memset(shiftM[:], 0.0)
        nc.gpsimd.affine_select(
            out=shiftM[:], in_=shiftM[:], compare_op=OP.not_equal, fill=1.0,
            base=ni, pattern=[[-1, npart]], channel_multiplier=1,
        )
        # bcastM[k, (g, j)] = 1 iff k == j  (broadcast group-0 col to all groups)
        bcastM = p_const.tile([ni, npart], F32, tag="bcastM")
        nc.gpsimd.memset(bcastM[:], 0.0)
        nc.gpsimd.affine_select(
            out=bcastM[:].rearrange("k (g j) -> k g j", j=ni),
            in_=bcastM[:].rearrange("k (g j) -> k g j", j=ni),
            compare_op=OP.not_equal, fill=1.0,
            base=0, pattern=[[0, ch], [-1, ni]], channel_multiplier=1,
        )

        # zero the 3+3 pad rows of kbuf (layout [ni, t_len+6, t_len])
        zpad = p_const.tile([ni, 3 * t_len], F32, tag="zpad")
        nc.gpsimd.memset(zpad[:], 0.0)
        nc.sync.dma_start(
            kbuf[:, 0:3, :].rearrange("i r c -> i (r c)"), zpad[:])
        nc.sync.dma_start(
            kbuf[:, t_len + 3:t_len + 6, :].rearrange("i r c -> i (r c)"), zpad[:])

        # ---------------- Phase A (same as v1, +3 row offset into kbuf) -----
        for b in range(bpc):
            nT = {}
            for sname in ("OTH", "TGT", "X"):
                src = ins[sname]
                xin = p_in.tile([128, nrowt * D], F32, tag=f"in_{sname}")
                nc.sync.dma_start(
                    xin[:].rearrange("p (t d) -> p t d", d=D),
                    src[b].rearrange("(t p) d -> p t d", p=128),
                )
                sq = p_astat.tile([128, nrowt * D], F32, tag=f"sq_{sname}")
                ss = p_astat.tile([128, nrowt], F32, tag=f"ss_{sname}")
                for t in range(nrowt):
                    nc.scalar.activation(
                        sq[:, t * D:(t + 1) * D], xin[:, t * D:(t + 1) * D],
                        AF.Square, accum_out=ss[:, t:t + 1],
                    )
                nrm = p_astat.tile([128, nrowt], F32, tag=f"nrm_{sname}")
                nc.scalar.activation(nrm[:], ss[:], AF.Sqrt)
                rnm = p_astat.tile([128, nrowt], F32, tag=f"rnm_{sname}")
                nc.vector.reciprocal(rnm[:], nrm[:])
                sn = p_asn.tile([128, nrowt * D], F32, tag=f"sn_{sname}")
                for t in range(nrowt):
                    nc.vector.tensor_scalar_mul(
                        sn[:, t * D:(t + 1) * D], xin[:, t * D:(t + 1) * D],
                        rnm[:, t:t + 1],
                    )
                snT = p_nt.tile([D, t_len], F32R, tag=f"nt_{sname}")
                for t in range(nrowt):
                    tp = p_psT.tile([D, 128], F32, tag="psT")
                    nc.tensor.transpose(tp[:], sn[:, t * D:(t + 1) * D], ident[:])
                    nc.scalar.copy(snT[:, t * 128:(t + 1) * 128], tp[:])
                nT[sname] = snT

            pairs = [("OTH", "X"), ("TGT", "X"), ("OTH", "OTH"), ("TGT", "TGT")]
            for ptype, (an, cn) in enumerate(pairs):
                inst = ptype * bpc + b
                aT, cT = nT[an], nT[cn]
                for rt in range(nrowt):
                    g = p_G.tile([128, t_len], F32, tag="G")
                    nc.tensor.matmul(
                        g[:], aT[:, rt * 128:(rt + 1) * 128], cT[:],
                        start=True, stop=True,
                    )
                    kt = p_K.tile([128, t_len], F32, tag="K")
                    nc.scalar.activation(kt[:], g[:], AF.Exp, bias=bias_m1[:])
                    nc.sync.dma_start(
                        kbuf[inst, 3 + rt * 128:3 + (rt + 1) * 128, :], kt[:])

        # ---------------- Phase B: wavefront row-scan -----------------------
        Ea = p_E.tile([npart, 129], F32, tag="Ea")
        Eb = p_E.tile([npart, 129], F32, tag="Eb")
        Etiles = [Ea, Eb]
        cacc = p_acc.tile([npart, 1], F32, tag="C")
        nc.gpsimd.memset(Ea[:], 0.0)
        nc.gpsimd.memset(Eb[:], 0.0)
        nc.gpsimd.memset(cacc[:], 0.0)
        nc.gpsimd.memset(Ea[0:ni, 0:1], 1.0)  # E[-1][-1] = 1 for group 0
        car_prev = p_carry.tile([npart, 1], F32, tag="car")
        car_prev_mm = nc.vector.memset(car_prev[:], 0.0)

        for w in range(nsteps):
            prev = Etiles[w % 2]
            newt = Etiles[(w + 1) % 2]
            kt = p_k.tile([npart, 128], F32, tag="krow")
            for g in range(ch):
                nc.sync.dma_start(
                    kt[g * ni:(g + 1) * ni, :],
                    kbuf[:, w - g + 3, g * 128:(g + 1) * 128],
                )
            s = p_s.tile([npart, 128], F32, tag="s")
            nc.vector.tensor_add(s[:], prev[:, 1:129], prev[:, 0:128])
            scan_i = nc.vector.tensor_tensor_scan(
                newt[:, 1:129], s[:], kt[:], car_prev[:, 0:1],
                OP.add, OP.mult,
            )
            add_dep_helper(scan_i.ins, car_prev_mm.ins,
                           reason="scan initial after PE carry shift")
            if (w + 1) % resc == 0 and w + 1 < t_len:
                # per-partition chunk max -> per-instance max across groups
                pmax = p_stat.tile([npart, 1], F32, tag="pmax")
                nc.vector.tensor_reduce(pmax[:], newt[:, 1:129], AX.X, OP.max)
                pmT = p_bc.tile([1, npart], F32, tag="bc")
                t1 = nc.tensor.transpose(pmT[:], pmax[:],
                                         ident[0:npart, 0:npart])
                mxrow = p_stat.tile([1, ni], F32, tag="mxrow")
                rd2 = nc.vector.tensor_reduce(
                    mxrow[:], pmT[:].rearrange("a (g i) -> a i g", i=ni),
                    AX.X, OP.max)
                add_dep_helper(rd2.ins, t1.ins, reason="reduce after PE T1")
                mxps = p_bc.tile([ni, 1], F32, tag="bc")
                t2 = nc.tensor.transpose(mxps[:], mxrow[:], ident[0:1, 0:1])
                mxcol = p_stat.tile([ni, 1], F32, tag="mxcol")
                cpm = nc.scalar.copy(mxcol[:], mxps[:])
                add_dep_helper(cpm.ins, t2.ins, reason="copy after PE T2")
                bc = p_bc.tile([npart, 1], F32, tag="bc")
                bc_mm = nc.tensor.matmul(bc[:], bcastM[:], mxcol[:],
                                         start=True, stop=True)
                rec = p_stat.tile([npart, 1], F32, tag="rec")
                rcp = nc.vector.reciprocal(rec[:], bc[:])
                add_dep_helper(rcp.ins, bc_mm.ins,
                               reason="recip after PE broadcast")
                nc.vector.tensor_scalar_mul(newt[:, 0:129], newt[:, 0:129], rec[:])
                lgr = p_stat.tile([npart, 1], F32, tag="lgr")
                nc.scalar.activation(lgr[:], rec[:], AF.Ln)
                nc.vector.tensor_sub(cacc[:], cacc[:], lgr[:])
            car = p_carry.tile([npart, 1], F32, tag="car")
            car_mm = nc.tensor.matmul(car[:], shiftM[:], newt[:, 128:129],
                                      start=True, stop=True)
            cp = nc.scalar.copy(prev[:, 0:1], car[:])
            add_dep_helper(cp.ins, car_mm.ins,
                           reason="carry copy after PE shift")
            car_prev = car
            car_prev_mm = car_mm

        last = Etiles[nsteps % 2]
        nc.sync.dma_start(outs["EOUT"].rearrange("(a b) -> a b", b=1),
                          last[(ch - 1) * ni:ch * ni, 128:129])
        nc.sync.dma_start(outs["COUT"].rearrange("(a b) -> a b", b=1),
                          cacc[(ch - 1) * ni:ch * ni, 0:1])


def _emit_wave2(tc: tile.TileContext, ins: dict, outs: dict, kbuf: bass.AP,
                t_len: int, bpc: int, lag: int, sb: int):
    """Wavefront DP v2: lagged chunks + batched kt DMA + pipelined rescale.

    Group g processes row w - g*lag at step w (lag>=2 gives the PE carry
    shift and ACT boundary copy slack off the DVE critical path).  kt rows
    for `sb` consecutive steps are fetched in ONE diagonal-AP DMA.  The
    rescale max is computed 8 steps before it is applied, so its reduce/
    transpose/broadcast chain also runs off the critical path.
    """
    nc = tc.nc
    ni = NTYPE * bpc
    ch = t_len // 128
    npart = ch * ni
    nrowt = ch
    pad = (ch - 1) * lag
    nsteps = t_len + pad
    nbatch = (nsteps + sb - 1) // sb
    krows = t_len + 2 * pad + sb  # top pad + rows + bottom pad (incl DMA overrun)

    with (
        tc.tile_pool(name="const", bufs=1) as p_const,
    ):
        ident = p_const.tile([128, 128], F32, tag="ident")
        make_identity(nc, ident[:])
        bias_m1 = p_const.tile([128, 1], F32, tag="biasm1")
        nc.gpsimd.memset(bias_m1[:], -1.0)
        # Group g lives on partition block (ch-1-g); group g's carry source is
        # group g-1 = partition block +ni.  shiftM[k, p] = 1 iff k == p + ni.
        shiftM = p_const.tile([npart, npart], F32, tag="shiftM")
        nc.gpsimd.memset(shiftM[:], 0.0)
        nc.gpsimd.affine_select(
            out=shiftM[:], in_=shiftM[:], compare_op=OP.not_equal, fill=1.0,
            base=-ni, pattern=[[-1, npart]], channel_multiplier=1,
        )
        # bcastM[k, (g, j)] = 1 iff k == j  (broadcast per-inst col to all groups)
        bcastM = p_const.tile([ni, npart], F32, tag="bcastM")
        nc.gpsimd.memset(bcastM[:], 0.0)
        nc.gpsimd.affine_select(
            out=bcastM[:].rearrange("k (g j) -> k g j", j=ni),
            in_=bcastM[:].rearrange("k (g j) -> k g j", j=ni),
            compare_op=OP.not_equal, fill=1.0,
            base=0, pattern=[[0, ch], [-1, ni]], channel_multiplier=1,
        )

        # zero the pad rows of kbuf (layout [ni, krows, t_len])
        nbot = krows - t_len - pad
        zpad = p_const.tile([ni, nbot * 512], F32, tag="zpad")
        nc.gpsimd.memset(zpad[:], 0.0)
        nc.sync.dma_start(
            kbuf[:, 0:pad, :].rearrange("i r c -> i (r c)"),
            zpad[:, 0:pad * 512])
        nc.sync.dma_start(
            kbuf[:, t_len + pad:krows, :].rearrange("i r c -> i (r c)"),
            zpad[:])

        # ---------------- Phase A (as v1, +pad row offset into kbuf) --------
        with (
            tc.tile_pool(name="ain", bufs=2) as p_in,
            tc.tile_pool(name="astat", bufs=2) as p_astat,
            tc.tile_pool(name="asn", bufs=2) as p_asn,
            tc.tile_pool(name="apsT", bufs=2, space="PSUM") as p_psT,
            tc.tile_pool(name="ant", bufs=2) as p_nt,
            tc.tile_pool(name="aG", bufs=2, space="PSUM") as p_G,
            tc.tile_pool(name="aK", bufs=3) as p_K,
        ):
            _emit_phaseA(tc, ins, kbuf, t_len, bpc, pad,
                         p_in, p_astat, p_asn, p_psT, p_nt, p_G, p_K,
                         ident, bias_m1)

        # ---------------- Phase B: lagged wavefront row-scan ----------------
        with (
            tc.tile_pool(name="bE", bufs=1) as p_E,
            tc.tile_pool(name="bS", bufs=2) as p_s,
            tc.tile_pool(name="bK", bufs=4) as p_k,
            tc.tile_pool(name="bC", bufs=lag + 2, space="PSUM") as p_carry,
            tc.tile_pool(name="bB", bufs=1, space="PSUM") as p_bc,
            tc.tile_pool(name="bstat", bufs=4) as p_stat,
            tc.tile_pool(name="bacc", bufs=1) as p_acc,
        ):
            _emit_phaseB(tc, outs, kbuf, t_len, bpc, lag, sb,
                         p_E, p_s, p_k, p_carry, p_bc, p_stat, p_acc,
                         ident, shiftM, bcastM)


def _emit_phaseA(tc, ins, kbuf, t_len, bpc, pad,
                 p_in, p_astat, p_asn, p_psT, p_nt, p_G, p_K,
                 ident, bias_m1):
    nc = tc.nc
    nrowt = t_len // 128
    if True:
        for b in range(bpc):
            nT = {}
            for sname in ("OTH", "TGT", "X"):
                src = ins[sname]
                xin = p_in.tile([128, nrowt * D], F32, tag=f"in_{sname}")
                nc.sync.dma_start(
                    xin[:].rearrange("p (t d) -> p t d", d=D),
                    src[b].rearrange("(t p) d -> p t d", p=128),
                )
                sq = p_astat.tile([128, nrowt * D], F32, tag=f"sq_{sname}")
                ss = p_astat.tile([128, nrowt], F32, tag=f"ss_{sname}")
                for t in range(nrowt):
                    nc.scalar.activation(
                        sq[:, t * D:(t + 1) * D], xin[:, t * D:(t + 1) * D],
                        AF.Square, accum_out=ss[:, t:t + 1],
                    )
                nrm = p_astat.tile([128, nrowt], F32, tag=f"nrm_{sname}")
                nc.scalar.activation(nrm[:], ss[:], AF.Sqrt)
                rnm = p_astat.tile([128, nrowt], F32, tag=f"rnm_{sname}")
                nc.vector.reciprocal(rnm[:], nrm[:])
                sn = p_asn.tile([128, nrowt * D], F32, tag=f"sn_{sname}")
                for t in range(nrowt):
                    nc.vector.tensor_scalar_mul(
                        sn[:, t * D:(t + 1) * D], xin[:, t * D:(t + 1) * D],
                        rnm[:, t:t + 1],
                    )
                snT = p_nt.tile([D, t_len], F32R, tag=f"nt_{sname}")
                for t in range(nrowt):
                    tp = p_psT.tile([D, 128], F32, tag="psT")
                    nc.tensor.transpose(tp[:], sn[:, t * D:(t + 1) * D], ident[:])
                    nc.scalar.copy(snT[:, t * 128:(t + 1) * 128], tp[:])
                nT[sname] = snT

            pairs = [("OTH", "X"), ("TGT", "X"), ("OTH", "OTH"), ("TGT", "TGT")]
            for ptype, (an, cn) in enumerate(pairs):
                inst = ptype * bpc + b
                aT, cT = nT[an], nT[cn]
                for rt in range(nrowt):
                    g = p_G.tile([128, t_len], F32, tag="G")
                    nc.tensor.matmul(
                        g[:], aT[:, rt * 128:(rt + 1) * 128], cT[:],
                        start=True, stop=True,
                    )
                    kt = p_K.tile([128, t_len], F32, tag="K")
                    nc.scalar.activation(kt[:], g[:], AF.Exp, bias=bias_m1[:])
                    nc.sync.dma_start(
                        kbuf[inst, pad + rt * 128:pad + (rt + 1) * 128, :], kt[:])


def _emit_phaseB(tc, outs, kbuf, t_len, bpc, lag, sb,
                 p_E, p_s, p_k, p_carry, p_bc, p_stat, p_acc,
                 ident, shiftM, bcastM):
    nc = tc.nc
    ni = NTYPE * bpc
    ch = t_len // 128
    npart = ch * ni
    pad = (ch - 1) * lag
    nsteps = t_len + pad
    nbatch = (nsteps + sb - 1) // sb
    krows = t_len + 2 * pad + sb
    if True:
        Ea = p_E.tile([npart, 129], F32, tag="Ea")
        Eb = p_E.tile([npart, 129], F32, tag="Eb")
        Etiles = [Ea, Eb]
        cacc = p_acc.tile([npart, 1], F32, tag="C")
        nc.gpsimd.memset(Ea[:], 0.0)
        nc.gpsimd.memset(Eb[:], 0.0)
        nc.gpsimd.memset(cacc[:], 0.0)
        # E[-1][-1] = 1 for group 0 (= partition block ch-1)
        nc.gpsimd.memset(Ea[(ch - 1) * ni:ch * ni, 0:1], 1.0)

        zcar = p_stat.tile([npart, 1], F32, tag="zcar")
        nc.vector.memset(zcar[:], 0.0)
        cars = {w: zcar for w in range(-lag, 0)}  # car_w for w<0 is zero

        ktbs = {}
        rec_pending = None  # (rec_tile,) scheduled for the next apply step

        def fetch_batch(bi):
            w0 = bi * sb
            ktb = p_k.tile([npart, sb * 128], F32, tag="ktb")
            # ktb[(ch-1-g)*ni + i, s*128 + c] = kbuf[i, w0+s-g*lag+pad, g*128+c]
            for g in range(ch):
                blk = ch - 1 - g
                src = kbuf.copy()
                src.ap = type(src.ap)([
                    [krows * t_len, ni],        # i
                    [t_len, sb],                # s (step within batch)
                    [1, 128],                   # c
                ])
                src.offset = (w0 - g * lag + pad) * t_len + g * 128
                nc.sync.dma_start(
                    ktb[blk * ni:(blk + 1) * ni, :]
                    .rearrange("i (s c) -> i s c", c=128), src)
            ktbs[bi] = ktb

        fetch_batch(0)
        fetch_batch(1)

        for w in range(nsteps):
            cur = Etiles[w % 2]       # rows w-1-g*lag (prev), written by scan w-1
            newt = Etiles[(w + 1) % 2]
            if w % sb == 0 and (w // sb) + 2 < nbatch:
                fetch_batch(w // sb + 2)

            # pipelined rescale: apply scale computed 8 steps ago
            if rec_pending is not None and w % RESC == 0:
                rec, = rec_pending
                rec_pending = None
                nc.vector.tensor_scalar_mul(cur[:, 0:129], cur[:, 0:129], rec[:])
                for t in range(w - lag, w):
                    nc.vector.tensor_scalar_mul(cars[t][:], cars[t][:], rec[:])
                lgr = p_stat.tile([npart, 1], F32, tag="lgr")
                nc.scalar.activation(lgr[:], rec[:], AF.Ln)
                nc.vector.tensor_sub(cacc[:], cacc[:], lgr[:])

            s = p_s.tile([npart, 128], F32, tag="s")
            nc.vector.tensor_add(s[:], cur[:, 1:129], cur[:, 0:128])
            ktb = ktbs[w // sb]
            nc.vector.tensor_tensor_scan(
                newt[:, 1:129], s[:], ktb[:, (w % sb) * 128:(w % sb + 1) * 128],
                cars[w - lag][:, 0:1], OP.add, OP.mult,
            )
            if w - lag - 1 >= 0:
                cars.pop(w - lag - 1, None)

            # carry shift for step w+lag / boundary copy for step w+1
            car = p_carry.tile([npart, 1], F32, tag="car")
            nc.tensor.matmul(car[:], shiftM[:], newt[:, 128:129],
                             start=True, stop=True)
            cars[w] = car
            # boundary E[w-g*lag, g*128-1] for add_{w+1} (same value as the
            # scan initial consumed by scan_w)
            nc.scalar.copy(newt[:, 0:1], cars[w - lag][:])

            # pipelined rescale: compute scale from this step's rows
            if (w + 8) % RESC == 0 and (w + 8) <= 480:
                pmax = p_stat.tile([npart, 1], F32, tag="pmax")
                nc.vector.tensor_reduce(pmax[:], newt[:, 1:129], AX.X, OP.max)
                pmT = p_bc.tile([1, npart], F32, tag="bc")
                t1 = nc.tensor.transpose(pmT[:], pmax[:],
                                         ident[0:npart, 0:npart])
                mxrow = p_stat.tile([1, ni], F32, tag="mxrow")
                rd2 = nc.vector.tensor_reduce(
                    mxrow[:], pmT[:].rearrange("a (g i) -> a i g", i=ni),
                    AX.X, OP.max)
                add_dep_helper(rd2.ins, t1.ins, reason="reduce after PE T1")
                mxps = p_bc.tile([ni, 1], F32, tag="bc")
                t2 = nc.tensor.transpose(mxps[:], mxrow[:], ident[0:1, 0:1])
                mxcol = p_stat.tile([ni, 1], F32, tag="mxcol")
                cpm = nc.scalar.copy(mxcol[:], mxps[:])
                add_dep_helper(cpm.ins, t2.ins, reason="copy after PE T2")
                bc = p_bc.tile([npart, 1], F32, tag="bc")
                bc_mm = nc.tensor.matmul(bc[:], bcastM[:], mxcol[:],
                                         start=True, stop=True)
                rec = p_stat.tile([npart, 1], F32, tag="rec")
                rcp = nc.vector.reciprocal(rec[:], bc[:])
                add_dep_helper(rcp.ins, bc_mm.ins,
                               reason="recip after PE broadcast")
                rec_pending = (rec,)

        # group ch-1 (final column chunk) lives on partition block 0
        last = Etiles[nsteps % 2]
        nc.sync.dma_start(outs["EOUT"].rearrange("(a b) -> a b", b=1),
                          last[0:ni, 128:129])
        nc.sync.dma_start(outs["COUT"].rearrange("(a b) -> a b", b=1),
                          cacc[0:ni, 0:1])


def _build(t_len=T, bpc=BPC, resc=RESC, num_devices=NCORES, wave=False,
           wave2=False, lag=2, sb=8):
    ni = NTYPE * bpc
    nc = bacc.Bacc(
        "TRN2", target_bir_lowering=False, debug=False, num_devices=num_devices,
    )
    ins = {
        name: nc.dram_tensor(name, [bpc, t_len, D], F32, kind="ExternalInput").ap()
        for name in ("TGT", "OTH", "X")
    }
    outs = {
        "EOUT": nc.dram_tensor("EOUT", [ni], F32, kind="ExternalOutput").ap(),
        "COUT": nc.dram_tensor("COUT", [ni], F32, kind="ExternalOutput").ap(),
    }
    if wave2:
        ch = t_len // 128
        pad = (ch - 1) * lag
        krows = t_len + 2 * pad + sb
        kbuf = nc.dram_tensor("KBUF", [ni, krows, t_len], F32).ap()
        with tile.TileContext(nc) as tc:
            _emit_wave2(tc, ins, outs, kbuf, t_len, bpc, lag, sb)
    elif wave:
        kbuf = nc.dram_tensor("KBUF", [ni, t_len + 6, t_len], F32).ap()
        with tile.TileContext(nc) as tc:
            _emit_wave(tc, ins, outs, kbuf, t_len, bpc, resc)
    else:
        kbuf = nc.dram_tensor("KBUF", [ni, t_len, t_len], F32).ap()
        with tile.TileContext(nc) as tc:
            _emit(tc, ins, outs, kbuf, t_len, bpc, resc)
    nc.compile()
    return nc


_NC = None


def _get_nc():
    global _NC
    if _NC is None:
        kv = os.environ.get("KWAVE", "0")
        _NC = _build(wave=kv == "1", wave2=kv == "2")
    return _NC


def _postprocess(results, labels):
    E = np.stack([r["EOUT"] for r in results])  # [8, 32]
    C = np.stack([r["COUT"] for r in results])  # [8, 32]
    R = -(np.log(E) + C)                        # [core, type*8+b]
    R = R.reshape(NCORES, NTYPE, BPC).transpose(1, 0, 2).reshape(NTYPE, B)
    diff = (R[0] - R[1] - 0.5 * R[2] + 0.5 * R[3]).astype(np.float32)
    lab = np.asarray(labels, dtype=np.float32)
    return np.float32(np.mean((diff - lab) ** 2, dtype=np.float32))


def kernel(TGT, OTH, X, labels):
    nc = _get_nc()
    TGT = np.ascontiguousarray(np.asarray(TGT, dtype=np.float32))
    OTH = np.ascontiguousarray(np.asarray(OTH, dtype=np.float32))
    X = np.ascontiguousarray(np.asarray(X, dtype=np.float32))
    in_maps = [
        {
            "TGT": TGT[c * BPC:(c + 1) * BPC],
            "OTH": OTH[c * BPC:(c + 1) * BPC],
            "X": X[c * BPC:(c + 1) * BPC],
        }
        for c in range(NCORES)
    ]
    res = run_bass_kernel_spmd(nc, in_maps, core_ids=list(range(NCORES)))
    return _postprocess(res.results, labels)



# revision 21
# speedup vs baseline: 1.8337x; 1.2730x over previous
"""Soft-DTW ranking loss kernel for Trainium2 (8 NeuronCores, SPMD data parallel).

Math: loss = mean((diff - labels)^2) where
  diff_b = sdtw(OTH_b,X_b) - sdtw(TGT_b,X_b) - 0.5*sdtw(OTH_b,OTH_b) + 0.5*sdtw(TGT_b,TGT_b)
(the sdtw(X,X) terms of the normalized soft-DTW cancel exactly).

Soft-DTW (gamma=1) is computed in the probability domain:
  E[i,j] = K[i,j] * (E[i-1,j] + E[i-1,j-1] + E[i,j-1]),  K = exp(<xn_i,yn_j> - 1)
which maps one DP row onto a single DVE tensor_tensor_scan:
  state = (s[t] + state) * K[t],   s = E_prev + shift1(E_prev)
with periodic per-instance rescaling (log-scale accumulated in C) to stay in
fp32 range. Each core handles 8 batch items x 4 DTW instances = 32 independent
DPs vectorized across SBUF partitions.
"""

import os
import sys

import numpy as np

for _p in ("/root/.axon_site", "/root/.axon_site/_ro/trn_rl_repo",
           "/root/.axon_site/_ro/pypackages", "/opt/trn_rl_repo", "/opt/pypackages"):
    if os.path.isdir(_p) and _p not in sys.path:
        sys.path.append(_p)

import concourse.bass as bass
import concourse.tile as tile
from concourse.tile import add_dep_helper
from concourse import bacc, mybir
from concourse.bass_utils import run_bass_kernel_spmd
from concourse.masks import make_identity

F32 = mybir.dt.float32
F32R = mybir.dt.float32r
AX = mybir.AxisListType
OP = mybir.AluOpType
AF = mybir.ActivationFunctionType

B, T, D = 64, 512, 64
NCORES = 8
BPC = B // NCORES          # batch items per core
NTYPE = 4                  # (OTH,X), (TGT,X), (OTH,OTH), (TGT,TGT)
RESC = 32                  # rescale cadence (rows)


def _emit(tc: tile.TileContext, ins: dict, outs: dict, kbuf: bass.AP,
          t_len: int, bpc: int, resc: int):
    nc = tc.nc
    ni = NTYPE * bpc
    nrowt = t_len // 128

    with (
        tc.tile_pool(name="const", bufs=1) as p_const,
        tc.tile_pool(name="ain", bufs=2) as p_in,
        tc.tile_pool(name="astat", bufs=2) as p_astat,
        tc.tile_pool(name="asn", bufs=2) as p_asn,
        tc.tile_pool(name="apsT", bufs=2, space="PSUM") as p_psT,
        tc.tile_pool(name="ant", bufs=2) as p_nt,
        tc.tile_pool(name="aG", bufs=2, space="PSUM") as p_G,
        tc.tile_pool(name="aK", bufs=3) as p_K,
        tc.tile_pool(name="bE", bufs=1) as p_E,
        tc.tile_pool(name="bS", bufs=2) as p_s,
        tc.tile_pool(name="bK", bufs=4) as p_k,
        tc.tile_pool(name="bstat", bufs=2) as p_stat,
        tc.tile_pool(name="bacc", bufs=1) as p_acc,
    ):
        ident = p_const.tile([128, 128], F32, tag="ident")
        make_identity(nc, ident[:])
        bias_m1 = p_const.tile([128, 1], F32, tag="biasm1")
        nc.gpsimd.memset(bias_m1[:], -1.0)

        # ---------------- Phase A: K = exp(<xn,yn> - 1) for all pairs -------
        for b in range(bpc):
            nT = {}
            for sname in ("OTH", "TGT", "X"):
                src = ins[sname]
                xin = p_in.tile([128, nrowt * D], F32, tag=f"in_{sname}")
                nc.sync.dma_start(
                    xin[:].rearrange("p (t d) -> p t d", d=D),
                    src[b].rearrange("(t p) d -> p t d", p=128),
                )
                sq = p_astat.tile([128, nrowt * D], F32, tag=f"sq_{sname}")
                ss = p_astat.tile([128, nrowt], F32, tag=f"ss_{sname}")
                for t in range(nrowt):
                    nc.scalar.activation(
                        sq[:, t * D:(t + 1) * D], xin[:, t * D:(t + 1) * D],
                        AF.Square, accum_out=ss[:, t:t + 1],
                    )
                nrm = p_astat.tile([128, nrowt], F32, tag=f"nrm_{sname}")
                nc.scalar.activation(nrm[:], ss[:], AF.Sqrt)
                rnm = p_astat.tile([128, nrowt], F32, tag=f"rnm_{sname}")
                nc.vector.reciprocal(rnm[:], nrm[:])
                sn = p_asn.tile([128, nrowt * D], F32, tag=f"sn_{sname}")
                for t in range(nrowt):
                    nc.vector.tensor_scalar_mul(
                        sn[:, t * D:(t + 1) * D], xin[:, t * D:(t + 1) * D],
                        rnm[:, t:t + 1],
                    )
                snT = p_nt.tile([D, t_len], F32R, tag=f"nt_{sname}")
                for t in range(nrowt):
                    tp = p_psT.tile([D, 128], F32, tag="psT")
                    nc.tensor.transpose(tp[:], sn[:, t * D:(t + 1) * D], ident[:])
                    nc.scalar.copy(snT[:, t * 128:(t + 1) * 128], tp[:])
                nT[sname] = snT

            pairs = [("OTH", "X"), ("TGT", "X"), ("OTH", "OTH"), ("TGT", "TGT")]
            for ptype, (an, cn) in enumerate(pairs):
                inst = ptype * bpc + b
                aT, cT = nT[an], nT[cn]
                for rt in range(nrowt):
                    g = p_G.tile([128, t_len], F32, tag="G")
                    nc.tensor.matmul(
                        g[:],
                        aT[:, rt * 128:(rt + 1) * 128],
                        cT[:],
                        start=True, stop=True,
                    )
                    kt = p_K.tile([128, t_len], F32, tag="K")
                    nc.scalar.activation(kt[:], g[:], AF.Exp, bias=bias_m1[:])
                    nc.sync.dma_start(kbuf[inst, rt * 128:(rt + 1) * 128, :], kt[:])

        # ---------------- Phase B: row-scan DP over all instances -----------
        Ea = p_E.tile([ni, t_len + 1], F32, tag="Ea")
        Eb = p_E.tile([ni, t_len + 1], F32, tag="Eb")
        cacc = p_acc.tile([ni, 1], F32, tag="C")
        nc.gpsimd.memset(Ea[:], 0.0)
        nc.gpsimd.memset(Eb[:], 0.0)
        nc.gpsimd.memset(cacc[:], 0.0)
        nc.gpsimd.memset(Ea[:, 0:1], 1.0)  # E[-1][-1] = exp(-0)

        cur, nxt = Ea, Eb
        for r in range(t_len):
            kt = p_k.tile([ni, t_len], F32, tag="krow")
            nc.sync.dma_start(kt[:], kbuf[:, r, :])
            s = p_s.tile([ni, t_len], F32, tag="s")
            nc.vector.tensor_add(s[:], cur[:, 1:t_len + 1], cur[:, 0:t_len])
            nc.vector.tensor_tensor_scan(
                nxt[:, 1:t_len + 1], s[:], kt[:], 0.0, OP.add, OP.mult,
            )
            if r == 0:
                # E[0][-1] = 0: clear the one-time E[-1][-1] = 1 boundary
                nc.vector.memset(Ea[:, 0:1], 0.0)
            if (r + 1) % resc == 0 and r != t_len - 1:
                mx = p_stat.tile([ni, 1], F32, tag="mx")
                nc.vector.tensor_reduce(mx[:], nxt[:, 1:t_len + 1], AX.X, OP.max)
                rec = p_stat.tile([ni, 1], F32, tag="rec")
                nc.vector.reciprocal(rec[:], mx[:])
                nc.vector.tensor_scalar_mul(nxt[:, 1:t_len + 1],
                                            nxt[:, 1:t_len + 1], rec[:])
                lg = p_stat.tile([ni, 1], F32, tag="lg")
                nc.scalar.activation(lg[:], mx[:], AF.Ln)
                nc.vector.tensor_add(cacc[:], cacc[:], lg[:])
            cur, nxt = nxt, cur

        nc.sync.dma_start(outs["EOUT"].rearrange("(a b) -> a b", b=1),
                          cur[:, t_len:t_len + 1])
        nc.sync.dma_start(outs["COUT"].rearrange("(a b) -> a b", b=1), cacc[:])


def _emit_wave(tc: tile.TileContext, ins: dict, outs: dict, kbuf: bass.AP,
               t_len: int, bpc: int, resc: int):
    """Wavefront DP: CH=t_len/128 column chunks on partition groups.

    Partition p = g*ni + inst handles column chunk g of instance inst.
    Wavefront step w: group g processes row r = w - g (K rows padded with 3
    zero rows on each side so inactive groups compute zeros). Cross-chunk
    carries (scan initial / shifted-row boundary) move between partition
    groups via a constant shift matmul on the (otherwise idle) PE.
    """
    nc = tc.nc
    ni = NTYPE * bpc
    ch = t_len // 128
    npart = ch * ni
    nrowt = ch
    nsteps = t_len + ch - 1

    with (
        tc.tile_pool(name="const", bufs=1) as p_const,
        tc.tile_pool(name="ain", bufs=2) as p_in,
        tc.tile_pool(name="astat", bufs=2) as p_astat,
        tc.tile_pool(name="asn", bufs=2) as p_asn,
        tc.tile_pool(name="apsT", bufs=2, space="PSUM") as p_psT,
        tc.tile_pool(name="ant", bufs=2) as p_nt,
        tc.tile_pool(name="aG", bufs=2, space="PSUM") as p_G,
        tc.tile_pool(name="aK", bufs=3) as p_K,
        tc.tile_pool(name="bE", bufs=1) as p_E,
        tc.tile_pool(name="bS", bufs=2) as p_s,
        tc.tile_pool(name="bK", bufs=8) as p_k,
        tc.tile_pool(name="bC", bufs=3, space="PSUM") as p_carry,
        tc.tile_pool(name="bB", bufs=1, space="PSUM") as p_bc,
        tc.tile_pool(name="bstat", bufs=2) as p_stat,
        tc.tile_pool(name="bacc", bufs=1) as p_acc,
    ):
        ident = p_const.tile([128, 128], F32, tag="ident")
        make_identity(nc, ident[:])
        bias_m1 = p_const.tile([128, 1], F32, tag="biasm1")
        nc.gpsimd.memset(bias_m1[:], -1.0)
        # shiftM[k, p] = 1 iff k == p - ni  (moves group g-1 -> g)
        shiftM = p_const.tile([npart, npart], F32, tag="shiftM")
        nc.gpsimd.memset(shiftM[:], 0.0)
        nc.gpsimd.affine_select(
            out=shiftM[:], in_=shiftM[:], compare_op=OP.not_equal, fill=1.0,
            base=ni, pattern=[[-1, npart]], channel_multiplier=1,
        )
        # bcastM[k, (g, j)] = 1 iff k == j  (broadcast group-0 col to all groups)
        bcastM = p_const.tile([ni, npart], F32, tag="bcastM")
        nc.gpsimd.memset(bcastM[:], 0.0)
        nc.gpsimd.affine_select(
            out=bcastM[:].rearrange("k (g j) -> k g j", j=ni),
            in_=bcastM[:].rearrange("k (g j) -> k g j", j=ni),
            compare_op=OP.not_equal, fill=1.0,
            base=0, pattern=[[0, ch], [-1, ni]], channel_multiplier=1,
        )

        # zero the 3+3 pad rows of kbuf (layout [ni, t_len+6, t_len])
        zpad = p_const.tile([ni, 3 * t_len], F32, tag="zpad")
        nc.gpsimd.memset(zpad[:], 0.0)
        nc.sync.dma_start(
            kbuf[:, 0:3, :].rearrange("i r c -> i (r c)"), zpad[:])
        nc.sync.dma_start(
            kbuf[:, t_len + 3:t_len + 6, :].rearrange("i r c -> i (r c)"), zpad[:])

        # ---------------- Phase A (same as v1, +3 row offset into kbuf) -----
        for b in range(bpc):
            nT = {}
            for sname in ("OTH", "TGT", "X"):
                src = ins[sname]
                xin = p_in.tile([128, nrowt * D], F32, tag=f"in_{sname}")
                nc.sync.dma_start(
                    xin[:].rearrange("p (t d) -> p t d", d=D),
                    src[b].rearrange("(t p) d -> p t d", p=128),
                )
                sq = p_astat.tile([128, nrowt * D], F32, tag=f"sq_{sname}")
                ss = p_astat.tile([128, nrowt], F32, tag=f"ss_{sname}")
                for t in range(nrowt):
                    nc.scalar.activation(
                        sq[:, t * D:(t + 1) * D], xin[:, t * D:(t + 1) * D],
                        AF.Square, accum_out=ss[:, t:t + 1],
                    )
                nrm = p_astat.tile([128, nrowt], F32, tag=f"nrm_{sname}")
                nc.scalar.activation(nrm[:], ss[:], AF.Sqrt)
                rnm = p_astat.tile([128, nrowt], F32, tag=f"rnm_{sname}")
                nc.vector.reciprocal(rnm[:], nrm[:])
                sn = p_asn.tile([128, nrowt * D], F32, tag=f"sn_{sname}")
                for t in range(nrowt):
                    nc.vector.tensor_scalar_mul(
                        sn[:, t * D:(t + 1) * D], xin[:, t * D:(t + 1) * D],
                        rnm[:, t:t + 1],
                    )
                snT = p_nt.tile([D, t_len], F32R, tag=f"nt_{sname}")
                for t in range(nrowt):
                    tp = p_psT.tile([D, 128], F32, tag="psT")
                    nc.tensor.transpose(tp[:], sn[:, t * D:(t + 1) * D], ident[:])
                    nc.scalar.copy(snT[:, t * 128:(t + 1) * 128], tp[:])
                nT[sname] = snT

            pairs = [("OTH", "X"), ("TGT", "X"), ("OTH", "OTH"), ("TGT", "TGT")]
            for ptype, (an, cn) in enumerate(pairs):
                inst = ptype * bpc + b
                aT, cT = nT[an], nT[cn]
                for rt in range(nrowt):
                    g = p_G.tile([128, t_len], F32, tag="G")
                    nc.tensor.matmul(
                        g[:], aT[:, rt * 128:(rt + 1) * 128], cT[:],
                        start=True, stop=True,
                    )
                    kt = p_K.tile([128, t_len], F32, tag="K")
                    nc.scalar.activation(kt[:], g[:], AF.Exp, bias=bias_m1[:])
                    nc.sync.dma_start(
                        kbuf[inst, 3 + rt * 128:3 + (rt + 1) * 128, :], kt[:])

        # ---------------- Phase B: wavefront row-scan -----------------------
        Ea = p_E.tile([npart, 129], F32, tag="Ea")
        Eb = p_E.tile([npart, 129], F32, tag="Eb")
        Etiles = [Ea, Eb]
        cacc = p_acc.tile([npart, 1], F32, tag="C")
        nc.gpsimd.memset(Ea[:], 0.0)
        nc.gpsimd.memset(Eb[:], 0.0)
        nc.gpsimd.memset(cacc[:], 0.0)
        nc.gpsimd.memset(Ea[0:ni, 0:1], 1.0)  # E[-1][-1] = 1 for group 0
        car_prev = p_carry.tile([npart, 1], F32, tag="car")
        car_prev_mm = nc.vector.memset(car_prev[:], 0.0)

        for w in range(nsteps):
            prev = Etiles[w % 2]
            newt = Etiles[(w + 1) % 2]
            kt = p_k.tile([npart, 128], F32, tag="krow")
            for g in range(ch):
                nc.sync.dma_start(
                    kt[g * ni:(g + 1) * ni, :],
                    kbuf[:, w - g + 3, g * 128:(g + 1) * 128],
                )
            s = p_s.tile([npart, 128], F32, tag="s")
            nc.vector.tensor_add(s[:], prev[:, 1:129], prev[:, 0:128])
            scan_i = nc.vector.tensor_tensor_scan(
                newt[:, 1:129], s[:], kt[:], car_prev[:, 0:1],
                OP.add, OP.mult,
            )
            add_dep_helper(scan_i.ins, car_prev_mm.ins,
                           reason="scan initial after PE carry shift")
            if (w + 1) % resc == 0 and w + 1 < t_len:
                # per-partition chunk max -> per-instance max across groups
                pmax = p_stat.tile([npart, 1], F32, tag="pmax")
                nc.vector.tensor_reduce(pmax[:], newt[:, 1:129], AX.X, OP.max)
                pmT = p_bc.tile([1, npart], F32, tag="bc")
                t1 = nc.tensor.transpose(pmT[:], pmax[:],
                                         ident[0:npart, 0:npart])
                mxrow = p_stat.tile([1, ni], F32, tag="mxrow")
                rd2 = nc.vector.tensor_reduce(
                    mxrow[:], pmT[:].rearrange("a (g i) -> a i g", i=ni),
                    AX.X, OP.max)
                add_dep_helper(rd2.ins, t1.ins, reason="reduce after PE T1")
                mxps = p_bc.tile([ni, 1], F32, tag="bc")
                t2 = nc.tensor.transpose(mxps[:], mxrow[:], ident[0:1, 0:1])
                mxcol = p_stat.tile([ni, 1], F32, tag="mxcol")
                cpm = nc.scalar.copy(mxcol[:], mxps[:])
                add_dep_helper(cpm.ins, t2.ins, reason="copy after PE T2")
                bc = p_bc.tile([npart, 1], F32, tag="bc")
                bc_mm = nc.tensor.matmul(bc[:], bcastM[:], mxcol[:],
                                         start=True, stop=True)
                rec = p_stat.tile([npart, 1], F32, tag="rec")
                rcp = nc.vector.reciprocal(rec[:], bc[:])
                add_dep_helper(rcp.ins, bc_mm.ins,
                               reason="recip after PE broadcast")
                nc.vector.tensor_scalar_mul(newt[:, 0:129], newt[:, 0:129], rec[:])
                lgr = p_stat.tile([npart, 1], F32, tag="lgr")
                nc.scalar.activation(lgr[:], rec[:], AF.Ln)
                nc.vector.tensor_sub(cacc[:], cacc[:], lgr[:])
            car = p_carry.tile([npart, 1], F32, tag="car")
            car_mm = nc.tensor.matmul(car[:], shiftM[:], newt[:, 128:129],
                                      start=True, stop=True)
            cp = nc.scalar.copy(prev[:, 0:1], car[:])
            add_dep_helper(cp.ins, car_mm.ins,
                           reason="carry copy after PE shift")
            car_prev = car
            car_prev_mm = car_mm

        last = Etiles[nsteps % 2]
        nc.sync.dma_start(outs["EOUT"].rearrange("(a b) -> a b", b=1),
                          last[(ch - 1) * ni:ch * ni, 128:129])
        nc.sync.dma_start(outs["COUT"].rearrange("(a b) -> a b", b=1),
                          cacc[(ch - 1) * ni:ch * ni, 0:1])


def _emit_wave2(tc: tile.TileContext, ins: dict, outs: dict, kbuf: bass.AP,
                t_len: int, bpc: int, lag: int, sb: int):
    """Wavefront DP v2: lagged chunks + batched kt DMA + pipelined rescale.

    Group g processes row w - g*lag at step w (lag>=2 gives the PE carry
    shift and ACT boundary copy slack off the DVE critical path).  kt rows
    for `sb` consecutive steps are fetched in ONE diagonal-AP DMA.  The
    rescale max is computed 8 steps before it is applied, so its reduce/
    transpose/broadcast chain also runs off the critical path.
    """
    nc = tc.nc
    ni = NTYPE * bpc
    ch = t_len // 128
    npart = ch * ni
    nrowt = ch
    pad = (ch - 1) * lag
    nsteps = t_len + pad
    nbatch = (nsteps + sb - 1) // sb
    krows = t_len + 2 * pad + sb  # top pad + rows + bottom pad (incl DMA overrun)

    with (
        tc.tile_pool(name="const", bufs=1) as p_const,
    ):
        ident = p_const.tile([128, 128], F32, tag="ident")
        make_identity(nc, ident[:])
        bias_m1 = p_const.tile([128, 1], F32, tag="biasm1")
        nc.gpsimd.memset(bias_m1[:], -1.0)
        # Group g lives on partition block (ch-1-g); carries move to the next
        # block via a partition-shift DMA (no PE involvement).
        # bcastM[k, (g, j)] = 1 iff k == j  (broadcast per-inst col to all groups)
        bcastM = p_const.tile([ni, npart], F32, tag="bcastM")
        nc.gpsimd.memset(bcastM[:], 0.0)
        nc.gpsimd.affine_select(
            out=bcastM[:].rearrange("k (g j) -> k g j", j=ni),
            in_=bcastM[:].rearrange("k (g j) -> k g j", j=ni),
            compare_op=OP.not_equal, fill=1.0,
            base=0, pattern=[[0, ch], [-1, ni]], channel_multiplier=1,
        )

        # zero the pad rows of kbuf (layout [ni, krows, t_len])
        nbot = krows - t_len - pad
        zpad = p_const.tile([ni, nbot * 512], F32, tag="zpad")
        nc.gpsimd.memset(zpad[:], 0.0)
        nc.sync.dma_start(
            kbuf[:, 0:pad, :].rearrange("i r c -> i (r c)"),
            zpad[:, 0:pad * 512])
        nc.sync.dma_start(
            kbuf[:, t_len + pad:krows, :].rearrange("i r c -> i (r c)"),
            zpad[:])

        # ---------------- Phase A (as v1, +pad row offset into kbuf) --------
        with (
            tc.tile_pool(name="ain", bufs=2) as p_in,
            tc.tile_pool(name="astat", bufs=2) as p_astat,
            tc.tile_pool(name="asn", bufs=2) as p_asn,
            tc.tile_pool(name="apsT", bufs=2, space="PSUM") as p_psT,
            tc.tile_pool(name="ant", bufs=2) as p_nt,
            tc.tile_pool(name="aG", bufs=2, space="PSUM") as p_G,
            tc.tile_pool(name="aK", bufs=3) as p_K,
        ):
            _emit_phaseA(tc, ins, kbuf, t_len, bpc, pad,
                         p_in, p_astat, p_asn, p_psT, p_nt, p_G, p_K,
                         ident, bias_m1)

        # ---------------- Phase B: lagged wavefront row-scan ----------------
        with (
            tc.tile_pool(name="bE", bufs=1) as p_E,
            tc.tile_pool(name="bS", bufs=2) as p_s,
            tc.tile_pool(name="bK", bufs=4) as p_k,
            tc.tile_pool(name="bB", bufs=1, space="PSUM") as p_bc,
            tc.tile_pool(name="bstat", bufs=4) as p_stat,
            tc.tile_pool(name="bacc", bufs=1) as p_acc,
        ):
            _emit_phaseB(tc, outs, kbuf, t_len, bpc, lag, sb,
                         p_E, p_s, p_k, p_bc, p_stat, p_acc,
                         ident, bcastM)


def _emit_phaseA(tc, ins, kbuf, t_len, bpc, pad,
                 p_in, p_astat, p_asn, p_psT, p_nt, p_G, p_K,
                 ident, bias_m1):
    nc = tc.nc
    nrowt = t_len // 128
    if True:
        for b in range(bpc):
            nT = {}
            for sname in ("OTH", "TGT", "X"):
                src = ins[sname]
                xin = p_in.tile([128, nrowt * D], F32, tag=f"in_{sname}")
                nc.sync.dma_start(
                    xin[:].rearrange("p (t d) -> p t d", d=D),
                    src[b].rearrange("(t p) d -> p t d", p=128),
                )
                sq = p_astat.tile([128, nrowt * D], F32, tag=f"sq_{sname}")
                ss = p_astat.tile([128, nrowt], F32, tag=f"ss_{sname}")
                for t in range(nrowt):
                    nc.scalar.activation(
                        sq[:, t * D:(t + 1) * D], xin[:, t * D:(t + 1) * D],
                        AF.Square, accum_out=ss[:, t:t + 1],
                    )
                nrm = p_astat.tile([128, nrowt], F32, tag=f"nrm_{sname}")
                nc.scalar.activation(nrm[:], ss[:], AF.Sqrt)
                rnm = p_astat.tile([128, nrowt], F32, tag=f"rnm_{sname}")
                nc.vector.reciprocal(rnm[:], nrm[:])
                sn = p_asn.tile([128, nrowt * D], F32, tag=f"sn_{sname}")
                for t in range(nrowt):
                    nc.vector.tensor_scalar_mul(
                        sn[:, t * D:(t + 1) * D], xin[:, t * D:(t + 1) * D],
                        rnm[:, t:t + 1],
                    )
                snT = p_nt.tile([D, t_len], F32R, tag=f"nt_{sname}")
                for t in range(nrowt):
                    tp = p_psT.tile([D, 128], F32, tag="psT")
                    nc.tensor.transpose(tp[:], sn[:, t * D:(t + 1) * D], ident[:])
                    nc.scalar.copy(snT[:, t * 128:(t + 1) * 128], tp[:])
                nT[sname] = snT

            pairs = [("OTH", "X"), ("TGT", "X"), ("OTH", "OTH"), ("TGT", "TGT")]
            for ptype, (an, cn) in enumerate(pairs):
                inst = ptype * bpc + b
                aT, cT = nT[an], nT[cn]
                for rt in range(nrowt):
                    g = p_G.tile([128, t_len], F32, tag="G")
                    nc.tensor.matmul(
                        g[:], aT[:, rt * 128:(rt + 1) * 128], cT[:],
                        start=True, stop=True,
                    )
                    kt = p_K.tile([128, t_len], F32, tag="K")
                    nc.scalar.activation(kt[:], g[:], AF.Exp, bias=bias_m1[:])
                    nc.sync.dma_start(
                        kbuf[inst, pad + rt * 128:pad + (rt + 1) * 128, :], kt[:])


def _emit_phaseB(tc, outs, kbuf, t_len, bpc, lag, sb,
                 p_E, p_s, p_k, p_bc, p_stat, p_acc,
                 ident, bcastM):
    nc = tc.nc
    ni = NTYPE * bpc
    ch = t_len // 128
    npart = ch * ni
    pad = (ch - 1) * lag
    nsteps = t_len + pad
    nbatch = (nsteps + sb - 1) // sb
    krows = t_len + 2 * pad + sb
    if True:
        NE = 4
        Etiles = []
        for j in range(NE):
            Ej = p_E.tile([npart, 129], F32, tag=f"E{j}")
            Etiles.append(Ej)
        cacc = p_acc.tile([npart, 1], F32, tag="C")
        for E in Etiles:
            nc.gpsimd.memset(E[:], 0.0)
        nc.gpsimd.memset(cacc[:], 0.0)
        # E[-1][-1] = 1 for group 0 (= partition block ch-1)
        nc.gpsimd.memset(Etiles[0][(ch - 1) * ni:ch * ni, 0:1], 1.0)

        ktbs = {}
        rec_pending = None  # (rec_tile,) scheduled for the next apply step

        def fetch_batch(bi):
            w0 = bi * sb
            ktb = p_k.tile([npart, sb * 128], F32, tag="ktb")
            # ktb[(ch-1-g)*ni + i, s*128 + c] = kbuf[i, w0+s-g*lag+pad, g*128+c]
            for g in range(ch):
                blk = ch - 1 - g
                src = kbuf.copy()
                src.ap = type(src.ap)([
                    [krows * t_len, ni],        # i
                    [t_len, sb],                # s (step within batch)
                    [1, 128],                   # c
                ])
                src.offset = (w0 - g * lag + pad) * t_len + g * 128
                nc.sync.dma_start(
                    ktb[blk * ni:(blk + 1) * ni, :]
                    .rearrange("i (s c) -> i s c", c=128), src)
            ktbs[bi] = ktb

        fetch_batch(0)
        fetch_batch(1)

        for w in range(nsteps):
            cur = Etiles[w % NE]      # rows w-1-g*lag (prev), written by scan w-1
            newt = Etiles[(w + 1) % NE]
            if w % sb == 0 and (w // sb) + 2 < nbatch:
                fetch_batch(w // sb + 2)

            # pipelined rescale: apply scale computed 8 steps ago
            if rec_pending is not None and w % RESC == 0:
                rec, = rec_pending
                rec_pending = None
                nc.vector.tensor_scalar_mul(cur[:, 0:129], cur[:, 0:129], rec[:])
                # boundary DMAs for steps w..w+lag-1 were issued pre-scale:
                # rescale their landing zones (col 0 of the dst tiles)
                for t in range(w, w + lag):
                    dst = Etiles[(t + 1) % NE]
                    nc.vector.tensor_scalar_mul(
                        dst[0:(ch - 1) * ni, 0:1], dst[0:(ch - 1) * ni, 0:1],
                        rec[0:(ch - 1) * ni])
                lgr = p_stat.tile([npart, 1], F32, tag="lgr")
                nc.scalar.activation(lgr[:], rec[:], AF.Ln)
                nc.vector.tensor_sub(cacc[:], cacc[:], lgr[:])

            s = p_s.tile([npart, 128], F32, tag="s")
            nc.vector.tensor_add(s[:], cur[:, 1:129], cur[:, 0:128])
            ktb = ktbs[w // sb]
            nc.vector.tensor_tensor_scan(
                newt[:, 1:129], s[:], ktb[:, (w % sb) * 128:(w % sb + 1) * 128],
                newt[:, 0:1], OP.add, OP.mult,
            )
            if w == 0:
                # clear the one-time E[-1][-1] = 1 seed (group 0 boundary is 0)
                nc.vector.memset(Etiles[0][(ch - 1) * ni:ch * ni, 0:1], 0.0)

            # boundary for step w+lag: E tile col 0 gets group g-1's scan
            # output boundary (partition shift by +ni) via SWDGE DMA
            if w + lag < nsteps:
                nc.gpsimd.dma_start(
                    Etiles[(w + lag + 1) % NE][0:(ch - 1) * ni, 0:1],
                    newt[ni:npart, 128:129])

            # pipelined rescale: compute scale from this step's rows
            if (w + 8) % RESC == 0 and (w + 8) <= 480:
                pmax = p_stat.tile([npart, 1], F32, tag="pmax")
                nc.vector.tensor_reduce(pmax[:], newt[:, 1:129], AX.X, OP.max)
                pmT = p_bc.tile([1, npart], F32, tag="bc")
                t1 = nc.tensor.transpose(pmT[:], pmax[:],
                                         ident[0:npart, 0:npart])
                mxrow = p_stat.tile([1, ni], F32, tag="mxrow")
                rd2 = nc.vector.tensor_reduce(
                    mxrow[:], pmT[:].rearrange("a (g i) -> a i g", i=ni),
                    AX.X, OP.max)
                add_dep_helper(rd2.ins, t1.ins, reason="reduce after PE T1")
                mxps = p_bc.tile([ni, 1], F32, tag="bc")
                t2 = nc.tensor.transpose(mxps[:], mxrow[:], ident[0:1, 0:1])
                mxcol = p_stat.tile([ni, 1], F32, tag="mxcol")
                cpm = nc.scalar.copy(mxcol[:], mxps[:])
                add_dep_helper(cpm.ins, t2.ins, reason="copy after PE T2")
                bc = p_bc.tile([npart, 1], F32, tag="bc")
                bc_mm = nc.tensor.matmul(bc[:], bcastM[:], mxcol[:],
                                         start=True, stop=True)
                rec = p_stat.tile([npart, 1], F32, tag="rec")
                rcp = nc.vector.reciprocal(rec[:], bc[:])
                add_dep_helper(rcp.ins, bc_mm.ins,
                               reason="recip after PE broadcast")
                rec_pending = (rec,)

        # group ch-1 (final column chunk) lives on partition block 0
        last = Etiles[nsteps % 2]
        nc.sync.dma_start(outs["EOUT"].rearrange("(a b) -> a b", b=1),
                          last[0:ni, 128:129])
        nc.sync.dma_start(outs["COUT"].rearrange("(a b) -> a b", b=1),
                          cacc[0:ni, 0:1])


def _build(t_len=T, bpc=BPC, resc=RESC, num_devices=NCORES, wave=False,
           wave2=False, lag=3, sb=8):
    ni = NTYPE * bpc
    nc = bacc.Bacc(
        "TRN2", target_bir_lowering=False, debug=False, num_devices=num_devices,
    )
    ins = {
        name: nc.dram_tensor(name, [bpc, t_len, D], F32, kind="ExternalInput").ap()
        for name in ("TGT", "OTH", "X")
    }
    outs = {
        "EOUT": nc.dram_tensor("EOUT", [ni], F32, kind="ExternalOutput").ap(),
        "COUT": nc.dram_tensor("COUT", [ni], F32, kind="ExternalOutput").ap(),
    }
    if wave2:
        ch = t_len // 128
        pad = (ch - 1) * lag
        krows = t_len + 2 * pad + sb
        kbuf = nc.dram_tensor("KBUF", [ni, krows, t_len], F32).ap()
        with tile.TileContext(nc) as tc:
            _emit_wave2(tc, ins, outs, kbuf, t_len, bpc, lag, sb)
    elif wave:
        kbuf = nc.dram_tensor("KBUF", [ni, t_len + 6, t_len], F32).ap()
        with tile.TileContext(nc) as tc:
            _emit_wave(tc, ins, outs, kbuf, t_len, bpc, resc)
    else:
        kbuf = nc.dram_tensor("KBUF", [ni, t_len, t_len], F32).ap()
        with tile.TileContext(nc) as tc:
            _emit(tc, ins, outs, kbuf, t_len, bpc, resc)
    nc.compile()
    return nc


_NC = None


def _get_nc():
    global _NC
    if _NC is None:
        kv = os.environ.get("KWAVE", "0")
        _NC = _build(wave=kv == "1", wave2=kv == "2")
    return _NC


def _postprocess(results, labels):
    E = np.stack([r["EOUT"] for r in results])  # [8, 32]
    C = np.stack([r["COUT"] for r in results])  # [8, 32]
    R = -(np.log(E) + C)                        # [core, type*8+b]
    R = R.reshape(NCORES, NTYPE, BPC).transpose(1, 0, 2).reshape(NTYPE, B)
    diff = (R[0] - R[1] - 0.5 * R[2] + 0.5 * R[3]).astype(np.float32)
    lab = np.asarray(labels, dtype=np.float32)
    return np.float32(np.mean((diff - lab) ** 2, dtype=np.float32))


def kernel(TGT, OTH, X, labels):
    nc = _get_nc()
    TGT = np.ascontiguousarray(np.asarray(TGT, dtype=np.float32))
    OTH = np.ascontiguousarray(np.asarray(OTH, dtype=np.float32))
    X = np.ascontiguousarray(np.asarray(X, dtype=np.float32))
    in_maps = [
        {
            "TGT": TGT[c * BPC:(c + 1) * BPC],
            "OTH": OTH[c * BPC:(c + 1) * BPC],
            "X": X[c * BPC:(c + 1) * BPC],
        }
        for c in range(NCORES)
    ]
    res = run_bass_kernel_spmd(nc, in_maps, core_ids=list(range(NCORES)))
    return _postprocess(res.results, labels)



# revision 24
# speedup vs baseline: 2.2849x; 1.2461x over previous
"""Soft-DTW ranking loss kernel for Trainium2 (8 NeuronCores, SPMD data parallel).

Math: loss = mean((diff - labels)^2) where
  diff_b = sdtw(OTH_b,X_b) - sdtw(TGT_b,X_b) - 0.5*sdtw(OTH_b,OTH_b) + 0.5*sdtw(TGT_b,TGT_b)
(the sdtw(X,X) terms of the normalized soft-DTW cancel exactly).

Soft-DTW (gamma=1) is computed in the probability domain:
  E[i,j] = K[i,j] * (E[i-1,j] + E[i-1,j-1] + E[i,j-1]),  K = exp(<xn_i,yn_j> - 1)
which maps one DP row onto a single DVE tensor_tensor_scan:
  state = (s[t] + state) * K[t],   s = E_prev + shift1(E_prev)
with periodic per-instance rescaling (log-scale accumulated in C) to stay in
fp32 range. Each core handles 8 batch items x 4 DTW instances = 32 independent
DPs vectorized across SBUF partitions.
"""

import os
import sys

import numpy as np

for _p in ("/root/.axon_site", "/root/.axon_site/_ro/trn_rl_repo",
           "/root/.axon_site/_ro/pypackages", "/opt/trn_rl_repo", "/opt/pypackages"):
    if os.path.isdir(_p) and _p not in sys.path:
        sys.path.append(_p)

import concourse.bass as bass
import concourse.tile as tile
from concourse.tile import add_dep_helper
from concourse import bacc, mybir
from concourse.bass_utils import run_bass_kernel_spmd
from concourse.masks import make_identity

F32 = mybir.dt.float32
F32R = mybir.dt.float32r
AX = mybir.AxisListType
OP = mybir.AluOpType
AF = mybir.ActivationFunctionType

B, T, D = 64, 512, 64
NCORES = 8
BPC = B // NCORES          # batch items per core
NTYPE = 4                  # (OTH,X), (TGT,X), (OTH,OTH), (TGT,TGT)
RESC = 32                  # rescale cadence (rows)


def _emit(tc: tile.TileContext, ins: dict, outs: dict, kbuf: bass.AP,
          t_len: int, bpc: int, resc: int):
    nc = tc.nc
    ni = NTYPE * bpc
    nrowt = t_len // 128

    with (
        tc.tile_pool(name="const", bufs=1) as p_const,
        tc.tile_pool(name="ain", bufs=2) as p_in,
        tc.tile_pool(name="astat", bufs=2) as p_astat,
        tc.tile_pool(name="asn", bufs=2) as p_asn,
        tc.tile_pool(name="apsT", bufs=2, space="PSUM") as p_psT,
        tc.tile_pool(name="ant", bufs=2) as p_nt,
        tc.tile_pool(name="aG", bufs=2, space="PSUM") as p_G,
        tc.tile_pool(name="aK", bufs=3) as p_K,
        tc.tile_pool(name="bE", bufs=1) as p_E,
        tc.tile_pool(name="bS", bufs=2) as p_s,
        tc.tile_pool(name="bK", bufs=4) as p_k,
        tc.tile_pool(name="bstat", bufs=2) as p_stat,
        tc.tile_pool(name="bacc", bufs=1) as p_acc,
    ):
        ident = p_const.tile([128, 128], F32, tag="ident")
        make_identity(nc, ident[:])
        bias_m1 = p_const.tile([128, 1], F32, tag="biasm1")
        nc.gpsimd.memset(bias_m1[:], -1.0)

        # ---------------- Phase A: K = exp(<xn,yn> - 1) for all pairs -------
        for b in range(bpc):
            nT = {}
            for sname in ("OTH", "TGT", "X"):
                src = ins[sname]
                xin = p_in.tile([128, nrowt * D], F32, tag=f"in_{sname}")
                nc.sync.dma_start(
                    xin[:].rearrange("p (t d) -> p t d", d=D),
                    src[b].rearrange("(t p) d -> p t d", p=128),
                )
                sq = p_astat.tile([128, nrowt * D], F32, tag=f"sq_{sname}")
                ss = p_astat.tile([128, nrowt], F32, tag=f"ss_{sname}")
                for t in range(nrowt):
                    nc.scalar.activation(
                        sq[:, t * D:(t + 1) * D], xin[:, t * D:(t + 1) * D],
                        AF.Square, accum_out=ss[:, t:t + 1],
                    )
                nrm = p_astat.tile([128, nrowt], F32, tag=f"nrm_{sname}")
                nc.scalar.activation(nrm[:], ss[:], AF.Sqrt)
                rnm = p_astat.tile([128, nrowt], F32, tag=f"rnm_{sname}")
                nc.vector.reciprocal(rnm[:], nrm[:])
                sn = p_asn.tile([128, nrowt * D], F32, tag=f"sn_{sname}")
                for t in range(nrowt):
                    nc.vector.tensor_scalar_mul(
                        sn[:, t * D:(t + 1) * D], xin[:, t * D:(t + 1) * D],
                        rnm[:, t:t + 1],
                    )
                snT = p_nt.tile([D, t_len], F32R, tag=f"nt_{sname}")
                for t in range(nrowt):
                    tp = p_psT.tile([D, 128], F32, tag="psT")
                    nc.tensor.transpose(tp[:], sn[:, t * D:(t + 1) * D], ident[:])
                    nc.scalar.copy(snT[:, t * 128:(t + 1) * 128], tp[:])
                nT[sname] = snT

            pairs = [("OTH", "X"), ("TGT", "X"), ("OTH", "OTH"), ("TGT", "TGT")]
            for ptype, (an, cn) in enumerate(pairs):
                inst = ptype * bpc + b
                aT, cT = nT[an], nT[cn]
                for rt in range(nrowt):
                    g = p_G.tile([128, t_len], F32, tag="G")
                    nc.tensor.matmul(
                        g[:],
                        aT[:, rt * 128:(rt + 1) * 128],
                        cT[:],
                        start=True, stop=True,
                    )
                    kt = p_K.tile([128, t_len], F32, tag="K")
                    nc.scalar.activation(kt[:], g[:], AF.Exp, bias=bias_m1[:])
                    nc.sync.dma_start(kbuf[inst, rt * 128:(rt + 1) * 128, :], kt[:])

        # ---------------- Phase B: row-scan DP over all instances -----------
        Ea = p_E.tile([ni, t_len + 1], F32, tag="Ea")
        Eb = p_E.tile([ni, t_len + 1], F32, tag="Eb")
        cacc = p_acc.tile([ni, 1], F32, tag="C")
        nc.gpsimd.memset(Ea[:], 0.0)
        nc.gpsimd.memset(Eb[:], 0.0)
        nc.gpsimd.memset(cacc[:], 0.0)
        nc.gpsimd.memset(Ea[:, 0:1], 1.0)  # E[-1][-1] = exp(-0)

        cur, nxt = Ea, Eb
        for r in range(t_len):
            kt = p_k.tile([ni, t_len], F32, tag="krow")
            nc.sync.dma_start(kt[:], kbuf[:, r, :])
            s = p_s.tile([ni, t_len], F32, tag="s")
            nc.vector.tensor_add(s[:], cur[:, 1:t_len + 1], cur[:, 0:t_len])
            nc.vector.tensor_tensor_scan(
                nxt[:, 1:t_len + 1], s[:], kt[:], 0.0, OP.add, OP.mult,
            )
            if r == 0:
                # E[0][-1] = 0: clear the one-time E[-1][-1] = 1 boundary
                nc.vector.memset(Ea[:, 0:1], 0.0)
            if (r + 1) % resc == 0 and r != t_len - 1:
                mx = p_stat.tile([ni, 1], F32, tag="mx")
                nc.vector.tensor_reduce(mx[:], nxt[:, 1:t_len + 1], AX.X, OP.max)
                rec = p_stat.tile([ni, 1], F32, tag="rec")
                nc.vector.reciprocal(rec[:], mx[:])
                nc.vector.tensor_scalar_mul(nxt[:, 1:t_len + 1],
                                            nxt[:, 1:t_len + 1], rec[:])
                lg = p_stat.tile([ni, 1], F32, tag="lg")
                nc.scalar.activation(lg[:], mx[:], AF.Ln)
                nc.vector.tensor_add(cacc[:], cacc[:], lg[:])
            cur, nxt = nxt, cur

        nc.sync.dma_start(outs["EOUT"].rearrange("(a b) -> a b", b=1),
                          cur[:, t_len:t_len + 1])
        nc.sync.dma_start(outs["COUT"].rearrange("(a b) -> a b", b=1), cacc[:])


def _emit_wave(tc: tile.TileContext, ins: dict, outs: dict, kbuf: bass.AP,
               t_len: int, bpc: int, resc: int):
    """Wavefront DP: CH=t_len/128 column chunks on partition groups.

    Partition p = g*ni + inst handles column chunk g of instance inst.
    Wavefront step w: group g processes row r = w - g (K rows padded with 3
    zero rows on each side so inactive groups compute zeros). Cross-chunk
    carries (scan initial / shifted-row boundary) move between partition
    groups via a constant shift matmul on the (otherwise idle) PE.
    """
    nc = tc.nc
    ni = NTYPE * bpc
    ch = t_len // 128
    npart = ch * ni
    nrowt = ch
    nsteps = t_len + ch - 1

    with (
        tc.tile_pool(name="const", bufs=1) as p_const,
        tc.tile_pool(name="ain", bufs=2) as p_in,
        tc.tile_pool(name="astat", bufs=2) as p_astat,
        tc.tile_pool(name="asn", bufs=2) as p_asn,
        tc.tile_pool(name="apsT", bufs=2, space="PSUM") as p_psT,
        tc.tile_pool(name="ant", bufs=2) as p_nt,
        tc.tile_pool(name="aG", bufs=2, space="PSUM") as p_G,
        tc.tile_pool(name="aK", bufs=3) as p_K,
        tc.tile_pool(name="bE", bufs=1) as p_E,
        tc.tile_pool(name="bS", bufs=2) as p_s,
        tc.tile_pool(name="bK", bufs=8) as p_k,
        tc.tile_pool(name="bC", bufs=3, space="PSUM") as p_carry,
        tc.tile_pool(name="bB", bufs=1, space="PSUM") as p_bc,
        tc.tile_pool(name="bstat", bufs=2) as p_stat,
        tc.tile_pool(name="bacc", bufs=1) as p_acc,
    ):
        ident = p_const.tile([128, 128], F32, tag="ident")
        make_identity(nc, ident[:])
        bias_m1 = p_const.tile([128, 1], F32, tag="biasm1")
        nc.gpsimd.memset(bias_m1[:], -1.0)
        # shiftM[k, p] = 1 iff k == p - ni  (moves group g-1 -> g)
        shiftM = p_const.tile([npart, npart], F32, tag="shiftM")
        nc.gpsimd.memset(shiftM[:], 0.0)
        nc.gpsimd.affine_select(
            out=shiftM[:], in_=shiftM[:], compare_op=OP.not_equal, fill=1.0,
            base=ni, pattern=[[-1, npart]], channel_multiplier=1,
        )
        # bcastM[k, (g, j)] = 1 iff k == j  (broadcast group-0 col to all groups)
        bcastM = p_const.tile([ni, npart], F32, tag="bcastM")
        nc.gpsimd.memset(bcastM[:], 0.0)
        nc.gpsimd.affine_select(
            out=bcastM[:].rearrange("k (g j) -> k g j", j=ni),
            in_=bcastM[:].rearrange("k (g j) -> k g j", j=ni),
            compare_op=OP.not_equal, fill=1.0,
            base=0, pattern=[[0, ch], [-1, ni]], channel_multiplier=1,
        )

        # zero the 3+3 pad rows of kbuf (layout [ni, t_len+6, t_len])
        zpad = p_const.tile([ni, 3 * t_len], F32, tag="zpad")
        nc.gpsimd.memset(zpad[:], 0.0)
        nc.sync.dma_start(
            kbuf[:, 0:3, :].rearrange("i r c -> i (r c)"), zpad[:])
        nc.sync.dma_start(
            kbuf[:, t_len + 3:t_len + 6, :].rearrange("i r c -> i (r c)"), zpad[:])

        # ---------------- Phase A (same as v1, +3 row offset into kbuf) -----
        for b in range(bpc):
            nT = {}
            for sname in ("OTH", "TGT", "X"):
                src = ins[sname]
                xin = p_in.tile([128, nrowt * D], F32, tag=f"in_{sname}")
                nc.sync.dma_start(
                    xin[:].rearrange("p (t d) -> p t d", d=D),
                    src[b].rearrange("(t p) d -> p t d", p=128),
                )
                sq = p_astat.tile([128, nrowt * D], F32, tag=f"sq_{sname}")
                ss = p_astat.tile([128, nrowt], F32, tag=f"ss_{sname}")
                for t in range(nrowt):
                    nc.scalar.activation(
                        sq[:, t * D:(t + 1) * D], xin[:, t * D:(t + 1) * D],
                        AF.Square, accum_out=ss[:, t:t + 1],
                    )
                nrm = p_astat.tile([128, nrowt], F32, tag=f"nrm_{sname}")
                nc.scalar.activation(nrm[:], ss[:], AF.Sqrt)
                rnm = p_astat.tile([128, nrowt], F32, tag=f"rnm_{sname}")
                nc.vector.reciprocal(rnm[:], nrm[:])
                sn = p_asn.tile([128, nrowt * D], F32, tag=f"sn_{sname}")
                for t in range(nrowt):
                    nc.vector.tensor_scalar_mul(
                        sn[:, t * D:(t + 1) * D], xin[:, t * D:(t + 1) * D],
                        rnm[:, t:t + 1],
                    )
                snT = p_nt.tile([D, t_len], F32R, tag=f"nt_{sname}")
                for t in range(nrowt):
                    tp = p_psT.tile([D, 128], F32, tag="psT")
                    nc.tensor.transpose(tp[:], sn[:, t * D:(t + 1) * D], ident[:])
                    nc.scalar.copy(snT[:, t * 128:(t + 1) * 128], tp[:])
                nT[sname] = snT

            pairs = [("OTH", "X"), ("TGT", "X"), ("OTH", "OTH"), ("TGT", "TGT")]
            for ptype, (an, cn) in enumerate(pairs):
                inst = ptype * bpc + b
                aT, cT = nT[an], nT[cn]
                for rt in range(nrowt):
                    g = p_G.tile([128, t_len], F32, tag="G")
                    nc.tensor.matmul(
                        g[:], aT[:, rt * 128:(rt + 1) * 128], cT[:],
                        start=True, stop=True,
                    )
                    kt = p_K.tile([128, t_len], F32, tag="K")
                    nc.scalar.activation(kt[:], g[:], AF.Exp, bias=bias_m1[:])
                    nc.sync.dma_start(
                        kbuf[inst, 3 + rt * 128:3 + (rt + 1) * 128, :], kt[:])

        # ---------------- Phase B: wavefront row-scan -----------------------
        Ea = p_E.tile([npart, 129], F32, tag="Ea")
        Eb = p_E.tile([npart, 129], F32, tag="Eb")
        Etiles = [Ea, Eb]
        cacc = p_acc.tile([npart, 1], F32, tag="C")
        nc.gpsimd.memset(Ea[:], 0.0)
        nc.gpsimd.memset(Eb[:], 0.0)
        nc.gpsimd.memset(cacc[:], 0.0)
        nc.gpsimd.memset(Ea[0:ni, 0:1], 1.0)  # E[-1][-1] = 1 for group 0
        car_prev = p_carry.tile([npart, 1], F32, tag="car")
        car_prev_mm = nc.vector.memset(car_prev[:], 0.0)

        for w in range(nsteps):
            prev = Etiles[w % 2]
            newt = Etiles[(w + 1) % 2]
            kt = p_k.tile([npart, 128], F32, tag="krow")
            for g in range(ch):
                nc.sync.dma_start(
                    kt[g * ni:(g + 1) * ni, :],
                    kbuf[:, w - g + 3, g * 128:(g + 1) * 128],
                )
            s = p_s.tile([npart, 128], F32, tag="s")
            nc.vector.tensor_add(s[:], prev[:, 1:129], prev[:, 0:128])
            scan_i = nc.vector.tensor_tensor_scan(
                newt[:, 1:129], s[:], kt[:], car_prev[:, 0:1],
                OP.add, OP.mult,
            )
            add_dep_helper(scan_i.ins, car_prev_mm.ins,
                           reason="scan initial after PE carry shift")
            if (w + 1) % resc == 0 and w + 1 < t_len:
                # per-partition chunk max -> per-instance max across groups
                pmax = p_stat.tile([npart, 1], F32, tag="pmax")
                nc.vector.tensor_reduce(pmax[:], newt[:, 1:129], AX.X, OP.max)
                pmT = p_bc.tile([1, npart], F32, tag="bc")
                t1 = nc.tensor.transpose(pmT[:], pmax[:],
                                         ident[0:npart, 0:npart])
                mxrow = p_stat.tile([1, ni], F32, tag="mxrow")
                rd2 = nc.vector.tensor_reduce(
                    mxrow[:], pmT[:].rearrange("a (g i) -> a i g", i=ni),
                    AX.X, OP.max)
                add_dep_helper(rd2.ins, t1.ins, reason="reduce after PE T1")
                mxps = p_bc.tile([ni, 1], F32, tag="bc")
                t2 = nc.tensor.transpose(mxps[:], mxrow[:], ident[0:1, 0:1])
                mxcol = p_stat.tile([ni, 1], F32, tag="mxcol")
                cpm = nc.scalar.copy(mxcol[:], mxps[:])
                add_dep_helper(cpm.ins, t2.ins, reason="copy after PE T2")
                bc = p_bc.tile([npart, 1], F32, tag="bc")
                bc_mm = nc.tensor.matmul(bc[:], bcastM[:], mxcol[:],
                                         start=True, stop=True)
                rec = p_stat.tile([npart, 1], F32, tag="rec")
                rcp = nc.vector.reciprocal(rec[:], bc[:])
                add_dep_helper(rcp.ins, bc_mm.ins,
                               reason="recip after PE broadcast")
                nc.vector.tensor_scalar_mul(newt[:, 0:129], newt[:, 0:129], rec[:])
                lgr = p_stat.tile([npart, 1], F32, tag="lgr")
                nc.scalar.activation(lgr[:], rec[:], AF.Ln)
                nc.vector.tensor_sub(cacc[:], cacc[:], lgr[:])
            car = p_carry.tile([npart, 1], F32, tag="car")
            car_mm = nc.tensor.matmul(car[:], shiftM[:], newt[:, 128:129],
                                      start=True, stop=True)
            cp = nc.scalar.copy(prev[:, 0:1], car[:])
            add_dep_helper(cp.ins, car_mm.ins,
                           reason="carry copy after PE shift")
            car_prev = car
            car_prev_mm = car_mm

        last = Etiles[nsteps % 2]
        nc.sync.dma_start(outs["EOUT"].rearrange("(a b) -> a b", b=1),
                          last[(ch - 1) * ni:ch * ni, 128:129])
        nc.sync.dma_start(outs["COUT"].rearrange("(a b) -> a b", b=1),
                          cacc[(ch - 1) * ni:ch * ni, 0:1])


def _emit_wave2(tc: tile.TileContext, ins: dict, outs: dict, kbuf: bass.AP,
                t_len: int, bpc: int, lag: int, sb: int):
    """Wavefront DP v2: lagged chunks + batched kt DMA + pipelined rescale.

    Group g processes row w - g*lag at step w (lag>=2 gives the PE carry
    shift and ACT boundary copy slack off the DVE critical path).  kt rows
    for `sb` consecutive steps are fetched in ONE diagonal-AP DMA.  The
    rescale max is computed 8 steps before it is applied, so its reduce/
    transpose/broadcast chain also runs off the critical path.
    """
    nc = tc.nc
    ni = NTYPE * bpc
    ch = t_len // 128
    npart = ch * ni
    nrowt = ch
    pad = (ch - 1) * lag
    nsteps = t_len + pad
    nbatch = (nsteps + sb - 1) // sb
    krows = t_len + 2 * pad + sb  # top pad + rows + bottom pad (incl DMA overrun)

    with (
        tc.tile_pool(name="const", bufs=1) as p_const,
    ):
        ident = p_const.tile([128, 128], F32, tag="ident")
        make_identity(nc, ident[:])
        bias_m1 = p_const.tile([128, 1], F32, tag="biasm1")
        nc.gpsimd.memset(bias_m1[:], -1.0)
        # Group g lives on partition block (ch-1-g); carries move to the next
        # block via a partition-shift DMA (no PE involvement).
        # bcastM[k, (g, j)] = 1 iff k == j  (broadcast per-inst col to all groups)
        bcastM = p_const.tile([ni, npart], F32, tag="bcastM")
        nc.gpsimd.memset(bcastM[:], 0.0)
        nc.gpsimd.affine_select(
            out=bcastM[:].rearrange("k (g j) -> k g j", j=ni),
            in_=bcastM[:].rearrange("k (g j) -> k g j", j=ni),
            compare_op=OP.not_equal, fill=1.0,
            base=0, pattern=[[0, ch], [-1, ni]], channel_multiplier=1,
        )

        # zero the pad rows of kbuf (layout [ni, krows, t_len])
        nbot = krows - t_len - pad
        zpad = p_const.tile([ni, nbot * 512], F32, tag="zpad")
        nc.gpsimd.memset(zpad[:], 0.0)
        nc.sync.dma_start(
            kbuf[:, 0:pad, :].rearrange("i r c -> i (r c)"),
            zpad[:, 0:pad * 512])
        nc.sync.dma_start(
            kbuf[:, t_len + pad:krows, :].rearrange("i r c -> i (r c)"),
            zpad[:])

        # ---------------- Phase A (as v1, +pad row offset into kbuf) --------
        with (
            tc.tile_pool(name="ain", bufs=2) as p_in,
            tc.tile_pool(name="astat", bufs=2) as p_astat,
            tc.tile_pool(name="asn", bufs=2) as p_asn,
            tc.tile_pool(name="apsT", bufs=2, space="PSUM") as p_psT,
            tc.tile_pool(name="ant", bufs=2) as p_nt,
            tc.tile_pool(name="aG", bufs=2, space="PSUM") as p_G,
            tc.tile_pool(name="aK", bufs=3) as p_K,
        ):
            _emit_phaseA(tc, ins, kbuf, t_len, bpc, pad,
                         p_in, p_astat, p_asn, p_psT, p_nt, p_G, p_K,
                         ident, bias_m1)

        # ---------------- Phase B: lagged wavefront row-scan ----------------
        with (
            tc.tile_pool(name="bE", bufs=1) as p_E,
            tc.tile_pool(name="bS", bufs=2) as p_s,
            tc.tile_pool(name="bK", bufs=4) as p_k,
            tc.tile_pool(name="bB", bufs=1, space="PSUM") as p_bc,
            tc.tile_pool(name="bstat", bufs=4) as p_stat,
            tc.tile_pool(name="bacc", bufs=1) as p_acc,
        ):
            _emit_phaseB(tc, outs, kbuf, t_len, bpc, lag, sb,
                         p_E, p_s, p_k, p_bc, p_stat, p_acc,
                         ident, bcastM)


def _emit_phaseA(tc, ins, kbuf, t_len, bpc, pad,
                 p_in, p_astat, p_asn, p_psT, p_nt, p_G, p_K,
                 ident, bias_m1):
    nc = tc.nc
    nrowt = t_len // 128
    if True:
        for b in range(bpc):
            nT = {}
            for sname in ("OTH", "TGT", "X"):
                src = ins[sname]
                xin = p_in.tile([128, nrowt * D], F32, tag=f"in_{sname}")
                nc.sync.dma_start(
                    xin[:].rearrange("p (t d) -> p t d", d=D),
                    src[b].rearrange("(t p) d -> p t d", p=128),
                )
                sq = p_astat.tile([128, nrowt * D], F32, tag=f"sq_{sname}")
                ss = p_astat.tile([128, nrowt], F32, tag=f"ss_{sname}")
                for t in range(nrowt):
                    nc.scalar.activation(
                        sq[:, t * D:(t + 1) * D], xin[:, t * D:(t + 1) * D],
                        AF.Square, accum_out=ss[:, t:t + 1],
                    )
                nrm = p_astat.tile([128, nrowt], F32, tag=f"nrm_{sname}")
                nc.scalar.activation(nrm[:], ss[:], AF.Sqrt)
                rnm = p_astat.tile([128, nrowt], F32, tag=f"rnm_{sname}")
                nc.vector.reciprocal(rnm[:], nrm[:])
                sn = p_asn.tile([128, nrowt * D], F32, tag=f"sn_{sname}")
                for t in range(nrowt):
                    nc.vector.tensor_scalar_mul(
                        sn[:, t * D:(t + 1) * D], xin[:, t * D:(t + 1) * D],
                        rnm[:, t:t + 1],
                    )
                snT = p_nt.tile([D, t_len], F32R, tag=f"nt_{sname}")
                for t in range(nrowt):
                    tp = p_psT.tile([D, 128], F32, tag="psT")
                    nc.tensor.transpose(tp[:], sn[:, t * D:(t + 1) * D], ident[:])
                    nc.scalar.copy(snT[:, t * 128:(t + 1) * 128], tp[:])
                nT[sname] = snT

            pairs = [("OTH", "X"), ("TGT", "X"), ("OTH", "OTH"), ("TGT", "TGT")]
            for ptype, (an, cn) in enumerate(pairs):
                inst = ptype * bpc + b
                aT, cT = nT[an], nT[cn]
                for rt in range(nrowt):
                    g = p_G.tile([128, t_len], F32, tag="G")
                    nc.tensor.matmul(
                        g[:], aT[:, rt * 128:(rt + 1) * 128], cT[:],
                        start=True, stop=True,
                    )
                    kt = p_K.tile([128, t_len], F32, tag="K")
                    nc.scalar.activation(kt[:], g[:], AF.Exp, bias=bias_m1[:])
                    nc.sync.dma_start(
                        kbuf[inst, pad + rt * 128:pad + (rt + 1) * 128, :], kt[:])


def _emit_phaseB(tc, outs, kbuf, t_len, bpc, lag, sb,
                 p_E, p_s, p_k, p_bc, p_stat, p_acc,
                 ident, bcastM):
    nc = tc.nc
    ni = NTYPE * bpc
    ch = t_len // 128
    npart = ch * ni
    pad = (ch - 1) * lag
    nsteps = t_len + pad
    nbatch = (nsteps + sb - 1) // sb
    krows = t_len + 2 * pad + sb
    if True:
        NE = 4
        Etiles = []
        for j in range(NE):
            Ej = p_E.tile([npart, 129], F32, tag=f"E{j}")
            Etiles.append(Ej)
        cacc = p_acc.tile([npart, 1], F32, tag="C")
        for E in Etiles:
            nc.gpsimd.memset(E[:], 0.0)
        nc.gpsimd.memset(cacc[:], 0.0)
        # E[-1][-1] = 1 for group 0 (= partition block ch-1)
        nc.gpsimd.memset(Etiles[0][(ch - 1) * ni:ch * ni, 0:1], 1.0)

        ktbs = {}
        rec_pending = None  # (rec_tile,) scheduled for the next apply step

        def fetch_batch(bi):
            w0 = bi * sb
            ktb = p_k.tile([npart, sb * 128], F32, tag="ktb")
            # ktb[(ch-1-g)*ni + i, s*128 + c] = kbuf[i, w0+s-g*lag+pad, g*128+c]
            for g in range(ch):
                blk = ch - 1 - g
                src = kbuf.copy()
                src.ap = type(src.ap)([
                    [krows * t_len, ni],        # i
                    [t_len, sb],                # s (step within batch)
                    [1, 128],                   # c
                ])
                src.offset = (w0 - g * lag + pad) * t_len + g * 128
                nc.sync.dma_start(
                    ktb[blk * ni:(blk + 1) * ni, :]
                    .rearrange("i (s c) -> i s c", c=128), src)
            ktbs[bi] = ktb

        fetch_batch(0)
        fetch_batch(1)

        for w in range(nsteps):
            cur = Etiles[w % NE]      # rows w-1-g*lag (prev), written by scan w-1
            newt = Etiles[(w + 1) % NE]
            if w % sb == 0 and (w // sb) + 2 < nbatch:
                fetch_batch(w // sb + 2)

            # pipelined rescale: apply scale computed 8 steps ago
            if rec_pending is not None and w % RESC == 0:
                rec, = rec_pending
                rec_pending = None
                nc.vector.tensor_scalar_mul(cur[:, 0:129], cur[:, 0:129], rec[:])
                # boundary DMAs for steps w..w+lag-1 were issued pre-scale:
                # rescale their landing zones (col 0 of the dst tiles)
                for t in range(w, w + lag):
                    dst = Etiles[(t + 1) % NE]
                    nc.vector.tensor_scalar_mul(
                        dst[0:(ch - 1) * ni, 0:1], dst[0:(ch - 1) * ni, 0:1],
                        rec[0:(ch - 1) * ni])
                lgr = p_stat.tile([npart, 1], F32, tag="lgr")
                nc.scalar.activation(lgr[:], rec[:], AF.Ln)
                nc.vector.tensor_sub(cacc[:], cacc[:], lgr[:])

            s = p_s.tile([npart, 128], F32, tag="s")
            nc.vector.tensor_add(s[:], cur[:, 1:129], cur[:, 0:128])
            ktb = ktbs[w // sb]
            nc.vector.tensor_tensor_scan(
                newt[:, 1:129], s[:], ktb[:, (w % sb) * 128:(w % sb + 1) * 128],
                newt[:, 0:1], OP.add, OP.mult,
            )
            if w == 0:
                # clear the one-time E[-1][-1] = 1 seed (group 0 boundary is 0)
                nc.vector.memset(Etiles[0][(ch - 1) * ni:ch * ni, 0:1], 0.0)

            # boundary for step w+lag: E tile col 0 gets group g-1's scan
            # output boundary (partition shift by +ni) via SWDGE DMA
            if w + lag < nsteps:
                nc.gpsimd.dma_start(
                    Etiles[(w + lag + 1) % NE][0:(ch - 1) * ni, 0:1],
                    newt[ni:npart, 128:129])

            # pipelined rescale: compute scale from this step's rows
            if (w + 8) % RESC == 0 and (w + 8) <= 480:
                pmax = p_stat.tile([npart, 1], F32, tag="pmax")
                nc.vector.tensor_reduce(pmax[:], newt[:, 1:129], AX.X, OP.max)
                pmT = p_bc.tile([1, npart], F32, tag="bc")
                t1 = nc.tensor.transpose(pmT[:], pmax[:],
                                         ident[0:npart, 0:npart])
                mxrow = p_stat.tile([1, ni], F32, tag="mxrow")
                rd2 = nc.vector.tensor_reduce(
                    mxrow[:], pmT[:].rearrange("a (g i) -> a i g", i=ni),
                    AX.X, OP.max)
                add_dep_helper(rd2.ins, t1.ins, reason="reduce after PE T1")
                mxps = p_bc.tile([ni, 1], F32, tag="bc")
                t2 = nc.tensor.transpose(mxps[:], mxrow[:], ident[0:1, 0:1])
                mxcol = p_stat.tile([ni, 1], F32, tag="mxcol")
                cpm = nc.scalar.copy(mxcol[:], mxps[:])
                add_dep_helper(cpm.ins, t2.ins, reason="copy after PE T2")
                bc = p_bc.tile([npart, 1], F32, tag="bc")
                bc_mm = nc.tensor.matmul(bc[:], bcastM[:], mxcol[:],
                                         start=True, stop=True)
                rec = p_stat.tile([npart, 1], F32, tag="rec")
                rcp = nc.vector.reciprocal(rec[:], bc[:])
                add_dep_helper(rcp.ins, bc_mm.ins,
                               reason="recip after PE broadcast")
                rec_pending = (rec,)

        # group ch-1 (final column chunk) lives on partition block 0
        last = Etiles[nsteps % 2]
        nc.sync.dma_start(outs["EOUT"].rearrange("(a b) -> a b", b=1),
                          last[0:ni, 128:129])
        nc.sync.dma_start(outs["COUT"].rearrange("(a b) -> a b", b=1),
                          cacc[0:ni, 0:1])


def _emit_wave3(tc: tile.TileContext, ins: dict, outs: dict, kbs: list,
                t_len: int, bpc: int, lag: int, sb: int):
    """Wavefront DP v3: phase A (K production) overlapped under phase B.

    kbs: 4 per-row-tile DRAM tensors (row-tile granular dep tracking).
    kbs[0] holds global rows [-pad, 128) at local r+pad; kbs[1]/[2] rows
    [128,256)/[256,384); kbs[3] rows [384, 512+pad+sb) incl bottom pad.
    Carries move between column-chunk groups via partition-shift DMAs on
    the gpsimd (SWDGE) queue; NE=8 E-tile rotation gives them slack.
    """
    nc = tc.nc
    ni = NTYPE * bpc
    ch = t_len // 128
    npart = ch * ni
    nrowt = ch
    pad = (ch - 1) * lag
    nsteps = t_len + pad
    nbatch = (nsteps + sb - 1) // sb
    NE = 8
    rt_base = [-pad, 128, 256, 384]
    rt_rows = [128 + pad, 128, 128, 128 + pad + sb]

    def rt_of(r):
        return max(0, min(3, r // 128))

    with (
        tc.tile_pool(name="const", bufs=1) as p_const,
        tc.tile_pool(name="ant", bufs=1) as p_nt,
        tc.tile_pool(name="aG", bufs=2, space="PSUM") as p_G,
        tc.tile_pool(name="aK", bufs=3) as p_K,
        tc.tile_pool(name="bE", bufs=1) as p_E,
        tc.tile_pool(name="bS", bufs=2) as p_s,
        tc.tile_pool(name="bK", bufs=4) as p_k,
        tc.tile_pool(name="bB", bufs=1, space="PSUM") as p_bc,
        tc.tile_pool(name="bstat", bufs=4) as p_stat,
        tc.tile_pool(name="bacc", bufs=1) as p_acc,
    ):
        ident = p_const.tile([128, 128], F32, tag="ident")
        make_identity(nc, ident[:])
        bias_m1 = p_const.tile([128, 1], F32, tag="biasm1")
        nc.gpsimd.memset(bias_m1[:], -1.0)
        bcastM = p_const.tile([ni, npart], F32, tag="bcastM")
        nc.gpsimd.memset(bcastM[:], 0.0)
        nc.gpsimd.affine_select(
            out=bcastM[:].rearrange("k (g j) -> k g j", j=ni),
            in_=bcastM[:].rearrange("k (g j) -> k g j", j=ni),
            compare_op=OP.not_equal, fill=1.0,
            base=0, pattern=[[0, ch], [-1, ni]], channel_multiplier=1,
        )

        # zero pads: top of kbs[0] (pad rows), bottom of kbs[3] (pad+sb rows)
        zp = p_const.tile([ni, 4 * t_len], F32, tag="zp")
        nc.gpsimd.memset(zp[:], 0.0)
        for lo in range(0, pad, 4):
            n = min(4, pad - lo)
            nc.sync.dma_start(
                kbs[0][:, lo:lo + n, :].rearrange("i r c -> i (r c)"),
                zp[:, 0:n * t_len])
        for lo in range(128, 128 + pad + sb, 4):
            n = min(4, 128 + pad + sb - lo)
            nc.sync.dma_start(
                kbs[3][:, lo:lo + n, :].rearrange("i r c -> i (r c)"),
                zp[:, 0:n * t_len])

        # ---- preamble: normalized+transposed sequences for all items ------
        nts = {}
        with (
            tc.tile_pool(name="ain", bufs=2) as p_in,
            tc.tile_pool(name="astat", bufs=2) as p_astat,
            tc.tile_pool(name="asn", bufs=2) as p_asn,
            tc.tile_pool(name="apsT", bufs=2, space="PSUM") as p_psT,
        ):
            for b in range(bpc):
                for sname in ("OTH", "TGT", "X"):
                    src = ins[sname]
                    xin = p_in.tile([128, nrowt * D], F32, tag=f"in_{sname}")
                    nc.sync.dma_start(
                        xin[:].rearrange("p (t d) -> p t d", d=D),
                        src[b].rearrange("(t p) d -> p t d", p=128),
                    )
                    sq = p_astat.tile([128, nrowt * D], F32, tag=f"sq_{sname}")
                    ss = p_astat.tile([128, nrowt], F32, tag=f"ss_{sname}")
                    for t in range(nrowt):
                        nc.scalar.activation(
                            sq[:, t * D:(t + 1) * D], xin[:, t * D:(t + 1) * D],
                            AF.Square, accum_out=ss[:, t:t + 1],
                        )
                    nrm = p_astat.tile([128, nrowt], F32, tag=f"nrm_{sname}")
                    nc.scalar.activation(nrm[:], ss[:], AF.Sqrt)
                    rnm = p_astat.tile([128, nrowt], F32, tag=f"rnm_{sname}")
                    nc.vector.reciprocal(rnm[:], nrm[:])
                    sn = p_asn.tile([128, nrowt * D], F32, tag=f"sn_{sname}")
                    for t in range(nrowt):
                        nc.vector.tensor_scalar_mul(
                            sn[:, t * D:(t + 1) * D], xin[:, t * D:(t + 1) * D],
                            rnm[:, t:t + 1],
                        )
                    snT = p_nt.tile([D, t_len], F32R, tag=f"nt_{b}_{sname}")
                    for t in range(nrowt):
                        tp = p_psT.tile([D, 128], F32, tag="psT")
                        nc.tensor.transpose(tp[:], sn[:, t * D:(t + 1) * D],
                                            ident[:])
                        nc.scalar.copy(snT[:, t * 128:(t + 1) * 128], tp[:])
                    nts[(b, sname)] = snT

        pairs = [("OTH", "X"), ("TGT", "X"), ("OTH", "OTH"), ("TGT", "TGT")]

        def produce(rt):
            loc = rt * 128 - rt_base[rt]
            for b in range(bpc):
                for ptype, (an, cn) in enumerate(pairs):
                    inst = ptype * bpc + b
                    aT, cT = nts[(b, an)], nts[(b, cn)]
                    g = p_G.tile([128, t_len], F32, tag="G")
                    nc.tensor.matmul(
                        g[:], aT[:, rt * 128:(rt + 1) * 128], cT[:],
                        start=True, stop=True,
                    )
                    kt = p_K.tile([128, t_len], F32, tag="K")
                    nc.scalar.activation(kt[:], g[:], AF.Exp, bias=bias_m1[:])
                    nc.scalar.dma_start(kbs[rt][inst, loc:loc + 128, :], kt[:])

        produce(0)
        produce(1)

        # ---------------- Phase B ------------------------------------------
        Etiles = []
        for j in range(NE):
            Ej = p_E.tile([npart, 129], F32, tag=f"E{j}")
            Etiles.append(Ej)
        cacc = p_acc.tile([npart, 1], F32, tag="C")
        for E in Etiles:
            nc.gpsimd.memset(E[:], 0.0)
        nc.gpsimd.memset(cacc[:], 0.0)
        nc.gpsimd.memset(Etiles[0][(ch - 1) * ni:ch * ni, 0:1], 1.0)

        ktbs = {}
        rec_pending = None

        def fetch_batch(bi):
            w0 = bi * sb
            ktb = p_k.tile([npart, sb * 128], F32, tag="ktb")
            # ktb[(ch-1-g)*ni+i, s*128+c] = K[i][row w0+s-g*lag][g*128+c]
            for g in range(ch):
                blk = ch - 1 - g
                r_lo = w0 - g * lag
                s = 0
                while s < sb:
                    r = r_lo + s
                    rt = rt_of(r)
                    krt = kbs[rt]
                    n = min(sb - s, rt_base[rt] + rt_rows[rt] - r)
                    src = krt.copy()
                    src.ap = type(src.ap)([
                        [rt_rows[rt] * t_len, ni],
                        [t_len, n],
                        [1, 128],
                    ])
                    src.offset = (r - rt_base[rt]) * t_len + g * 128
                    nc.sync.dma_start(
                        ktb[blk * ni:(blk + 1) * ni, s * 128:(s + n) * 128]
                        .rearrange("i (s c) -> i s c", c=128), src)
                    s += n
            ktbs[bi] = ktb

        fetch_batch(0)
        fetch_batch(1)

        for w in range(nsteps):
            cur = Etiles[w % NE]
            newt = Etiles[(w + 1) % NE]
            if w % sb == 0 and (w // sb) + 2 < nbatch:
                fetch_batch(w // sb + 2)
            if w == 120:
                produce(2)
            if w == 248:
                produce(3)

            if rec_pending is not None and w % RESC == 0:
                rec, = rec_pending
                rec_pending = None
                nc.vector.tensor_scalar_mul(cur[:, 0:129], cur[:, 0:129], rec[:])
                for t in range(w, w + lag):
                    dst = Etiles[(t + 1) % NE]
                    nc.vector.tensor_scalar_mul(
                        dst[0:(ch - 1) * ni, 0:1], dst[0:(ch - 1) * ni, 0:1],
                        rec[0:(ch - 1) * ni])
                lgr = p_stat.tile([npart, 1], F32, tag="lgr")
                nc.scalar.activation(lgr[:], rec[:], AF.Ln)
                nc.vector.tensor_sub(cacc[:], cacc[:], lgr[:])

            s = p_s.tile([npart, 128], F32, tag="s")
            nc.vector.tensor_add(s[:], cur[:, 1:129], cur[:, 0:128])
            ktb = ktbs[w // sb]
            nc.vector.tensor_tensor_scan(
                newt[:, 1:129], s[:], ktb[:, (w % sb) * 128:(w % sb + 1) * 128],
                newt[:, 0:1], OP.add, OP.mult,
            )
            if w == 0:
                nc.vector.memset(Etiles[0][(ch - 1) * ni:ch * ni, 0:1], 0.0)

            if w + lag < nsteps:
                nc.gpsimd.dma_start(
                    Etiles[(w + lag + 1) % NE][0:(ch - 1) * ni, 0:1],
                    newt[ni:npart, 128:129])

            if (w + 8) % RESC == 0 and (w + 8) <= 480:
                pmax = p_stat.tile([npart, 1], F32, tag="pmax")
                nc.vector.tensor_reduce(pmax[:], newt[:, 1:129], AX.X, OP.max)
                pmT = p_bc.tile([1, npart], F32, tag="bc")
                t1 = nc.tensor.transpose(pmT[:], pmax[:],
                                         ident[0:npart, 0:npart])
                mxrow = p_stat.tile([1, ni], F32, tag="mxrow")
                rd2 = nc.vector.tensor_reduce(
                    mxrow[:], pmT[:].rearrange("a (g i) -> a i g", i=ni),
                    AX.X, OP.max)
                add_dep_helper(rd2.ins, t1.ins, reason="reduce after PE T1")
                mxps = p_bc.tile([ni, 1], F32, tag="bc")
                t2 = nc.tensor.transpose(mxps[:], mxrow[:], ident[0:1, 0:1])
                mxcol = p_stat.tile([ni, 1], F32, tag="mxcol")
                cpm = nc.scalar.copy(mxcol[:], mxps[:])
                add_dep_helper(cpm.ins, t2.ins, reason="copy after PE T2")
                bc = p_bc.tile([npart, 1], F32, tag="bc")
                bc_mm = nc.tensor.matmul(bc[:], bcastM[:], mxcol[:],
                                         start=True, stop=True)
                rec = p_stat.tile([npart, 1], F32, tag="rec")
                rcp = nc.vector.reciprocal(rec[:], bc[:])
                add_dep_helper(rcp.ins, bc_mm.ins,
                               reason="recip after PE broadcast")
                rec_pending = (rec,)

        last = Etiles[nsteps % NE]
        nc.sync.dma_start(outs["EOUT"].rearrange("(a b) -> a b", b=1),
                          last[0:ni, 128:129])
        nc.sync.dma_start(outs["COUT"].rearrange("(a b) -> a b", b=1),
                          cacc[0:ni, 0:1])


def _build(t_len=T, bpc=BPC, resc=RESC, num_devices=NCORES, wave=False,
           wave2=False, wave3=False, lag=3, sb=8):
    ni = NTYPE * bpc
    nc = bacc.Bacc(
        "TRN2", target_bir_lowering=False, debug=False, num_devices=num_devices,
    )
    ins = {
        name: nc.dram_tensor(name, [bpc, t_len, D], F32, kind="ExternalInput").ap()
        for name in ("TGT", "OTH", "X")
    }
    outs = {
        "EOUT": nc.dram_tensor("EOUT", [ni], F32, kind="ExternalOutput").ap(),
        "COUT": nc.dram_tensor("COUT", [ni], F32, kind="ExternalOutput").ap(),
    }
    if wave3:
        lag = 5
        pad = (t_len // 128 - 1) * lag
        rt_rows = [128 + pad, 128, 128, 128 + pad + sb]
        kbs = [
            nc.dram_tensor(f"KBUF{i}", [ni, rt_rows[i], t_len], F32).ap()
            for i in range(4)
        ]
        with tile.TileContext(nc) as tc:
            _emit_wave3(tc, ins, outs, kbs, t_len, bpc, lag, sb)
    elif wave2:
        ch = t_len // 128
        pad = (ch - 1) * lag
        krows = t_len + 2 * pad + sb
        kbuf = nc.dram_tensor("KBUF", [ni, krows, t_len], F32).ap()
        with tile.TileContext(nc) as tc:
            _emit_wave2(tc, ins, outs, kbuf, t_len, bpc, lag, sb)
    elif wave:
        kbuf = nc.dram_tensor("KBUF", [ni, t_len + 6, t_len], F32).ap()
        with tile.TileContext(nc) as tc:
            _emit_wave(tc, ins, outs, kbuf, t_len, bpc, resc)
    else:
        kbuf = nc.dram_tensor("KBUF", [ni, t_len, t_len], F32).ap()
        with tile.TileContext(nc) as tc:
            _emit(tc, ins, outs, kbuf, t_len, bpc, resc)
    nc.compile()
    return nc


_NC = None


def _get_nc():
    global _NC
    if _NC is None:
        kv = os.environ.get("KWAVE", "0")
        _NC = _build(wave=kv == "1", wave2=kv == "2", wave3=kv == "3")
    return _NC


def _postprocess(results, labels):
    E = np.stack([r["EOUT"] for r in results])  # [8, 32]
    C = np.stack([r["COUT"] for r in results])  # [8, 32]
    R = -(np.log(E) + C)                        # [core, type*8+b]
    R = R.reshape(NCORES, NTYPE, BPC).transpose(1, 0, 2).reshape(NTYPE, B)
    diff = (R[0] - R[1] - 0.5 * R[2] + 0.5 * R[3]).astype(np.float32)
    lab = np.asarray(labels, dtype=np.float32)
    return np.float32(np.mean((diff - lab) ** 2, dtype=np.float32))


def kernel(TGT, OTH, X, labels):
    nc = _get_nc()
    TGT = np.ascontiguousarray(np.asarray(TGT, dtype=np.float32))
    OTH = np.ascontiguousarray(np.asarray(OTH, dtype=np.float32))
    X = np.ascontiguousarray(np.asarray(X, dtype=np.float32))
    in_maps = [
        {
            "TGT": TGT[c * BPC:(c + 1) * BPC],
            "OTH": OTH[c * BPC:(c + 1) * BPC],
            "X": X[c * BPC:(c + 1) * BPC],
        }
        for c in range(NCORES)
    ]
    res = run_bass_kernel_spmd(nc, in_maps, core_ids=list(range(NCORES)))
    return _postprocess(res.results, labels)



# revision 27
# speedup vs baseline: 2.3853x; 1.0439x over previous
"""Soft-DTW ranking loss kernel for Trainium2 (8 NeuronCores, SPMD data parallel).

Math: loss = mean((diff - labels)^2) where
  diff_b = sdtw(OTH_b,X_b) - sdtw(TGT_b,X_b) - 0.5*sdtw(OTH_b,OTH_b) + 0.5*sdtw(TGT_b,TGT_b)
(the sdtw(X,X) terms of the normalized soft-DTW cancel exactly).

Soft-DTW (gamma=1) is computed in the probability domain:
  E[i,j] = K[i,j] * (E[i-1,j] + E[i-1,j-1] + E[i,j-1]),  K = exp(<xn_i,yn_j> - 1)
which maps one DP row onto a single DVE tensor_tensor_scan:
  state = (s[t] + state) * K[t],   s = E_prev + shift1(E_prev)
with periodic per-instance rescaling (log-scale accumulated in C) to stay in
fp32 range. Each core handles 8 batch items x 4 DTW instances = 32 independent
DPs vectorized across SBUF partitions.
"""

import os
import sys

import numpy as np

for _p in ("/root/.axon_site", "/root/.axon_site/_ro/trn_rl_repo",
           "/root/.axon_site/_ro/pypackages", "/opt/trn_rl_repo", "/opt/pypackages"):
    if os.path.isdir(_p) and _p not in sys.path:
        sys.path.append(_p)

import concourse.bass as bass
import concourse.tile as tile
from concourse.tile import add_dep_helper
from concourse import bacc, mybir
from concourse.bass_utils import run_bass_kernel_spmd
from concourse.masks import make_identity

F32 = mybir.dt.float32
F32R = mybir.dt.float32r
AX = mybir.AxisListType
OP = mybir.AluOpType
AF = mybir.ActivationFunctionType

B, T, D = 64, 512, 64
NCORES = 8
BPC = B // NCORES          # batch items per core
NTYPE = 4                  # (OTH,X), (TGT,X), (OTH,OTH), (TGT,TGT)
RESC = 32                  # rescale cadence (rows)


def _emit(tc: tile.TileContext, ins: dict, outs: dict, kbuf: bass.AP,
          t_len: int, bpc: int, resc: int):
    nc = tc.nc
    ni = NTYPE * bpc
    nrowt = t_len // 128

    with (
        tc.tile_pool(name="const", bufs=1) as p_const,
        tc.tile_pool(name="ain", bufs=2) as p_in,
        tc.tile_pool(name="astat", bufs=2) as p_astat,
        tc.tile_pool(name="asn", bufs=2) as p_asn,
        tc.tile_pool(name="apsT", bufs=2, space="PSUM") as p_psT,
        tc.tile_pool(name="ant", bufs=2) as p_nt,
        tc.tile_pool(name="aG", bufs=2, space="PSUM") as p_G,
        tc.tile_pool(name="aK", bufs=3) as p_K,
        tc.tile_pool(name="bE", bufs=1) as p_E,
        tc.tile_pool(name="bS", bufs=2) as p_s,
        tc.tile_pool(name="bK", bufs=4) as p_k,
        tc.tile_pool(name="bstat", bufs=2) as p_stat,
        tc.tile_pool(name="bacc", bufs=1) as p_acc,
    ):
        ident = p_const.tile([128, 128], F32, tag="ident")
        make_identity(nc, ident[:])
        bias_m1 = p_const.tile([128, 1], F32, tag="biasm1")
        nc.gpsimd.memset(bias_m1[:], -1.0)

        # ---------------- Phase A: K = exp(<xn,yn> - 1) for all pairs -------
        for b in range(bpc):
            nT = {}
            for sname in ("OTH", "TGT", "X"):
                src = ins[sname]
                xin = p_in.tile([128, nrowt * D], F32, tag=f"in_{sname}")
                nc.sync.dma_start(
                    xin[:].rearrange("p (t d) -> p t d", d=D),
                    src[b].rearrange("(t p) d -> p t d", p=128),
                )
                sq = p_astat.tile([128, nrowt * D], F32, tag=f"sq_{sname}")
                ss = p_astat.tile([128, nrowt], F32, tag=f"ss_{sname}")
                for t in range(nrowt):
                    nc.scalar.activation(
                        sq[:, t * D:(t + 1) * D], xin[:, t * D:(t + 1) * D],
                        AF.Square, accum_out=ss[:, t:t + 1],
                    )
                nrm = p_astat.tile([128, nrowt], F32, tag=f"nrm_{sname}")
                nc.scalar.activation(nrm[:], ss[:], AF.Sqrt)
                rnm = p_astat.tile([128, nrowt], F32, tag=f"rnm_{sname}")
                nc.vector.reciprocal(rnm[:], nrm[:])
                sn = p_asn.tile([128, nrowt * D], F32, tag=f"sn_{sname}")
                for t in range(nrowt):
                    nc.vector.tensor_scalar_mul(
                        sn[:, t * D:(t + 1) * D], xin[:, t * D:(t + 1) * D],
                        rnm[:, t:t + 1],
                    )
                snT = p_nt.tile([D, t_len], F32R, tag=f"nt_{sname}")
                for t in range(nrowt):
                    tp = p_psT.tile([D, 128], F32, tag="psT")
                    nc.tensor.transpose(tp[:], sn[:, t * D:(t + 1) * D], ident[:])
                    nc.scalar.copy(snT[:, t * 128:(t + 1) * 128], tp[:])
                nT[sname] = snT

            pairs = [("OTH", "X"), ("TGT", "X"), ("OTH", "OTH"), ("TGT", "TGT")]
            for ptype, (an, cn) in enumerate(pairs):
                inst = ptype * bpc + b
                aT, cT = nT[an], nT[cn]
                for rt in range(nrowt):
                    g = p_G.tile([128, t_len], F32, tag="G")
                    nc.tensor.matmul(
                        g[:],
                        aT[:, rt * 128:(rt + 1) * 128],
                        cT[:],
                        start=True, stop=True,
                    )
                    kt = p_K.tile([128, t_len], F32, tag="K")
                    nc.scalar.activation(kt[:], g[:], AF.Exp, bias=bias_m1[:])
                    nc.sync.dma_start(kbuf[inst, rt * 128:(rt + 1) * 128, :], kt[:])

        # ---------------- Phase B: row-scan DP over all instances -----------
        Ea = p_E.tile([ni, t_len + 1], F32, tag="Ea")
        Eb = p_E.tile([ni, t_len + 1], F32, tag="Eb")
        cacc = p_acc.tile([ni, 1], F32, tag="C")
        nc.gpsimd.memset(Ea[:], 0.0)
        nc.gpsimd.memset(Eb[:], 0.0)
        nc.gpsimd.memset(cacc[:], 0.0)
        nc.gpsimd.memset(Ea[:, 0:1], 1.0)  # E[-1][-1] = exp(-0)

        cur, nxt = Ea, Eb
        for r in range(t_len):
            kt = p_k.tile([ni, t_len], F32, tag="krow")
            nc.sync.dma_start(kt[:], kbuf[:, r, :])
            s = p_s.tile([ni, t_len], F32, tag="s")
            nc.vector.tensor_add(s[:], cur[:, 1:t_len + 1], cur[:, 0:t_len])
            nc.vector.tensor_tensor_scan(
                nxt[:, 1:t_len + 1], s[:], kt[:], 0.0, OP.add, OP.mult,
            )
            if r == 0:
                # E[0][-1] = 0: clear the one-time E[-1][-1] = 1 boundary
                nc.vector.memset(Ea[:, 0:1], 0.0)
            if (r + 1) % resc == 0 and r != t_len - 1:
                mx = p_stat.tile([ni, 1], F32, tag="mx")
                nc.vector.tensor_reduce(mx[:], nxt[:, 1:t_len + 1], AX.X, OP.max)
                rec = p_stat.tile([ni, 1], F32, tag="rec")
                nc.vector.reciprocal(rec[:], mx[:])
                nc.vector.tensor_scalar_mul(nxt[:, 1:t_len + 1],
                                            nxt[:, 1:t_len + 1], rec[:])
                lg = p_stat.tile([ni, 1], F32, tag="lg")
                nc.scalar.activation(lg[:], mx[:], AF.Ln)
                nc.vector.tensor_add(cacc[:], cacc[:], lg[:])
            cur, nxt = nxt, cur

        nc.sync.dma_start(outs["EOUT"].rearrange("(a b) -> a b", b=1),
                          cur[:, t_len:t_len + 1])
        nc.sync.dma_start(outs["COUT"].rearrange("(a b) -> a b", b=1), cacc[:])


def _emit_wave(tc: tile.TileContext, ins: dict, outs: dict, kbuf: bass.AP,
               t_len: int, bpc: int, resc: int):
    """Wavefront DP: CH=t_len/128 column chunks on partition groups.

    Partition p = g*ni + inst handles column chunk g of instance inst.
    Wavefront step w: group g processes row r = w - g (K rows padded with 3
    zero rows on each side so inactive groups compute zeros). Cross-chunk
    carries (scan initial / shifted-row boundary) move between partition
    groups via a constant shift matmul on the (otherwise idle) PE.
    """
    nc = tc.nc
    ni = NTYPE * bpc
    ch = t_len // 128
    npart = ch * ni
    nrowt = ch
    nsteps = t_len + ch - 1

    with (
        tc.tile_pool(name="const", bufs=1) as p_const,
        tc.tile_pool(name="ain", bufs=2) as p_in,
        tc.tile_pool(name="astat", bufs=2) as p_astat,
        tc.tile_pool(name="asn", bufs=2) as p_asn,
        tc.tile_pool(name="apsT", bufs=2, space="PSUM") as p_psT,
        tc.tile_pool(name="ant", bufs=2) as p_nt,
        tc.tile_pool(name="aG", bufs=2, space="PSUM") as p_G,
        tc.tile_pool(name="aK", bufs=3) as p_K,
        tc.tile_pool(name="bE", bufs=1) as p_E,
        tc.tile_pool(name="bS", bufs=2) as p_s,
        tc.tile_pool(name="bK", bufs=8) as p_k,
        tc.tile_pool(name="bC", bufs=3, space="PSUM") as p_carry,
        tc.tile_pool(name="bB", bufs=1, space="PSUM") as p_bc,
        tc.tile_pool(name="bstat", bufs=2) as p_stat,
        tc.tile_pool(name="bacc", bufs=1) as p_acc,
    ):
        ident = p_const.tile([128, 128], F32, tag="ident")
        make_identity(nc, ident[:])
        bias_m1 = p_const.tile([128, 1], F32, tag="biasm1")
        nc.gpsimd.memset(bias_m1[:], -1.0)
        # shiftM[k, p] = 1 iff k == p - ni  (moves group g-1 -> g)
        shiftM = p_const.tile([npart, npart], F32, tag="shiftM")
        nc.gpsimd.memset(shiftM[:], 0.0)
        nc.gpsimd.affine_select(
            out=shiftM[:], in_=shiftM[:], compare_op=OP.not_equal, fill=1.0,
            base=ni, pattern=[[-1, npart]], channel_multiplier=1,
        )
        # bcastM[k, (g, j)] = 1 iff k == j  (broadcast group-0 col to all groups)
        bcastM = p_const.tile([ni, npart], F32, tag="bcastM")
        nc.gpsimd.memset(bcastM[:], 0.0)
        nc.gpsimd.affine_select(
            out=bcastM[:].rearrange("k (g j) -> k g j", j=ni),
            in_=bcastM[:].rearrange("k (g j) -> k g j", j=ni),
            compare_op=OP.not_equal, fill=1.0,
            base=0, pattern=[[0, ch], [-1, ni]], channel_multiplier=1,
        )

        # zero the 3+3 pad rows of kbuf (layout [ni, t_len+6, t_len])
        zpad = p_const.tile([ni, 3 * t_len], F32, tag="zpad")
        nc.gpsimd.memset(zpad[:], 0.0)
        nc.sync.dma_start(
            kbuf[:, 0:3, :].rearrange("i r c -> i (r c)"), zpad[:])
        nc.sync.dma_start(
            kbuf[:, t_len + 3:t_len + 6, :].rearrange("i r c -> i (r c)"), zpad[:])

        # ---------------- Phase A (same as v1, +3 row offset into kbuf) -----
        for b in range(bpc):
            nT = {}
            for sname in ("OTH", "TGT", "X"):
                src = ins[sname]
                xin = p_in.tile([128, nrowt * D], F32, tag=f"in_{sname}")
                nc.sync.dma_start(
                    xin[:].rearrange("p (t d) -> p t d", d=D),
                    src[b].rearrange("(t p) d -> p t d", p=128),
                )
                sq = p_astat.tile([128, nrowt * D], F32, tag=f"sq_{sname}")
                ss = p_astat.tile([128, nrowt], F32, tag=f"ss_{sname}")
                for t in range(nrowt):
                    nc.scalar.activation(
                        sq[:, t * D:(t + 1) * D], xin[:, t * D:(t + 1) * D],
                        AF.Square, accum_out=ss[:, t:t + 1],
                    )
                nrm = p_astat.tile([128, nrowt], F32, tag=f"nrm_{sname}")
                nc.scalar.activation(nrm[:], ss[:], AF.Sqrt)
                rnm = p_astat.tile([128, nrowt], F32, tag=f"rnm_{sname}")
                nc.vector.reciprocal(rnm[:], nrm[:])
                sn = p_asn.tile([128, nrowt * D], F32, tag=f"sn_{sname}")
                for t in range(nrowt):
                    nc.vector.tensor_scalar_mul(
                        sn[:, t * D:(t + 1) * D], xin[:, t * D:(t + 1) * D],
                        rnm[:, t:t + 1],
                    )
                snT = p_nt.tile([D, t_len], F32R, tag=f"nt_{sname}")
                for t in range(nrowt):
                    tp = p_psT.tile([D, 128], F32, tag="psT")
                    nc.tensor.transpose(tp[:], sn[:, t * D:(t + 1) * D], ident[:])
                    nc.scalar.copy(snT[:, t * 128:(t + 1) * 128], tp[:])
                nT[sname] = snT

            pairs = [("OTH", "X"), ("TGT", "X"), ("OTH", "OTH"), ("TGT", "TGT")]
            for ptype, (an, cn) in enumerate(pairs):
                inst = ptype * bpc + b
                aT, cT = nT[an], nT[cn]
                for rt in range(nrowt):
                    g = p_G.tile([128, t_len], F32, tag="G")
                    nc.tensor.matmul(
                        g[:], aT[:, rt * 128:(rt + 1) * 128], cT[:],
                        start=True, stop=True,
                    )
                    kt = p_K.tile([128, t_len], F32, tag="K")
                    nc.scalar.activation(kt[:], g[:], AF.Exp, bias=bias_m1[:])
                    nc.sync.dma_start(
                        kbuf[inst, 3 + rt * 128:3 + (rt + 1) * 128, :], kt[:])

        # ---------------- Phase B: wavefront row-scan -----------------------
        Ea = p_E.tile([npart, 129], F32, tag="Ea")
        Eb = p_E.tile([npart, 129], F32, tag="Eb")
        Etiles = [Ea, Eb]
        cacc = p_acc.tile([npart, 1], F32, tag="C")
        nc.gpsimd.memset(Ea[:], 0.0)
        nc.gpsimd.memset(Eb[:], 0.0)
        nc.gpsimd.memset(cacc[:], 0.0)
        nc.gpsimd.memset(Ea[0:ni, 0:1], 1.0)  # E[-1][-1] = 1 for group 0
        car_prev = p_carry.tile([npart, 1], F32, tag="car")
        car_prev_mm = nc.vector.memset(car_prev[:], 0.0)

        for w in range(nsteps):
            prev = Etiles[w % 2]
            newt = Etiles[(w + 1) % 2]
            kt = p_k.tile([npart, 128], F32, tag="krow")
            for g in range(ch):
                nc.sync.dma_start(
                    kt[g * ni:(g + 1) * ni, :],
                    kbuf[:, w - g + 3, g * 128:(g + 1) * 128],
                )
            s = p_s.tile([npart, 128], F32, tag="s")
            nc.vector.tensor_add(s[:], prev[:, 1:129], prev[:, 0:128])
            scan_i = nc.vector.tensor_tensor_scan(
                newt[:, 1:129], s[:], kt[:], car_prev[:, 0:1],
                OP.add, OP.mult,
            )
            add_dep_helper(scan_i.ins, car_prev_mm.ins,
                           reason="scan initial after PE carry shift")
            if (w + 1) % resc == 0 and w + 1 < t_len:
                # per-partition chunk max -> per-instance max across groups
                pmax = p_stat.tile([npart, 1], F32, tag="pmax")
                nc.vector.tensor_reduce(pmax[:], newt[:, 1:129], AX.X, OP.max)
                pmT = p_bc.tile([1, npart], F32, tag="bc")
                t1 = nc.tensor.transpose(pmT[:], pmax[:],
                                         ident[0:npart, 0:npart])
                mxrow = p_stat.tile([1, ni], F32, tag="mxrow")
                rd2 = nc.vector.tensor_reduce(
                    mxrow[:], pmT[:].rearrange("a (g i) -> a i g", i=ni),
                    AX.X, OP.max)
                add_dep_helper(rd2.ins, t1.ins, reason="reduce after PE T1")
                mxps = p_bc.tile([ni, 1], F32, tag="bc")
                t2 = nc.tensor.transpose(mxps[:], mxrow[:], ident[0:1, 0:1])
                mxcol = p_stat.tile([ni, 1], F32, tag="mxcol")
                cpm = nc.scalar.copy(mxcol[:], mxps[:])
                add_dep_helper(cpm.ins, t2.ins, reason="copy after PE T2")
                bc = p_bc.tile([npart, 1], F32, tag="bc")
                bc_mm = nc.tensor.matmul(bc[:], bcastM[:], mxcol[:],
                                         start=True, stop=True)
                rec = p_stat.tile([npart, 1], F32, tag="rec")
                rcp = nc.vector.reciprocal(rec[:], bc[:])
                add_dep_helper(rcp.ins, bc_mm.ins,
                               reason="recip after PE broadcast")
                nc.vector.tensor_scalar_mul(newt[:, 0:129], newt[:, 0:129], rec[:])
                lgr = p_stat.tile([npart, 1], F32, tag="lgr")
                nc.scalar.activation(lgr[:], rec[:], AF.Ln)
                nc.vector.tensor_sub(cacc[:], cacc[:], lgr[:])
            car = p_carry.tile([npart, 1], F32, tag="car")
            car_mm = nc.tensor.matmul(car[:], shiftM[:], newt[:, 128:129],
                                      start=True, stop=True)
            cp = nc.scalar.copy(prev[:, 0:1], car[:])
            add_dep_helper(cp.ins, car_mm.ins,
                           reason="carry copy after PE shift")
            car_prev = car
            car_prev_mm = car_mm

        last = Etiles[nsteps % 2]
        nc.sync.dma_start(outs["EOUT"].rearrange("(a b) -> a b", b=1),
                          last[(ch - 1) * ni:ch * ni, 128:129])
        nc.sync.dma_start(outs["COUT"].rearrange("(a b) -> a b", b=1),
                          cacc[(ch - 1) * ni:ch * ni, 0:1])


def _emit_wave2(tc: tile.TileContext, ins: dict, outs: dict, kbuf: bass.AP,
                t_len: int, bpc: int, lag: int, sb: int):
    """Wavefront DP v2: lagged chunks + batched kt DMA + pipelined rescale.

    Group g processes row w - g*lag at step w (lag>=2 gives the PE carry
    shift and ACT boundary copy slack off the DVE critical path).  kt rows
    for `sb` consecutive steps are fetched in ONE diagonal-AP DMA.  The
    rescale max is computed 8 steps before it is applied, so its reduce/
    transpose/broadcast chain also runs off the critical path.
    """
    nc = tc.nc
    ni = NTYPE * bpc
    ch = t_len // 128
    npart = ch * ni
    nrowt = ch
    pad = (ch - 1) * lag
    nsteps = t_len + pad
    nbatch = (nsteps + sb - 1) // sb
    krows = t_len + 2 * pad + sb  # top pad + rows + bottom pad (incl DMA overrun)

    with (
        tc.tile_pool(name="const", bufs=1) as p_const,
    ):
        ident = p_const.tile([128, 128], F32, tag="ident")
        make_identity(nc, ident[:])
        bias_m1 = p_const.tile([128, 1], F32, tag="biasm1")
        nc.gpsimd.memset(bias_m1[:], -1.0)
        # Group g lives on partition block (ch-1-g); carries move to the next
        # block via a partition-shift DMA (no PE involvement).
        # bcastM[k, (g, j)] = 1 iff k == j  (broadcast per-inst col to all groups)
        bcastM = p_const.tile([ni, npart], F32, tag="bcastM")
        nc.gpsimd.memset(bcastM[:], 0.0)
        nc.gpsimd.affine_select(
            out=bcastM[:].rearrange("k (g j) -> k g j", j=ni),
            in_=bcastM[:].rearrange("k (g j) -> k g j", j=ni),
            compare_op=OP.not_equal, fill=1.0,
            base=0, pattern=[[0, ch], [-1, ni]], channel_multiplier=1,
        )

        # zero the pad rows of kbuf (layout [ni, krows, t_len])
        nbot = krows - t_len - pad
        zpad = p_const.tile([ni, nbot * 512], F32, tag="zpad")
        nc.gpsimd.memset(zpad[:], 0.0)
        nc.sync.dma_start(
            kbuf[:, 0:pad, :].rearrange("i r c -> i (r c)"),
            zpad[:, 0:pad * 512])
        nc.sync.dma_start(
            kbuf[:, t_len + pad:krows, :].rearrange("i r c -> i (r c)"),
            zpad[:])

        # ---------------- Phase A (as v1, +pad row offset into kbuf) --------
        with (
            tc.tile_pool(name="ain", bufs=2) as p_in,
            tc.tile_pool(name="astat", bufs=2) as p_astat,
            tc.tile_pool(name="asn", bufs=2) as p_asn,
            tc.tile_pool(name="apsT", bufs=2, space="PSUM") as p_psT,
            tc.tile_pool(name="ant", bufs=2) as p_nt,
            tc.tile_pool(name="aG", bufs=2, space="PSUM") as p_G,
            tc.tile_pool(name="aK", bufs=3) as p_K,
        ):
            _emit_phaseA(tc, ins, kbuf, t_len, bpc, pad,
                         p_in, p_astat, p_asn, p_psT, p_nt, p_G, p_K,
                         ident, bias_m1)

        # ---------------- Phase B: lagged wavefront row-scan ----------------
        with (
            tc.tile_pool(name="bE", bufs=1) as p_E,
            tc.tile_pool(name="bS", bufs=2) as p_s,
            tc.tile_pool(name="bK", bufs=4) as p_k,
            tc.tile_pool(name="bB", bufs=1, space="PSUM") as p_bc,
            tc.tile_pool(name="bstat", bufs=4) as p_stat,
            tc.tile_pool(name="bacc", bufs=1) as p_acc,
        ):
            _emit_phaseB(tc, outs, kbuf, t_len, bpc, lag, sb,
                         p_E, p_s, p_k, p_bc, p_stat, p_acc,
                         ident, bcastM)


def _emit_phaseA(tc, ins, kbuf, t_len, bpc, pad,
                 p_in, p_astat, p_asn, p_psT, p_nt, p_G, p_K,
                 ident, bias_m1):
    nc = tc.nc
    nrowt = t_len // 128
    if True:
        for b in range(bpc):
            nT = {}
            for sname in ("OTH", "TGT", "X"):
                src = ins[sname]
                xin = p_in.tile([128, nrowt * D], F32, tag=f"in_{sname}")
                nc.sync.dma_start(
                    xin[:].rearrange("p (t d) -> p t d", d=D),
                    src[b].rearrange("(t p) d -> p t d", p=128),
                )
                sq = p_astat.tile([128, nrowt * D], F32, tag=f"sq_{sname}")
                ss = p_astat.tile([128, nrowt], F32, tag=f"ss_{sname}")
                for t in range(nrowt):
                    nc.scalar.activation(
                        sq[:, t * D:(t + 1) * D], xin[:, t * D:(t + 1) * D],
                        AF.Square, accum_out=ss[:, t:t + 1],
                    )
                nrm = p_astat.tile([128, nrowt], F32, tag=f"nrm_{sname}")
                nc.scalar.activation(nrm[:], ss[:], AF.Sqrt)
                rnm = p_astat.tile([128, nrowt], F32, tag=f"rnm_{sname}")
                nc.vector.reciprocal(rnm[:], nrm[:])
                sn = p_asn.tile([128, nrowt * D], F32, tag=f"sn_{sname}")
                for t in range(nrowt):
                    nc.vector.tensor_scalar_mul(
                        sn[:, t * D:(t + 1) * D], xin[:, t * D:(t + 1) * D],
                        rnm[:, t:t + 1],
                    )
                snT = p_nt.tile([D, t_len], F32R, tag=f"nt_{sname}")
                for t in range(nrowt):
                    tp = p_psT.tile([D, 128], F32, tag="psT")
                    nc.tensor.transpose(tp[:], sn[:, t * D:(t + 1) * D], ident[:])
                    nc.scalar.copy(snT[:, t * 128:(t + 1) * 128], tp[:])
                nT[sname] = snT

            pairs = [("OTH", "X"), ("TGT", "X"), ("OTH", "OTH"), ("TGT", "TGT")]
            for ptype, (an, cn) in enumerate(pairs):
                inst = ptype * bpc + b
                aT, cT = nT[an], nT[cn]
                for rt in range(nrowt):
                    g = p_G.tile([128, t_len], F32, tag="G")
                    nc.tensor.matmul(
                        g[:], aT[:, rt * 128:(rt + 1) * 128], cT[:],
                        start=True, stop=True,
                    )
                    kt = p_K.tile([128, t_len], F32, tag="K")
                    nc.scalar.activation(kt[:], g[:], AF.Exp, bias=bias_m1[:])
                    nc.sync.dma_start(
                        kbuf[inst, pad + rt * 128:pad + (rt + 1) * 128, :], kt[:])


def _emit_phaseB(tc, outs, kbuf, t_len, bpc, lag, sb,
                 p_E, p_s, p_k, p_bc, p_stat, p_acc,
                 ident, bcastM):
    nc = tc.nc
    ni = NTYPE * bpc
    ch = t_len // 128
    npart = ch * ni
    pad = (ch - 1) * lag
    nsteps = t_len + pad
    nbatch = (nsteps + sb - 1) // sb
    krows = t_len + 2 * pad + sb
    if True:
        NE = 4
        Etiles = []
        for j in range(NE):
            Ej = p_E.tile([npart, 129], F32, tag=f"E{j}")
            Etiles.append(Ej)
        cacc = p_acc.tile([npart, 1], F32, tag="C")
        for E in Etiles:
            nc.gpsimd.memset(E[:], 0.0)
        nc.gpsimd.memset(cacc[:], 0.0)
        # E[-1][-1] = 1 for group 0 (= partition block ch-1)
        nc.gpsimd.memset(Etiles[0][(ch - 1) * ni:ch * ni, 0:1], 1.0)

        ktbs = {}
        rec_pending = None  # (rec_tile,) scheduled for the next apply step

        def fetch_batch(bi):
            w0 = bi * sb
            ktb = p_k.tile([npart, sb * 128], F32, tag="ktb")
            # ktb[(ch-1-g)*ni + i, s*128 + c] = kbuf[i, w0+s-g*lag+pad, g*128+c]
            for g in range(ch):
                blk = ch - 1 - g
                src = kbuf.copy()
                src.ap = type(src.ap)([
                    [krows * t_len, ni],        # i
                    [t_len, sb],                # s (step within batch)
                    [1, 128],                   # c
                ])
                src.offset = (w0 - g * lag + pad) * t_len + g * 128
                nc.sync.dma_start(
                    ktb[blk * ni:(blk + 1) * ni, :]
                    .rearrange("i (s c) -> i s c", c=128), src)
            ktbs[bi] = ktb

        fetch_batch(0)
        fetch_batch(1)

        for w in range(nsteps):
            cur = Etiles[w % NE]      # rows w-1-g*lag (prev), written by scan w-1
            newt = Etiles[(w + 1) % NE]
            if w % sb == 0 and (w // sb) + 2 < nbatch:
                fetch_batch(w // sb + 2)

            # pipelined rescale: apply scale computed 8 steps ago
            if rec_pending is not None and w % RESC == 0:
                rec, = rec_pending
                rec_pending = None
                nc.vector.tensor_scalar_mul(cur[:, 0:129], cur[:, 0:129], rec[:])
                # boundary DMAs for steps w..w+lag-1 were issued pre-scale:
                # rescale their landing zones (col 0 of the dst tiles)
                for t in range(w, w + lag):
                    dst = Etiles[(t + 1) % NE]
                    nc.vector.tensor_scalar_mul(
                        dst[0:(ch - 1) * ni, 0:1], dst[0:(ch - 1) * ni, 0:1],
                        rec[0:(ch - 1) * ni])
                lgr = p_stat.tile([npart, 1], F32, tag="lgr")
                nc.scalar.activation(lgr[:], rec[:], AF.Ln)
                nc.vector.tensor_sub(cacc[:], cacc[:], lgr[:])

            s = p_s.tile([npart, 128], F32, tag="s")
            nc.vector.tensor_add(s[:], cur[:, 1:129], cur[:, 0:128])
            ktb = ktbs[w // sb]
            nc.vector.tensor_tensor_scan(
                newt[:, 1:129], s[:], ktb[:, (w % sb) * 128:(w % sb + 1) * 128],
                newt[:, 0:1], OP.add, OP.mult,
            )
            if w == 0:
                # clear the one-time E[-1][-1] = 1 seed (group 0 boundary is 0)
                nc.vector.memset(Etiles[0][(ch - 1) * ni:ch * ni, 0:1], 0.0)

            # boundary for step w+lag: E tile col 0 gets group g-1's scan
            # output boundary (partition shift by +ni) via SWDGE DMA
            if w + lag < nsteps:
                nc.gpsimd.dma_start(
                    Etiles[(w + lag + 1) % NE][0:(ch - 1) * ni, 0:1],
                    newt[ni:npart, 128:129])

            # pipelined rescale: compute scale from this step's rows
            if (w + 8) % RESC == 0 and (w + 8) <= 480:
                pmax = p_stat.tile([npart, 1], F32, tag="pmax")
                nc.vector.tensor_reduce(pmax[:], newt[:, 1:129], AX.X, OP.max)
                pmT = p_bc.tile([1, npart], F32, tag="bc")
                t1 = nc.tensor.transpose(pmT[:], pmax[:],
                                         ident[0:npart, 0:npart])
                mxrow = p_stat.tile([1, ni], F32, tag="mxrow")
                rd2 = nc.vector.tensor_reduce(
                    mxrow[:], pmT[:].rearrange("a (g i) -> a i g", i=ni),
                    AX.X, OP.max)
                add_dep_helper(rd2.ins, t1.ins, reason="reduce after PE T1")
                mxps = p_bc.tile([ni, 1], F32, tag="bc")
                t2 = nc.tensor.transpose(mxps[:], mxrow[:], ident[0:1, 0:1])
                mxcol = p_stat.tile([ni, 1], F32, tag="mxcol")
                cpm = nc.scalar.copy(mxcol[:], mxps[:])
                add_dep_helper(cpm.ins, t2.ins, reason="copy after PE T2")
                bc = p_bc.tile([npart, 1], F32, tag="bc")
                bc_mm = nc.tensor.matmul(bc[:], bcastM[:], mxcol[:],
                                         start=True, stop=True)
                rec = p_stat.tile([npart, 1], F32, tag="rec")
                rcp = nc.vector.reciprocal(rec[:], bc[:])
                add_dep_helper(rcp.ins, bc_mm.ins,
                               reason="recip after PE broadcast")
                rec_pending = (rec,)

        # group ch-1 (final column chunk) lives on partition block 0
        last = Etiles[nsteps % 2]
        nc.sync.dma_start(outs["EOUT"].rearrange("(a b) -> a b", b=1),
                          last[0:ni, 128:129])
        nc.sync.dma_start(outs["COUT"].rearrange("(a b) -> a b", b=1),
                          cacc[0:ni, 0:1])


def _emit_wave3(tc: tile.TileContext, ins: dict, outs: dict, kbs: list,
                t_len: int, bpc: int, lag: int, sb: int):
    """Wavefront DP v3: phase A (K production) overlapped under phase B.

    kbs: 4 per-row-tile DRAM tensors (row-tile granular dep tracking).
    kbs[0] holds global rows [-pad, 128) at local r+pad; kbs[1]/[2] rows
    [128,256)/[256,384); kbs[3] rows [384, 512+pad+sb) incl bottom pad.
    Carries move between column-chunk groups via partition-shift DMAs on
    the gpsimd (SWDGE) queue; NE=8 E-tile rotation gives them slack.
    """
    nc = tc.nc
    ni = NTYPE * bpc
    ch = t_len // 128
    npart = ch * ni
    nrowt = ch
    pad = (ch - 1) * lag
    nsteps = t_len + pad
    nbatch = (nsteps + sb - 1) // sb
    NE = 8
    rt_base = [-pad, 128, 256, 384]
    rt_rows = [128 + pad, 128, 128, 128 + pad + sb]

    def rt_of(r):
        return max(0, min(3, r // 128))

    with (
        tc.tile_pool(name="const", bufs=1) as p_const,
        tc.tile_pool(name="ant", bufs=1) as p_nt,
        tc.tile_pool(name="aG", bufs=2, space="PSUM") as p_G,
        tc.tile_pool(name="aK", bufs=3) as p_K,
        tc.tile_pool(name="bE", bufs=1) as p_E,
        tc.tile_pool(name="bS", bufs=2) as p_s,
        tc.tile_pool(name="bK", bufs=4) as p_k,
        tc.tile_pool(name="bB", bufs=1, space="PSUM") as p_bc,
        tc.tile_pool(name="bstat", bufs=4) as p_stat,
        tc.tile_pool(name="bacc", bufs=1) as p_acc,
    ):
        ident = p_const.tile([128, 128], F32, tag="ident")
        make_identity(nc, ident[:])
        bias_m1 = p_const.tile([128, 1], F32, tag="biasm1")
        nc.gpsimd.memset(bias_m1[:], -1.0)
        bcastM = p_const.tile([ni, npart], F32, tag="bcastM")
        nc.gpsimd.memset(bcastM[:], 0.0)
        nc.gpsimd.affine_select(
            out=bcastM[:].rearrange("k (g j) -> k g j", j=ni),
            in_=bcastM[:].rearrange("k (g j) -> k g j", j=ni),
            compare_op=OP.not_equal, fill=1.0,
            base=0, pattern=[[0, ch], [-1, ni]], channel_multiplier=1,
        )

        # zero pads: top of kbs[0] (pad rows), bottom of kbs[3] (pad+sb rows)
        zp = p_const.tile([ni, 4 * t_len], F32, tag="zp")
        nc.gpsimd.memset(zp[:], 0.0)
        for lo in range(0, pad, 4):
            n = min(4, pad - lo)
            nc.sync.dma_start(
                kbs[0][:, lo:lo + n, :].rearrange("i r c -> i (r c)"),
                zp[:, 0:n * t_len])
        for lo in range(128, 128 + pad + sb, 4):
            n = min(4, 128 + pad + sb - lo)
            nc.sync.dma_start(
                kbs[3][:, lo:lo + n, :].rearrange("i r c -> i (r c)"),
                zp[:, 0:n * t_len])

        pairs = [("OTH", "X"), ("TGT", "X"), ("OTH", "OTH"), ("TGT", "TGT")]
        nts = {}

        def produce_item(rt, b):
            loc = rt * 128 - rt_base[rt]
            for ptype, (an, cn) in enumerate(pairs):
                inst = ptype * bpc + b
                aT, cT = nts[(b, an)], nts[(b, cn)]
                g = p_G.tile([128, t_len], F32, tag="G")
                nc.tensor.matmul(
                    g[:], aT[:, rt * 128:(rt + 1) * 128], cT[:],
                    start=True, stop=True,
                )
                kt = p_K.tile([128, t_len], F32, tag="K")
                nc.scalar.activation(kt[:], g[:], AF.Exp, bias=bias_m1[:])
                nc.scalar.dma_start(kbs[rt][inst, loc:loc + 128, :], kt[:])

        def produce(rt):
            for b in range(bpc):
                produce_item(rt, b)

        # ---- preamble: normalized+transposed sequences for all items ------
        with (
            tc.tile_pool(name="ain", bufs=2) as p_in,
            tc.tile_pool(name="astat", bufs=2) as p_astat,
            tc.tile_pool(name="asn", bufs=2) as p_asn,
            tc.tile_pool(name="apsT", bufs=2, space="PSUM") as p_psT,
        ):
            for b in range(bpc):
                for sname in ("OTH", "TGT", "X"):
                    src = ins[sname]
                    xin = p_in.tile([128, nrowt * D], F32, tag=f"in_{sname}")
                    nc.sync.dma_start(
                        xin[:].rearrange("p (t d) -> p t d", d=D),
                        src[b].rearrange("(t p) d -> p t d", p=128),
                    )
                    sq = p_astat.tile([128, nrowt * D], F32, tag=f"sq_{sname}")
                    ss = p_astat.tile([128, nrowt], F32, tag=f"ss_{sname}")
                    for t in range(nrowt):
                        nc.scalar.activation(
                            sq[:, t * D:(t + 1) * D], xin[:, t * D:(t + 1) * D],
                            AF.Square, accum_out=ss[:, t:t + 1],
                        )
                    nrm = p_astat.tile([128, nrowt], F32, tag=f"nrm_{sname}")
                    nc.scalar.activation(nrm[:], ss[:], AF.Sqrt)
                    rnm = p_astat.tile([128, nrowt], F32, tag=f"rnm_{sname}")
                    nc.vector.reciprocal(rnm[:], nrm[:])
                    sn = p_asn.tile([128, nrowt * D], F32, tag=f"sn_{sname}")
                    for t in range(nrowt):
                        nc.vector.tensor_scalar_mul(
                            sn[:, t * D:(t + 1) * D], xin[:, t * D:(t + 1) * D],
                            rnm[:, t:t + 1],
                        )
                    snT = p_nt.tile([D, t_len], F32R, tag=f"nt_{b}_{sname}")
                    for t in range(nrowt):
                        tp = p_psT.tile([D, 128], F32, tag="psT")
                        nc.tensor.transpose(tp[:], sn[:, t * D:(t + 1) * D],
                                            ident[:])
                        nc.vector.tensor_copy(snT[:, t * 128:(t + 1) * 128],
                                              tp[:])
                    nts[(b, sname)] = snT
                produce_item(0, b)

        for b in range(bpc):
            produce_item(1, b)

        # ---------------- Phase B ------------------------------------------
        Etiles = []
        for j in range(NE):
            Ej = p_E.tile([npart, 129], F32, tag=f"E{j}")
            Etiles.append(Ej)
        cacc = p_acc.tile([npart, 1], F32, tag="C")
        for E in Etiles:
            nc.gpsimd.memset(E[:], 0.0)
        nc.gpsimd.memset(cacc[:], 0.0)
        nc.gpsimd.memset(Etiles[0][(ch - 1) * ni:ch * ni, 0:1], 1.0)

        ktbs = {}
        rec_pending = None

        def fetch_batch(bi):
            w0 = bi * sb
            ktb = p_k.tile([npart, sb * 128], F32, tag="ktb")
            # ktb[(ch-1-g)*ni+i, s*128+c] = K[i][row w0+s-g*lag][g*128+c]
            for g in range(ch):
                blk = ch - 1 - g
                r_lo = w0 - g * lag
                s = 0
                while s < sb:
                    r = r_lo + s
                    rt = rt_of(r)
                    krt = kbs[rt]
                    n = min(sb - s, rt_base[rt] + rt_rows[rt] - r)
                    src = krt.copy()
                    src.ap = type(src.ap)([
                        [rt_rows[rt] * t_len, ni],
                        [t_len, n],
                        [1, 128],
                    ])
                    src.offset = (r - rt_base[rt]) * t_len + g * 128
                    nc.sync.dma_start(
                        ktb[blk * ni:(blk + 1) * ni, s * 128:(s + n) * 128]
                        .rearrange("i (s c) -> i s c", c=128), src)
                    s += n
            ktbs[bi] = ktb

        fetch_batch(0)
        fetch_batch(1)

        fixups = {}
        for w in range(nsteps):
            cur = Etiles[w % NE]
            newt = Etiles[(w + 1) % NE]
            if w % sb == 0 and (w // sb) + 2 < nbatch:
                fetch_batch(w // sb + 2)
            if w == 100:
                produce(2)
            if w == 230:
                produce(3)

            apply_rec = None
            if rec_pending is not None and w % RESC == 0:
                apply_rec, = rec_pending
                rec_pending = None
                # rescale the state via the s tile (avoids a WAR stall with
                # the in-flight boundary DMA that reads cur's col 128); the
                # boundary values DMA'd from pre-scale rows get fixed up at
                # the step that consumes them (fixups dict).
                for t in range(w, w + lag):
                    fixups[t] = apply_rec
                lgr = p_stat.tile([npart, 1], F32, tag="lgr")
                nc.scalar.activation(lgr[:], apply_rec[:], AF.Ln)
                nc.vector.tensor_sub(cacc[:], cacc[:], lgr[:])

            s = p_s.tile([npart, 128], F32, tag="s")
            nc.vector.tensor_add(s[:], cur[:, 1:129], cur[:, 0:128])
            if apply_rec is not None:
                nc.vector.tensor_scalar_mul(s[:], s[:], apply_rec[:])
            fx = fixups.pop(w, None)
            if fx is not None:
                nc.vector.tensor_scalar_mul(
                    newt[0:(ch - 1) * ni, 0:1], newt[0:(ch - 1) * ni, 0:1],
                    fx[0:(ch - 1) * ni])
            ktb = ktbs[w // sb]
            nc.vector.tensor_tensor_scan(
                newt[:, 1:129], s[:], ktb[:, (w % sb) * 128:(w % sb + 1) * 128],
                newt[:, 0:1], OP.add, OP.mult,
            )
            if w == 0:
                nc.vector.memset(Etiles[0][(ch - 1) * ni:ch * ni, 0:1], 0.0)

            if w + lag < nsteps:
                nc.gpsimd.dma_start(
                    Etiles[(w + lag + 1) % NE][0:(ch - 1) * ni, 0:1],
                    newt[ni:npart, 128:129])

            if (w + 8) % RESC == 0 and (w + 8) <= 480:
                pmax = p_stat.tile([npart, 1], F32, tag="pmax")
                nc.vector.tensor_reduce(pmax[:], newt[:, 1:129], AX.X, OP.max)
                pmT = p_bc.tile([1, npart], F32, tag="bc")
                t1 = nc.tensor.transpose(pmT[:], pmax[:],
                                         ident[0:npart, 0:npart])
                mxrow = p_stat.tile([1, ni], F32, tag="mxrow")
                rd2 = nc.vector.tensor_reduce(
                    mxrow[:], pmT[:].rearrange("a (g i) -> a i g", i=ni),
                    AX.X, OP.max)
                add_dep_helper(rd2.ins, t1.ins, reason="reduce after PE T1")
                mxps = p_bc.tile([ni, 1], F32, tag="bc")
                t2 = nc.tensor.transpose(mxps[:], mxrow[:], ident[0:1, 0:1])
                mxcol = p_stat.tile([ni, 1], F32, tag="mxcol")
                cpm = nc.scalar.copy(mxcol[:], mxps[:])
                add_dep_helper(cpm.ins, t2.ins, reason="copy after PE T2")
                bc = p_bc.tile([npart, 1], F32, tag="bc")
                bc_mm = nc.tensor.matmul(bc[:], bcastM[:], mxcol[:],
                                         start=True, stop=True)
                rec = p_stat.tile([npart, 1], F32, tag="rec")
                rcp = nc.vector.reciprocal(rec[:], bc[:])
                add_dep_helper(rcp.ins, bc_mm.ins,
                               reason="recip after PE broadcast")
                rec_pending = (rec,)

        last = Etiles[nsteps % NE]
        nc.sync.dma_start(outs["EOUT"].rearrange("(a b) -> a b", b=1),
                          last[0:ni, 128:129])
        nc.sync.dma_start(outs["COUT"].rearrange("(a b) -> a b", b=1),
                          cacc[0:ni, 0:1])


def _build(t_len=T, bpc=BPC, resc=RESC, num_devices=NCORES, wave=False,
           wave2=False, wave3=False, lag=3, sb=8):
    ni = NTYPE * bpc
    nc = bacc.Bacc(
        "TRN2", target_bir_lowering=False, debug=False, num_devices=num_devices,
    )
    ins = {
        name: nc.dram_tensor(name, [bpc, t_len, D], F32, kind="ExternalInput").ap()
        for name in ("TGT", "OTH", "X")
    }
    outs = {
        "EOUT": nc.dram_tensor("EOUT", [ni], F32, kind="ExternalOutput").ap(),
        "COUT": nc.dram_tensor("COUT", [ni], F32, kind="ExternalOutput").ap(),
    }
    if wave3:
        lag = 5
        pad = (t_len // 128 - 1) * lag
        rt_rows = [128 + pad, 128, 128, 128 + pad + sb]
        kbs = [
            nc.dram_tensor(f"KBUF{i}", [ni, rt_rows[i], t_len], F32).ap()
            for i in range(4)
        ]
        with tile.TileContext(nc) as tc:
            _emit_wave3(tc, ins, outs, kbs, t_len, bpc, lag, sb)
    elif wave2:
        ch = t_len // 128
        pad = (ch - 1) * lag
        krows = t_len + 2 * pad + sb
        kbuf = nc.dram_tensor("KBUF", [ni, krows, t_len], F32).ap()
        with tile.TileContext(nc) as tc:
            _emit_wave2(tc, ins, outs, kbuf, t_len, bpc, lag, sb)
    elif wave:
        kbuf = nc.dram_tensor("KBUF", [ni, t_len + 6, t_len], F32).ap()
        with tile.TileContext(nc) as tc:
            _emit_wave(tc, ins, outs, kbuf, t_len, bpc, resc)
    else:
        kbuf = nc.dram_tensor("KBUF", [ni, t_len, t_len], F32).ap()
        with tile.TileContext(nc) as tc:
            _emit(tc, ins, outs, kbuf, t_len, bpc, resc)
    nc.compile()
    return nc


_NC = None


def _get_nc():
    global _NC
    if _NC is None:
        kv = os.environ.get("KWAVE", "0")
        _NC = _build(wave=kv == "1", wave2=kv == "2", wave3=kv == "3")
    return _NC


def _postprocess(results, labels):
    E = np.stack([r["EOUT"] for r in results])  # [8, 32]
    C = np.stack([r["COUT"] for r in results])  # [8, 32]
    R = -(np.log(E) + C)                        # [core, type*8+b]
    R = R.reshape(NCORES, NTYPE, BPC).transpose(1, 0, 2).reshape(NTYPE, B)
    diff = (R[0] - R[1] - 0.5 * R[2] + 0.5 * R[3]).astype(np.float32)
    lab = np.asarray(labels, dtype=np.float32)
    return np.float32(np.mean((diff - lab) ** 2, dtype=np.float32))


def kernel(TGT, OTH, X, labels):
    nc = _get_nc()
    TGT = np.ascontiguousarray(np.asarray(TGT, dtype=np.float32))
    OTH = np.ascontiguousarray(np.asarray(OTH, dtype=np.float32))
    X = np.ascontiguousarray(np.asarray(X, dtype=np.float32))
    in_maps = [
        {
            "TGT": TGT[c * BPC:(c + 1) * BPC],
            "OTH": OTH[c * BPC:(c + 1) * BPC],
            "X": X[c * BPC:(c + 1) * BPC],
        }
        for c in range(NCORES)
    ]
    res = run_bass_kernel_spmd(nc, in_maps, core_ids=list(range(NCORES)))
    return _postprocess(res.results, labels)



# revision 35
# speedup vs baseline: 2.5034x; 1.0495x over previous
"""Soft-DTW ranking loss kernel for Trainium2 (8 NeuronCores, SPMD data parallel).

Math: loss = mean((diff - labels)^2) where
  diff_b = sdtw(OTH_b,X_b) - sdtw(TGT_b,X_b) - 0.5*sdtw(OTH_b,OTH_b) + 0.5*sdtw(TGT_b,TGT_b)
(the sdtw(X,X) terms of the normalized soft-DTW cancel exactly).

Soft-DTW (gamma=1) is computed in the probability domain:
  E[i,j] = K[i,j] * (E[i-1,j] + E[i-1,j-1] + E[i,j-1]),  K = exp(<xn_i,yn_j> - 1)
which maps one DP row onto a single DVE tensor_tensor_scan:
  state = (s[t] + state) * K[t],   s = E_prev + shift1(E_prev)
with periodic per-instance rescaling (log-scale accumulated in C) to stay in
fp32 range. Each core handles 8 batch items x 4 DTW instances = 32 independent
DPs vectorized across SBUF partitions.
"""

import os
import sys

import numpy as np

for _p in ("/root/.axon_site", "/root/.axon_site/_ro/trn_rl_repo",
           "/root/.axon_site/_ro/pypackages", "/opt/trn_rl_repo", "/opt/pypackages"):
    if os.path.isdir(_p) and _p not in sys.path:
        sys.path.append(_p)

import concourse.bass as bass
import concourse.tile as tile
from concourse.tile import add_dep_helper
from concourse import bacc, mybir
from concourse.bass_utils import run_bass_kernel_spmd
from concourse.masks import make_identity

F32 = mybir.dt.float32
F32R = mybir.dt.float32r
AX = mybir.AxisListType
OP = mybir.AluOpType
AF = mybir.ActivationFunctionType

B, T, D = 64, 512, 64
NCORES = 8
BPC = B // NCORES          # batch items per core
NTYPE = 4                  # (OTH,X), (TGT,X), (OTH,OTH), (TGT,TGT)
RESC = 32                  # rescale cadence (rows)


def _emit(tc: tile.TileContext, ins: dict, outs: dict, kbuf: bass.AP,
          t_len: int, bpc: int, resc: int):
    nc = tc.nc
    ni = NTYPE * bpc
    nrowt = t_len // 128

    with (
        tc.tile_pool(name="const", bufs=1) as p_const,
        tc.tile_pool(name="ain", bufs=2) as p_in,
        tc.tile_pool(name="astat", bufs=2) as p_astat,
        tc.tile_pool(name="asn", bufs=2) as p_asn,
        tc.tile_pool(name="apsT", bufs=2, space="PSUM") as p_psT,
        tc.tile_pool(name="ant", bufs=2) as p_nt,
        tc.tile_pool(name="aG", bufs=2, space="PSUM") as p_G,
        tc.tile_pool(name="aK", bufs=3) as p_K,
        tc.tile_pool(name="bE", bufs=1) as p_E,
        tc.tile_pool(name="bS", bufs=2) as p_s,
        tc.tile_pool(name="bK", bufs=4) as p_k,
        tc.tile_pool(name="bstat", bufs=2) as p_stat,
        tc.tile_pool(name="bacc", bufs=1) as p_acc,
    ):
        ident = p_const.tile([128, 128], F32, tag="ident")
        make_identity(nc, ident[:])
        bias_m1 = p_const.tile([128, 1], F32, tag="biasm1")
        nc.gpsimd.memset(bias_m1[:], -1.0)

        # ---------------- Phase A: K = exp(<xn,yn> - 1) for all pairs -------
        for b in range(bpc):
            nT = {}
            for sname in ("OTH", "TGT", "X"):
                src = ins[sname]
                xin = p_in.tile([128, nrowt * D], F32, tag=f"in_{sname}")
                nc.sync.dma_start(
                    xin[:].rearrange("p (t d) -> p t d", d=D),
                    src[b].rearrange("(t p) d -> p t d", p=128),
                )
                sq = p_astat.tile([128, nrowt * D], F32, tag=f"sq_{sname}")
                ss = p_astat.tile([128, nrowt], F32, tag=f"ss_{sname}")
                for t in range(nrowt):
                    nc.scalar.activation(
                        sq[:, t * D:(t + 1) * D], xin[:, t * D:(t + 1) * D],
                        AF.Square, accum_out=ss[:, t:t + 1],
                    )
                nrm = p_astat.tile([128, nrowt], F32, tag=f"nrm_{sname}")
                nc.scalar.activation(nrm[:], ss[:], AF.Sqrt)
                rnm = p_astat.tile([128, nrowt], F32, tag=f"rnm_{sname}")
                nc.vector.reciprocal(rnm[:], nrm[:])
                sn = p_asn.tile([128, nrowt * D], F32, tag=f"sn_{sname}")
                for t in range(nrowt):
                    nc.vector.tensor_scalar_mul(
                        sn[:, t * D:(t + 1) * D], xin[:, t * D:(t + 1) * D],
                        rnm[:, t:t + 1],
                    )
                snT = p_nt.tile([D, t_len], F32R, tag=f"nt_{sname}")
                for t in range(nrowt):
                    tp = p_psT.tile([D, 128], F32, tag="psT")
                    nc.tensor.transpose(tp[:], sn[:, t * D:(t + 1) * D], ident[:])
                    nc.scalar.copy(snT[:, t * 128:(t + 1) * 128], tp[:])
                nT[sname] = snT

            pairs = [("OTH", "X"), ("TGT", "X"), ("OTH", "OTH"), ("TGT", "TGT")]
            for ptype, (an, cn) in enumerate(pairs):
                inst = ptype * bpc + b
                aT, cT = nT[an], nT[cn]
                for rt in range(nrowt):
                    g = p_G.tile([128, t_len], F32, tag="G")
                    nc.tensor.matmul(
                        g[:],
                        aT[:, rt * 128:(rt + 1) * 128],
                        cT[:],
                        start=True, stop=True,
                    )
                    kt = p_K.tile([128, t_len], F32, tag="K")
                    nc.scalar.activation(kt[:], g[:], AF.Exp, bias=bias_m1[:])
                    nc.sync.dma_start(kbuf[inst, rt * 128:(rt + 1) * 128, :], kt[:])

        # ---------------- Phase B: row-scan DP over all instances -----------
        Ea = p_E.tile([ni, t_len + 1], F32, tag="Ea")
        Eb = p_E.tile([ni, t_len + 1], F32, tag="Eb")
        cacc = p_acc.tile([ni, 1], F32, tag="C")
        nc.gpsimd.memset(Ea[:], 0.0)
        nc.gpsimd.memset(Eb[:], 0.0)
        nc.gpsimd.memset(cacc[:], 0.0)
        nc.gpsimd.memset(Ea[:, 0:1], 1.0)  # E[-1][-1] = exp(-0)

        cur, nxt = Ea, Eb
        for r in range(t_len):
            kt = p_k.tile([ni, t_len], F32, tag="krow")
            nc.sync.dma_start(kt[:], kbuf[:, r, :])
            s = p_s.tile([ni, t_len], F32, tag="s")
            nc.vector.tensor_add(s[:], cur[:, 1:t_len + 1], cur[:, 0:t_len])
            nc.vector.tensor_tensor_scan(
                nxt[:, 1:t_len + 1], s[:], kt[:], 0.0, OP.add, OP.mult,
            )
            if r == 0:
                # E[0][-1] = 0: clear the one-time E[-1][-1] = 1 boundary
                nc.vector.memset(Ea[:, 0:1], 0.0)
            if (r + 1) % resc == 0 and r != t_len - 1:
                mx = p_stat.tile([ni, 1], F32, tag="mx")
                nc.vector.tensor_reduce(mx[:], nxt[:, 1:t_len + 1], AX.X, OP.max)
                rec = p_stat.tile([ni, 1], F32, tag="rec")
                nc.vector.reciprocal(rec[:], mx[:])
                nc.vector.tensor_scalar_mul(nxt[:, 1:t_len + 1],
                                            nxt[:, 1:t_len + 1], rec[:])
                lg = p_stat.tile([ni, 1], F32, tag="lg")
                nc.scalar.activation(lg[:], mx[:], AF.Ln)
                nc.vector.tensor_add(cacc[:], cacc[:], lg[:])
            cur, nxt = nxt, cur

        nc.sync.dma_start(outs["EOUT"].rearrange("(a b) -> a b", b=1),
                          cur[:, t_len:t_len + 1])
        nc.sync.dma_start(outs["COUT"].rearrange("(a b) -> a b", b=1), cacc[:])


def _emit_wave(tc: tile.TileContext, ins: dict, outs: dict, kbuf: bass.AP,
               t_len: int, bpc: int, resc: int):
    """Wavefront DP: CH=t_len/128 column chunks on partition groups.

    Partition p = g*ni + inst handles column chunk g of instance inst.
    Wavefront step w: group g processes row r = w - g (K rows padded with 3
    zero rows on each side so inactive groups compute zeros). Cross-chunk
    carries (scan initial / shifted-row boundary) move between partition
    groups via a constant shift matmul on the (otherwise idle) PE.
    """
    nc = tc.nc
    ni = NTYPE * bpc
    ch = t_len // 128
    npart = ch * ni
    nrowt = ch
    nsteps = t_len + ch - 1

    with (
        tc.tile_pool(name="const", bufs=1) as p_const,
        tc.tile_pool(name="ain", bufs=2) as p_in,
        tc.tile_pool(name="astat", bufs=2) as p_astat,
        tc.tile_pool(name="asn", bufs=2) as p_asn,
        tc.tile_pool(name="apsT", bufs=2, space="PSUM") as p_psT,
        tc.tile_pool(name="ant", bufs=2) as p_nt,
        tc.tile_pool(name="aG", bufs=2, space="PSUM") as p_G,
        tc.tile_pool(name="aK", bufs=3) as p_K,
        tc.tile_pool(name="bE", bufs=1) as p_E,
        tc.tile_pool(name="bS", bufs=2) as p_s,
        tc.tile_pool(name="bK", bufs=8) as p_k,
        tc.tile_pool(name="bC", bufs=3, space="PSUM") as p_carry,
        tc.tile_pool(name="bB", bufs=1, space="PSUM") as p_bc,
        tc.tile_pool(name="bstat", bufs=2) as p_stat,
        tc.tile_pool(name="bacc", bufs=1) as p_acc,
    ):
        ident = p_const.tile([128, 128], F32, tag="ident")
        make_identity(nc, ident[:])
        bias_m1 = p_const.tile([128, 1], F32, tag="biasm1")
        nc.gpsimd.memset(bias_m1[:], -1.0)
        # shiftM[k, p] = 1 iff k == p - ni  (moves group g-1 -> g)
        shiftM = p_const.tile([npart, npart], F32, tag="shiftM")
        nc.gpsimd.memset(shiftM[:], 0.0)
        nc.gpsimd.affine_select(
            out=shiftM[:], in_=shiftM[:], compare_op=OP.not_equal, fill=1.0,
            base=ni, pattern=[[-1, npart]], channel_multiplier=1,
        )
        # bcastM[k, (g, j)] = 1 iff k == j  (broadcast group-0 col to all groups)
        bcastM = p_const.tile([ni, npart], F32, tag="bcastM")
        nc.gpsimd.memset(bcastM[:], 0.0)
        nc.gpsimd.affine_select(
            out=bcastM[:].rearrange("k (g j) -> k g j", j=ni),
            in_=bcastM[:].rearrange("k (g j) -> k g j", j=ni),
            compare_op=OP.not_equal, fill=1.0,
            base=0, pattern=[[0, ch], [-1, ni]], channel_multiplier=1,
        )

        # zero the 3+3 pad rows of kbuf (layout [ni, t_len+6, t_len])
        zpad = p_const.tile([ni, 3 * t_len], F32, tag="zpad")
        nc.gpsimd.memset(zpad[:], 0.0)
        nc.sync.dma_start(
            kbuf[:, 0:3, :].rearrange("i r c -> i (r c)"), zpad[:])
        nc.sync.dma_start(
            kbuf[:, t_len + 3:t_len + 6, :].rearrange("i r c -> i (r c)"), zpad[:])

        # ---------------- Phase A (same as v1, +3 row offset into kbuf) -----
        for b in range(bpc):
            nT = {}
            for sname in ("OTH", "TGT", "X"):
                src = ins[sname]
                xin = p_in.tile([128, nrowt * D], F32, tag=f"in_{sname}")
                nc.sync.dma_start(
                    xin[:].rearrange("p (t d) -> p t d", d=D),
                    src[b].rearrange("(t p) d -> p t d", p=128),
                )
                sq = p_astat.tile([128, nrowt * D], F32, tag=f"sq_{sname}")
                ss = p_astat.tile([128, nrowt], F32, tag=f"ss_{sname}")
                for t in range(nrowt):
                    nc.scalar.activation(
                        sq[:, t * D:(t + 1) * D], xin[:, t * D:(t + 1) * D],
                        AF.Square, accum_out=ss[:, t:t + 1],
                    )
                nrm = p_astat.tile([128, nrowt], F32, tag=f"nrm_{sname}")
                nc.scalar.activation(nrm[:], ss[:], AF.Sqrt)
                rnm = p_astat.tile([128, nrowt], F32, tag=f"rnm_{sname}")
                nc.vector.reciprocal(rnm[:], nrm[:])
                sn = p_asn.tile([128, nrowt * D], F32, tag=f"sn_{sname}")
                for t in range(nrowt):
                    nc.vector.tensor_scalar_mul(
                        sn[:, t * D:(t + 1) * D], xin[:, t * D:(t + 1) * D],
                        rnm[:, t:t + 1],
                    )
                snT = p_nt.tile([D, t_len], F32R, tag=f"nt_{sname}")
                for t in range(nrowt):
                    tp = p_psT.tile([D, 128], F32, tag="psT")
                    nc.tensor.transpose(tp[:], sn[:, t * D:(t + 1) * D], ident[:])
                    nc.scalar.copy(snT[:, t * 128:(t + 1) * 128], tp[:])
                nT[sname] = snT

            pairs = [("OTH", "X"), ("TGT", "X"), ("OTH", "OTH"), ("TGT", "TGT")]
            for ptype, (an, cn) in enumerate(pairs):
                inst = ptype * bpc + b
                aT, cT = nT[an], nT[cn]
                for rt in range(nrowt):
                    g = p_G.tile([128, t_len], F32, tag="G")
                    nc.tensor.matmul(
                        g[:], aT[:, rt * 128:(rt + 1) * 128], cT[:],
                        start=True, stop=True,
                    )
                    kt = p_K.tile([128, t_len], F32, tag="K")
                    nc.scalar.activation(kt[:], g[:], AF.Exp, bias=bias_m1[:])
                    nc.sync.dma_start(
                        kbuf[inst, 3 + rt * 128:3 + (rt + 1) * 128, :], kt[:])

        # ---------------- Phase B: wavefront row-scan -----------------------
        Ea = p_E.tile([npart, 129], F32, tag="Ea")
        Eb = p_E.tile([npart, 129], F32, tag="Eb")
        Etiles = [Ea, Eb]
        cacc = p_acc.tile([npart, 1], F32, tag="C")
        nc.gpsimd.memset(Ea[:], 0.0)
        nc.gpsimd.memset(Eb[:], 0.0)
        nc.gpsimd.memset(cacc[:], 0.0)
        nc.gpsimd.memset(Ea[0:ni, 0:1], 1.0)  # E[-1][-1] = 1 for group 0
        car_prev = p_carry.tile([npart, 1], F32, tag="car")
        car_prev_mm = nc.vector.memset(car_prev[:], 0.0)

        for w in range(nsteps):
            prev = Etiles[w % 2]
            newt = Etiles[(w + 1) % 2]
            kt = p_k.tile([npart, 128], F32, tag="krow")
            for g in range(ch):
                nc.sync.dma_start(
                    kt[g * ni:(g + 1) * ni, :],
                    kbuf[:, w - g + 3, g * 128:(g + 1) * 128],
                )
            s = p_s.tile([npart, 128], F32, tag="s")
            nc.vector.tensor_add(s[:], prev[:, 1:129], prev[:, 0:128])
            scan_i = nc.vector.tensor_tensor_scan(
                newt[:, 1:129], s[:], kt[:], car_prev[:, 0:1],
                OP.add, OP.mult,
            )
            add_dep_helper(scan_i.ins, car_prev_mm.ins,
                           reason="scan initial after PE carry shift")
            if (w + 1) % resc == 0 and w + 1 < t_len:
                # per-partition chunk max -> per-instance max across groups
                pmax = p_stat.tile([npart, 1], F32, tag="pmax")
                nc.vector.tensor_reduce(pmax[:], newt[:, 1:129], AX.X, OP.max)
                pmT = p_bc.tile([1, npart], F32, tag="bc")
                t1 = nc.tensor.transpose(pmT[:], pmax[:],
                                         ident[0:npart, 0:npart])
                mxrow = p_stat.tile([1, ni], F32, tag="mxrow")
                rd2 = nc.vector.tensor_reduce(
                    mxrow[:], pmT[:].rearrange("a (g i) -> a i g", i=ni),
                    AX.X, OP.max)
                add_dep_helper(rd2.ins, t1.ins, reason="reduce after PE T1")
                mxps = p_bc.tile([ni, 1], F32, tag="bc")
                t2 = nc.tensor.transpose(mxps[:], mxrow[:], ident[0:1, 0:1])
                mxcol = p_stat.tile([ni, 1], F32, tag="mxcol")
                cpm = nc.scalar.copy(mxcol[:], mxps[:])
                add_dep_helper(cpm.ins, t2.ins, reason="copy after PE T2")
                bc = p_bc.tile([npart, 1], F32, tag="bc")
                bc_mm = nc.tensor.matmul(bc[:], bcastM[:], mxcol[:],
                                         start=True, stop=True)
                rec = p_stat.tile([npart, 1], F32, tag="rec")
                rcp = nc.vector.reciprocal(rec[:], bc[:])
                add_dep_helper(rcp.ins, bc_mm.ins,
                               reason="recip after PE broadcast")
                nc.vector.tensor_scalar_mul(newt[:, 0:129], newt[:, 0:129], rec[:])
                lgr = p_stat.tile([npart, 1], F32, tag="lgr")
                nc.scalar.activation(lgr[:], rec[:], AF.Ln)
                nc.vector.tensor_sub(cacc[:], cacc[:], lgr[:])
            car = p_carry.tile([npart, 1], F32, tag="car")
            car_mm = nc.tensor.matmul(car[:], shiftM[:], newt[:, 128:129],
                                      start=True, stop=True)
            cp = nc.scalar.copy(prev[:, 0:1], car[:])
            add_dep_helper(cp.ins, car_mm.ins,
                           reason="carry copy after PE shift")
            car_prev = car
            car_prev_mm = car_mm

        last = Etiles[nsteps % 2]
        nc.sync.dma_start(outs["EOUT"].rearrange("(a b) -> a b", b=1),
                          last[(ch - 1) * ni:ch * ni, 128:129])
        nc.sync.dma_start(outs["COUT"].rearrange("(a b) -> a b", b=1),
                          cacc[(ch - 1) * ni:ch * ni, 0:1])


def _emit_wave2(tc: tile.TileContext, ins: dict, outs: dict, kbuf: bass.AP,
                t_len: int, bpc: int, lag: int, sb: int):
    """Wavefront DP v2: lagged chunks + batched kt DMA + pipelined rescale.

    Group g processes row w - g*lag at step w (lag>=2 gives the PE carry
    shift and ACT boundary copy slack off the DVE critical path).  kt rows
    for `sb` consecutive steps are fetched in ONE diagonal-AP DMA.  The
    rescale max is computed 8 steps before it is applied, so its reduce/
    transpose/broadcast chain also runs off the critical path.
    """
    nc = tc.nc
    ni = NTYPE * bpc
    ch = t_len // 128
    npart = ch * ni
    nrowt = ch
    pad = (ch - 1) * lag
    nsteps = t_len + pad
    nbatch = (nsteps + sb - 1) // sb
    krows = t_len + 2 * pad + sb  # top pad + rows + bottom pad (incl DMA overrun)

    with (
        tc.tile_pool(name="const", bufs=1) as p_const,
    ):
        ident = p_const.tile([128, 128], F32, tag="ident")
        make_identity(nc, ident[:])
        bias_m1 = p_const.tile([128, 1], F32, tag="biasm1")
        nc.gpsimd.memset(bias_m1[:], -1.0)
        # Group g lives on partition block (ch-1-g); carries move to the next
        # block via a partition-shift DMA (no PE involvement).
        # bcastM[k, (g, j)] = 1 iff k == j  (broadcast per-inst col to all groups)
        bcastM = p_const.tile([ni, npart], F32, tag="bcastM")
        nc.gpsimd.memset(bcastM[:], 0.0)
        nc.gpsimd.affine_select(
            out=bcastM[:].rearrange("k (g j) -> k g j", j=ni),
            in_=bcastM[:].rearrange("k (g j) -> k g j", j=ni),
            compare_op=OP.not_equal, fill=1.0,
            base=0, pattern=[[0, ch], [-1, ni]], channel_multiplier=1,
        )

        # zero the pad rows of kbuf (layout [ni, krows, t_len])
        nbot = krows - t_len - pad
        zpad = p_const.tile([ni, nbot * 512], F32, tag="zpad")
        nc.gpsimd.memset(zpad[:], 0.0)
        nc.sync.dma_start(
            kbuf[:, 0:pad, :].rearrange("i r c -> i (r c)"),
            zpad[:, 0:pad * 512])
        nc.sync.dma_start(
            kbuf[:, t_len + pad:krows, :].rearrange("i r c -> i (r c)"),
            zpad[:])

        # ---------------- Phase A (as v1, +pad row offset into kbuf) --------
        with (
            tc.tile_pool(name="ain", bufs=2) as p_in,
            tc.tile_pool(name="astat", bufs=2) as p_astat,
            tc.tile_pool(name="asn", bufs=2) as p_asn,
            tc.tile_pool(name="apsT", bufs=2, space="PSUM") as p_psT,
            tc.tile_pool(name="ant", bufs=2) as p_nt,
            tc.tile_pool(name="aG", bufs=2, space="PSUM") as p_G,
            tc.tile_pool(name="aK", bufs=3) as p_K,
        ):
            _emit_phaseA(tc, ins, kbuf, t_len, bpc, pad,
                         p_in, p_astat, p_asn, p_psT, p_nt, p_G, p_K,
                         ident, bias_m1)

        # ---------------- Phase B: lagged wavefront row-scan ----------------
        with (
            tc.tile_pool(name="bE", bufs=1) as p_E,
            tc.tile_pool(name="bS", bufs=2) as p_s,
            tc.tile_pool(name="bK", bufs=4) as p_k,
            tc.tile_pool(name="bB", bufs=1, space="PSUM") as p_bc,
            tc.tile_pool(name="bstat", bufs=4) as p_stat,
            tc.tile_pool(name="bacc", bufs=1) as p_acc,
        ):
            _emit_phaseB(tc, outs, kbuf, t_len, bpc, lag, sb,
                         p_E, p_s, p_k, p_bc, p_stat, p_acc,
                         ident, bcastM)


def _emit_phaseA(tc, ins, kbuf, t_len, bpc, pad,
                 p_in, p_astat, p_asn, p_psT, p_nt, p_G, p_K,
                 ident, bias_m1):
    nc = tc.nc
    nrowt = t_len // 128
    if True:
        for b in range(bpc):
            nT = {}
            for sname in ("OTH", "TGT", "X"):
                src = ins[sname]
                xin = p_in.tile([128, nrowt * D], F32, tag=f"in_{sname}")
                nc.sync.dma_start(
                    xin[:].rearrange("p (t d) -> p t d", d=D),
                    src[b].rearrange("(t p) d -> p t d", p=128),
                )
                sq = p_astat.tile([128, nrowt * D], F32, tag=f"sq_{sname}")
                ss = p_astat.tile([128, nrowt], F32, tag=f"ss_{sname}")
                for t in range(nrowt):
                    nc.scalar.activation(
                        sq[:, t * D:(t + 1) * D], xin[:, t * D:(t + 1) * D],
                        AF.Square, accum_out=ss[:, t:t + 1],
                    )
                nrm = p_astat.tile([128, nrowt], F32, tag=f"nrm_{sname}")
                nc.scalar.activation(nrm[:], ss[:], AF.Sqrt)
                rnm = p_astat.tile([128, nrowt], F32, tag=f"rnm_{sname}")
                nc.vector.reciprocal(rnm[:], nrm[:])
                sn = p_asn.tile([128, nrowt * D], F32, tag=f"sn_{sname}")
                for t in range(nrowt):
                    nc.vector.tensor_scalar_mul(
                        sn[:, t * D:(t + 1) * D], xin[:, t * D:(t + 1) * D],
                        rnm[:, t:t + 1],
                    )
                snT = p_nt.tile([D, t_len], F32R, tag=f"nt_{sname}")
                for t in range(nrowt):
                    tp = p_psT.tile([D, 128], F32, tag="psT")
                    nc.tensor.transpose(tp[:], sn[:, t * D:(t + 1) * D], ident[:])
                    nc.scalar.copy(snT[:, t * 128:(t + 1) * 128], tp[:])
                nT[sname] = snT

            pairs = [("OTH", "X"), ("TGT", "X"), ("OTH", "OTH"), ("TGT", "TGT")]
            for ptype, (an, cn) in enumerate(pairs):
                inst = ptype * bpc + b
                aT, cT = nT[an], nT[cn]
                for rt in range(nrowt):
                    g = p_G.tile([128, t_len], F32, tag="G")
                    nc.tensor.matmul(
                        g[:], aT[:, rt * 128:(rt + 1) * 128], cT[:],
                        start=True, stop=True,
                    )
                    kt = p_K.tile([128, t_len], F32, tag="K")
                    nc.scalar.activation(kt[:], g[:], AF.Exp, bias=bias_m1[:])
                    nc.sync.dma_start(
                        kbuf[inst, pad + rt * 128:pad + (rt + 1) * 128, :], kt[:])


def _emit_phaseB(tc, outs, kbuf, t_len, bpc, lag, sb,
                 p_E, p_s, p_k, p_bc, p_stat, p_acc,
                 ident, bcastM):
    nc = tc.nc
    ni = NTYPE * bpc
    ch = t_len // 128
    npart = ch * ni
    pad = (ch - 1) * lag
    nsteps = t_len + pad
    nbatch = (nsteps + sb - 1) // sb
    krows = t_len + 2 * pad + sb
    if True:
        NE = 4
        Etiles = []
        for j in range(NE):
            Ej = p_E.tile([npart, 129], F32, tag=f"E{j}")
            Etiles.append(Ej)
        cacc = p_acc.tile([npart, 1], F32, tag="C")
        for E in Etiles:
            nc.gpsimd.memset(E[:], 0.0)
        nc.gpsimd.memset(cacc[:], 0.0)
        # E[-1][-1] = 1 for group 0 (= partition block ch-1)
        nc.gpsimd.memset(Etiles[0][(ch - 1) * ni:ch * ni, 0:1], 1.0)

        ktbs = {}
        rec_pending = None  # (rec_tile,) scheduled for the next apply step

        def fetch_batch(bi):
            w0 = bi * sb
            ktb = p_k.tile([npart, sb * 128], F32, tag="ktb")
            # ktb[(ch-1-g)*ni + i, s*128 + c] = kbuf[i, w0+s-g*lag+pad, g*128+c]
            for g in range(ch):
                blk = ch - 1 - g
                src = kbuf.copy()
                src.ap = type(src.ap)([
                    [krows * t_len, ni],        # i
                    [t_len, sb],                # s (step within batch)
                    [1, 128],                   # c
                ])
                src.offset = (w0 - g * lag + pad) * t_len + g * 128
                nc.sync.dma_start(
                    ktb[blk * ni:(blk + 1) * ni, :]
                    .rearrange("i (s c) -> i s c", c=128), src)
            ktbs[bi] = ktb

        fetch_batch(0)
        fetch_batch(1)

        for w in range(nsteps):
            cur = Etiles[w % NE]      # rows w-1-g*lag (prev), written by scan w-1
            newt = Etiles[(w + 1) % NE]
            if w % sb == 0 and (w // sb) + 2 < nbatch:
                fetch_batch(w // sb + 2)

            # pipelined rescale: apply scale computed 8 steps ago
            if rec_pending is not None and w % RESC == 0:
                rec, = rec_pending
                rec_pending = None
                nc.vector.tensor_scalar_mul(cur[:, 0:129], cur[:, 0:129], rec[:])
                # boundary DMAs for steps w..w+lag-1 were issued pre-scale:
                # rescale their landing zones (col 0 of the dst tiles)
                for t in range(w, w + lag):
                    dst = Etiles[(t + 1) % NE]
                    nc.vector.tensor_scalar_mul(
                        dst[0:(ch - 1) * ni, 0:1], dst[0:(ch - 1) * ni, 0:1],
                        rec[0:(ch - 1) * ni])
                lgr = p_stat.tile([npart, 1], F32, tag="lgr")
                nc.scalar.activation(lgr[:], rec[:], AF.Ln)
                nc.vector.tensor_sub(cacc[:], cacc[:], lgr[:])

            s = p_s.tile([npart, 128], F32, tag="s")
            nc.vector.tensor_add(s[:], cur[:, 1:129], cur[:, 0:128])
            ktb = ktbs[w // sb]
            nc.vector.tensor_tensor_scan(
                newt[:, 1:129], s[:], ktb[:, (w % sb) * 128:(w % sb + 1) * 128],
                newt[:, 0:1], OP.add, OP.mult,
            )
            if w == 0:
                # clear the one-time E[-1][-1] = 1 seed (group 0 boundary is 0)
                nc.vector.memset(Etiles[0][(ch - 1) * ni:ch * ni, 0:1], 0.0)

            # boundary for step w+lag: E tile col 0 gets group g-1's scan
            # output boundary (partition shift by +ni) via SWDGE DMA
            if w + lag < nsteps:
                nc.gpsimd.dma_start(
                    Etiles[(w + lag + 1) % NE][0:(ch - 1) * ni, 0:1],
                    newt[ni:npart, 128:129])

            # pipelined rescale: compute scale from this step's rows
            if (w + 8) % RESC == 0 and (w + 8) <= 480:
                pmax = p_stat.tile([npart, 1], F32, tag="pmax")
                nc.vector.tensor_reduce(pmax[:], newt[:, 1:129], AX.X, OP.max)
                pmT = p_bc.tile([1, npart], F32, tag="bc")
                t1 = nc.tensor.transpose(pmT[:], pmax[:],
                                         ident[0:npart, 0:npart])
                mxrow = p_stat.tile([1, ni], F32, tag="mxrow")
                rd2 = nc.vector.tensor_reduce(
                    mxrow[:], pmT[:].rearrange("a (g i) -> a i g", i=ni),
                    AX.X, OP.max)
                add_dep_helper(rd2.ins, t1.ins, reason="reduce after PE T1")
                mxps = p_bc.tile([ni, 1], F32, tag="bc")
                t2 = nc.tensor.transpose(mxps[:], mxrow[:], ident[0:1, 0:1])
                mxcol = p_stat.tile([ni, 1], F32, tag="mxcol")
                cpm = nc.scalar.copy(mxcol[:], mxps[:])
                add_dep_helper(cpm.ins, t2.ins, reason="copy after PE T2")
                bc = p_bc.tile([npart, 1], F32, tag="bc")
                bc_mm = nc.tensor.matmul(bc[:], bcastM[:], mxcol[:],
                                         start=True, stop=True)
                rec = p_stat.tile([npart, 1], F32, tag="rec")
                rcp = nc.vector.reciprocal(rec[:], bc[:])
                add_dep_helper(rcp.ins, bc_mm.ins,
                               reason="recip after PE broadcast")
                rec_pending = (rec,)

        # group ch-1 (final column chunk) lives on partition block 0
        last = Etiles[nsteps % 2]
        nc.sync.dma_start(outs["EOUT"].rearrange("(a b) -> a b", b=1),
                          last[0:ni, 128:129])
        nc.sync.dma_start(outs["COUT"].rearrange("(a b) -> a b", b=1),
                          cacc[0:ni, 0:1])


def _emit_wave3(tc: tile.TileContext, ins: dict, outs: dict, kbs: list,
                t_len: int, bpc: int, lag: int, sb: int):
    """Wavefront DP v3: phase A (K production) overlapped under phase B.

    kbs: 4 per-row-tile DRAM tensors (row-tile granular dep tracking).
    kbs[0] holds global rows [-pad, 128) at local r+pad; kbs[1]/[2] rows
    [128,256)/[256,384); kbs[3] rows [384, 512+pad+sb) incl bottom pad.
    Carries move between column-chunk groups via partition-shift DMAs on
    the gpsimd (SWDGE) queue; NE=8 E-tile rotation gives them slack.
    """
    nc = tc.nc
    ni = NTYPE * bpc
    ch = t_len // 128
    npart = ch * ni
    nrowt = ch
    pad = (ch - 1) * lag
    nsteps = t_len + pad
    nbatch = (nsteps + sb - 1) // sb
    NE = 8
    rt_base = [-pad, 128, 256, 384]
    rt_rows = [128 + pad, 128, 128, 128 + pad + sb]

    def rt_of(r):
        return max(0, min(3, r // 128))

    with (
        tc.tile_pool(name="const", bufs=1) as p_const,
        tc.tile_pool(name="ant", bufs=1) as p_nt,
        tc.tile_pool(name="aG", bufs=2, space="PSUM") as p_G,
        tc.tile_pool(name="aK", bufs=3) as p_K,
        tc.tile_pool(name="bE", bufs=1) as p_E,
        tc.tile_pool(name="bS", bufs=2) as p_s,
        tc.tile_pool(name="bK", bufs=4) as p_k,
        tc.tile_pool(name="bB", bufs=1, space="PSUM") as p_bc,
        tc.tile_pool(name="bstat", bufs=4) as p_stat,
        tc.tile_pool(name="bacc", bufs=1) as p_acc,
    ):
        ident = p_const.tile([128, 128], F32, tag="ident")
        make_identity(nc, ident[:])
        bias_m1 = p_const.tile([128, 1], F32, tag="biasm1")
        nc.gpsimd.memset(bias_m1[:], -1.0)
        bcastM = p_const.tile([ni, npart], F32, tag="bcastM")
        nc.gpsimd.memset(bcastM[:], 0.0)
        nc.gpsimd.affine_select(
            out=bcastM[:].rearrange("k (g j) -> k g j", j=ni),
            in_=bcastM[:].rearrange("k (g j) -> k g j", j=ni),
            compare_op=OP.not_equal, fill=1.0,
            base=0, pattern=[[0, ch], [-1, ni]], channel_multiplier=1,
        )

        # zero pads: top of kbs[0] (pad rows), bottom of kbs[3] (pad+sb rows)
        zp = p_const.tile([ni, 4 * t_len], F32, tag="zp")
        nc.gpsimd.memset(zp[:], 0.0)
        for lo in range(0, pad, 4):
            n = min(4, pad - lo)
            nc.sync.dma_start(
                kbs[0][:, lo:lo + n, :].rearrange("i r c -> i (r c)"),
                zp[:, 0:n * t_len])
        for lo in range(128, 128 + pad + sb, 4):
            n = min(4, 128 + pad + sb - lo)
            nc.sync.dma_start(
                kbs[3][:, lo:lo + n, :].rearrange("i r c -> i (r c)"),
                zp[:, 0:n * t_len])

        pairs = [("OTH", "X"), ("TGT", "X"), ("OTH", "OTH"), ("TGT", "TGT")]
        nts = {}

        def produce_item(rt, b):
            loc = rt * 128 - rt_base[rt]
            for ptype, (an, cn) in enumerate(pairs):
                inst = ptype * bpc + b
                aT, cT = nts[(b, an)], nts[(b, cn)]
                g = p_G.tile([128, t_len], F32, tag="G")
                nc.tensor.matmul(
                    g[:], aT[:, rt * 128:(rt + 1) * 128], cT[:],
                    start=True, stop=True,
                )
                kt = p_K.tile([128, t_len], F32, tag="K")
                nc.scalar.activation(kt[:], g[:], AF.Exp, bias=bias_m1[:])
                nc.scalar.dma_start(kbs[rt][inst, loc:loc + 128, :], kt[:])

        def produce(rt):
            for b in range(bpc):
                produce_item(rt, b)

        # ---- preamble: normalized+transposed sequences for all items ------
        with (
            tc.tile_pool(name="ain", bufs=2) as p_in,
            tc.tile_pool(name="astat", bufs=2) as p_astat,
            tc.tile_pool(name="asn", bufs=2) as p_asn,
            tc.tile_pool(name="apsT", bufs=2, space="PSUM") as p_psT,
        ):
            for b in range(bpc):
                for sname in ("OTH", "TGT", "X"):
                    src = ins[sname]
                    xin = p_in.tile([128, nrowt * D], F32, tag=f"in_{sname}")
                    nc.sync.dma_start(
                        xin[:].rearrange("p (t d) -> p t d", d=D),
                        src[b].rearrange("(t p) d -> p t d", p=128),
                    )
                    sq = p_astat.tile([128, nrowt * D], F32, tag=f"sq_{sname}")
                    ss = p_astat.tile([128, nrowt], F32, tag=f"ss_{sname}")
                    for t in range(nrowt):
                        nc.scalar.activation(
                            sq[:, t * D:(t + 1) * D], xin[:, t * D:(t + 1) * D],
                            AF.Square, accum_out=ss[:, t:t + 1],
                        )
                    nrm = p_astat.tile([128, nrowt], F32, tag=f"nrm_{sname}")
                    nc.scalar.activation(nrm[:], ss[:], AF.Sqrt)
                    rnm = p_astat.tile([128, nrowt], F32, tag=f"rnm_{sname}")
                    nc.vector.reciprocal(rnm[:], nrm[:])
                    sn = p_asn.tile([128, nrowt * D], F32, tag=f"sn_{sname}")
                    for t in range(nrowt):
                        nc.vector.tensor_scalar_mul(
                            sn[:, t * D:(t + 1) * D], xin[:, t * D:(t + 1) * D],
                            rnm[:, t:t + 1],
                        )
                    snT = p_nt.tile([D, t_len], F32R, tag=f"nt_{b}_{sname}")
                    for t in range(nrowt):
                        tp = p_psT.tile([D, 128], F32, tag="psT")
                        nc.tensor.transpose(tp[:], sn[:, t * D:(t + 1) * D],
                                            ident[:])
                        nc.vector.tensor_copy(snT[:, t * 128:(t + 1) * 128],
                                              tp[:])
                    nts[(b, sname)] = snT
                produce_item(0, b)

        for b in range(bpc):
            produce_item(1, b)

        # ---------------- Phase B ------------------------------------------
        Etiles = []
        for j in range(NE):
            Ej = p_E.tile([npart, 129], F32, tag=f"E{j}")
            Etiles.append(Ej)
        cacc = p_acc.tile([npart, 1], F32, tag="C")
        for E in Etiles:
            nc.gpsimd.memset(E[:], 0.0)
        nc.gpsimd.memset(cacc[:], 0.0)
        nc.gpsimd.memset(Etiles[0][(ch - 1) * ni:ch * ni, 0:1], 1.0)

        ktbs = {}
        rec_pending = None

        def fetch_batch(bi):
            w0 = bi * sb
            ktb = p_k.tile([npart, sb * 128], F32, tag="ktb")
            # ktb[(ch-1-g)*ni+i, s*128+c] = K[i][row w0+s-g*lag][g*128+c]
            for g in range(ch):
                blk = ch - 1 - g
                r_lo = w0 - g * lag
                s = 0
                while s < sb:
                    r = r_lo + s
                    rt = rt_of(r)
                    krt = kbs[rt]
                    n = min(sb - s, rt_base[rt] + rt_rows[rt] - r)
                    src = krt.copy()
                    src.ap = type(src.ap)([
                        [rt_rows[rt] * t_len, ni],
                        [t_len, n],
                        [1, 128],
                    ])
                    src.offset = (r - rt_base[rt]) * t_len + g * 128
                    nc.sync.dma_start(
                        ktb[blk * ni:(blk + 1) * ni, s * 128:(s + n) * 128]
                        .rearrange("i (s c) -> i s c", c=128), src)
                    s += n
            ktbs[bi] = ktb

        fetch_batch(0)
        fetch_batch(1)

        fixups = {}
        for w in range(nsteps):
            cur = Etiles[w % NE]
            newt = Etiles[(w + 1) % NE]
            if w % sb == 0 and (w // sb) + 2 < nbatch:
                fetch_batch(w // sb + 2)
            if w == 40:
                produce(2)
            if w == 160:
                produce(3)

            apply_rec = None
            if rec_pending is not None and w % RESC == 0:
                apply_rec, = rec_pending
                rec_pending = None
                # rescale the state via the s tile (avoids a WAR stall with
                # the in-flight boundary DMA that reads cur's col 128); the
                # boundary values DMA'd from pre-scale rows get fixed up at
                # the step that consumes them (fixups dict).
                for t in range(w, w + lag):
                    fixups[t] = apply_rec
                lgr = p_stat.tile([npart, 1], F32, tag="lgr")
                nc.scalar.activation(lgr[:], apply_rec[:], AF.Ln)
                nc.vector.tensor_sub(cacc[:], cacc[:], lgr[:])

            s = p_s.tile([npart, 128], F32, tag="s")
            nc.vector.tensor_add(s[:], cur[:, 1:129], cur[:, 0:128])
            if apply_rec is not None:
                nc.vector.tensor_scalar_mul(s[:], s[:], apply_rec[:])
            fx = fixups.pop(w, None)
            if fx is not None:
                nc.vector.tensor_scalar_mul(
                    newt[0:(ch - 1) * ni, 0:1], newt[0:(ch - 1) * ni, 0:1],
                    fx[0:(ch - 1) * ni])
            ktb = ktbs[w // sb]
            nc.vector.tensor_tensor_scan(
                newt[:, 1:129], s[:], ktb[:, (w % sb) * 128:(w % sb + 1) * 128],
                newt[:, 0:1], OP.add, OP.mult,
            )
            if w == 0:
                nc.vector.memset(Etiles[0][(ch - 1) * ni:ch * ni, 0:1], 0.0)

            if w + lag < nsteps:
                nc.gpsimd.dma_start(
                    Etiles[(w + lag + 1) % NE][0:(ch - 1) * ni, 0:1],
                    newt[ni:npart, 128:129])

            if (w + 8) % RESC == 0 and (w + 8) <= 480:
                pmax = p_stat.tile([npart, 1], F32, tag="pmax")
                nc.vector.tensor_reduce(pmax[:], newt[:, 1:129], AX.X, OP.max)
                pmT = p_bc.tile([1, npart], F32, tag="bc")
                t1 = nc.tensor.transpose(pmT[:], pmax[:],
                                         ident[0:npart, 0:npart])
                mxrow = p_stat.tile([1, ni], F32, tag="mxrow")
                rd2 = nc.vector.tensor_reduce(
                    mxrow[:], pmT[:].rearrange("a (g i) -> a i g", i=ni),
                    AX.X, OP.max)
                add_dep_helper(rd2.ins, t1.ins, reason="reduce after PE T1")
                mxps = p_bc.tile([ni, 1], F32, tag="bc")
                t2 = nc.tensor.transpose(mxps[:], mxrow[:], ident[0:1, 0:1])
                mxcol = p_stat.tile([ni, 1], F32, tag="mxcol")
                cpm = nc.scalar.copy(mxcol[:], mxps[:])
                add_dep_helper(cpm.ins, t2.ins, reason="copy after PE T2")
                bc = p_bc.tile([npart, 1], F32, tag="bc")
                bc_mm = nc.tensor.matmul(bc[:], bcastM[:], mxcol[:],
                                         start=True, stop=True)
                rec = p_stat.tile([npart, 1], F32, tag="rec")
                rcp = nc.vector.reciprocal(rec[:], bc[:])
                add_dep_helper(rcp.ins, bc_mm.ins,
                               reason="recip after PE broadcast")
                rec_pending = (rec,)

        last = Etiles[nsteps % NE]
        nc.sync.dma_start(outs["EOUT"].rearrange("(a b) -> a b", b=1),
                          last[0:ni, 128:129])
        nc.sync.dma_start(outs["COUT"].rearrange("(a b) -> a b", b=1),
                          cacc[0:ni, 0:1])


def _build(t_len=T, bpc=BPC, resc=RESC, num_devices=NCORES, wave=False,
           wave2=False, wave3=False, lag=3, sb=8):
    ni = NTYPE * bpc
    nc = bacc.Bacc(
        "TRN2", target_bir_lowering=False, debug=False, num_devices=num_devices,
    )
    ins = {
        name: nc.dram_tensor(name, [bpc, t_len, D], F32, kind="ExternalInput").ap()
        for name in ("TGT", "OTH", "X")
    }
    outs = {
        "EOUT": nc.dram_tensor("EOUT", [ni], F32, kind="ExternalOutput").ap(),
        "COUT": nc.dram_tensor("COUT", [ni], F32, kind="ExternalOutput").ap(),
    }
    if wave3:
        lag = 5
        sb = int(os.environ.get('KSB', '16'))
        pad = (t_len // 128 - 1) * lag
        rt_rows = [128 + pad, 128, 128, 128 + pad + sb]
        kbs = [
            nc.dram_tensor(f"KBUF{i}", [ni, rt_rows[i], t_len], F32).ap()
            for i in range(4)
        ]
        with tile.TileContext(nc) as tc:
            _emit_wave3(tc, ins, outs, kbs, t_len, bpc, lag, sb)
    elif wave2:
        ch = t_len // 128
        pad = (ch - 1) * lag
        krows = t_len + 2 * pad + sb
        kbuf = nc.dram_tensor("KBUF", [ni, krows, t_len], F32).ap()
        with tile.TileContext(nc) as tc:
            _emit_wave2(tc, ins, outs, kbuf, t_len, bpc, lag, sb)
    elif wave:
        kbuf = nc.dram_tensor("KBUF", [ni, t_len + 6, t_len], F32).ap()
        with tile.TileContext(nc) as tc:
            _emit_wave(tc, ins, outs, kbuf, t_len, bpc, resc)
    else:
        kbuf = nc.dram_tensor("KBUF", [ni, t_len, t_len], F32).ap()
        with tile.TileContext(nc) as tc:
            _emit(tc, ins, outs, kbuf, t_len, bpc, resc)
    nc.compile()
    return nc


_NC = None


def _get_nc():
    global _NC
    if _NC is None:
        kv = os.environ.get("KWAVE", "0")
        _NC = _build(wave=kv == "1", wave2=kv == "2", wave3=kv == "3")
    return _NC


def _postprocess(results, labels):
    E = np.stack([r["EOUT"] for r in results])  # [8, 32]
    C = np.stack([r["COUT"] for r in results])  # [8, 32]
    R = -(np.log(E) + C)                        # [core, type*8+b]
    R = R.reshape(NCORES, NTYPE, BPC).transpose(1, 0, 2).reshape(NTYPE, B)
    diff = (R[0] - R[1] - 0.5 * R[2] + 0.5 * R[3]).astype(np.float32)
    lab = np.asarray(labels, dtype=np.float32)
    return np.float32(np.mean((diff - lab) ** 2, dtype=np.float32))


def kernel(TGT, OTH, X, labels):
    nc = _get_nc()
    TGT = np.ascontiguousarray(np.asarray(TGT, dtype=np.float32))
    OTH = np.ascontiguousarray(np.asarray(OTH, dtype=np.float32))
    X = np.ascontiguousarray(np.asarray(X, dtype=np.float32))
    in_maps = [
        {
            "TGT": TGT[c * BPC:(c + 1) * BPC],
            "OTH": OTH[c * BPC:(c + 1) * BPC],
            "X": X[c * BPC:(c + 1) * BPC],
        }
        for c in range(NCORES)
    ]
    res = run_bass_kernel_spmd(nc, in_maps, core_ids=list(range(NCORES)))
    return _postprocess(res.results, labels)

